# revision 16
# baseline (speedup 1.0000x reference)
"""Swin-style window-attention encoder as a Bass/Tile kernel for TRN2 — v3.

Key design vs v1:
- Residual master X lives in SBUF as FP32 [128, 4, T+16] (channel-major) —
  the residual stream never rounds to bf16 (bf16 master measured 2.2e-2 max
  rel err, over the 2e-2 budget; f32 master 7.8e-3). Matmul inputs are
  bf16 copies cast on the (otherwise idle) GPSIMD engine; per-token mean
  stats contract the f32 master directly (f32 matmul, tiny N).
- Weights are baked into the NEFF as inline consts — per-launch IO is just
  x (bf16 in) and out (bf16).
- Per-layer phase batching: [attention (Exp table)] -> [LN1 rows (Sqrt) +
  apply] -> [FFN (Relu, no table load)] -> [LN2 rows + apply]. 2 activation
  table loads per layer instead of ~64.
- Attention softmax denominators: collected per head into smat rows (act
  Copy), broadcast to 128 partitions via one e2 matmul, ONE fat [128,144]
  DVE reciprocal per head-pair (no 1-lane recips).
- LN row math on [128,36] shuffled layout (SBUF->SBUF strided DMA), not
  1-lane [1,T] ops.
- O-proj / QK-proj / stats at window-pair (288 tokens) granularity; FFN at
  512-token chunks.
"""
from contextlib import ExitStack

import numpy as np
import ml_dtypes

import concourse.bass as bass
import concourse.bacc as bacc
import concourse.tile as tile
import concourse.mybir as mybir

F32 = mybir.dt.float32
F32R = mybir.dt.float32r
BF16 = mybir.dt.bfloat16
AF = mybir.ActivationFunctionType

WS = 12
N = WS * WS          # 144 tokens per window
C = 512
NH = 8
HD = 64
FF = 2048
EPS = 1e-5


def _insdim_ap(row_ap, stride, num, at=1):
    """Insert a dim of (stride, num) at position `at` of the AP (default:
    right after the partition dim). stride=0 -> broadcast; else gather."""
    dims = [list(d) for d in row_ap.ap]
    return bass.AP(
        tensor=row_ap.tensor,
        offset=row_ap.offset,
        ap=dims[:at] + [[stride, num]] + dims[at:],
    )


def _bcast_ap(row_ap, parts):
    return _insdim_ap(row_ap, 0, parts)


def build(nc: bass.Bass, NW: int, NL: int, w: dict,
          skip_attn=False, skip_ffn=False, skip_heads=False,
          pb=(4, 4), winb=2, epb=3, sqb=1, hbb=1, scb=2, la=3, xbb=2,
          trivial_gb=False, trivial_bias=False):
    """w: packed numpy weight dict (see pack_weights)."""
    T = NW * N
    PAIRS = NW // 2
    NCH = T // 512
    assert T % 512 == 0

    d = {}
    d["x"] = nc.dram_tensor("x", [128, 4, T], BF16, kind="ExternalInput").ap()
    d["out"] = nc.dram_tensor("out", [128, 4, T], BF16, kind="ExternalOutput").ap()
    cst = {nm: nc.inline_tensor(arr, name=nm).ap() for nm, arr in w.items()}

    with tile.TileContext(nc) as tc, ExitStack() as ctx:
        P = lambda name, bufs, **kw: ctx.enter_context(
            tc.tile_pool(name=name, bufs=bufs, **kw)
        )
        xp = P("xmaster", 1)
        cons = P("consts", 1)
        wpA = P("wtsA", 1)     # attention-phase weights
        wpF = P("wtsF", 1)     # ffn-phase weights
        winp = P("win", winb)  # per-pair working tiles
        ep = P("eptiles", epb)  # P tiles
        etp = P("ettiles", 2)   # exp tiles (short-lived)
        sqp = P("sqtiles", sqb)  # squared-x tiles for stats
        scp = P("sctiles", scb)  # recip rows [128,144]
        rowp = P("rows", 2)    # LN stat rows (short-lived, per region)
        hp = P("hbuf", hbb)
        xbp = P("xbcast", 1)
        xb2p = P("xb2cast", 1)
        psmm = P("psmm", pb[0], space="PSUM")
        psaux = P("psaux", pb[1], space="PSUM")

        # ---- persistent tiles ----
        X = xp.tile([128, 4, T + 16], F32, tag="X")
        for tq in range(NCH):
            xin = sqp.tile([128, 4, 512], BF16, tag="xsq2")
            nc.sync.dma_start(out=xin,
                              in_=d["x"][:, :, tq * 512:(tq + 1) * 512])
            nc.vector.tensor_copy(out=X[:, :, tq * 512:(tq + 1) * 512], in_=xin)
        ones = cons.tile([128, 1], BF16, tag="ones")       # value 1/512
        nc.sync.dma_start(out=ones, in_=cst["c_ones"])
        ones1 = cons.tile([1, 128], BF16, tag="ones1")     # bcast lhsT (1.0)
        nc.sync.dma_start(out=ones1, in_=cst["c_ones1"])
        e2 = cons.tile([64, 128], BF16, tag="e2")
        nc.sync.dma_start(out=e2, in_=cst["c_e2"])
        eps128 = cons.tile([128, 1], F32, tag="eps128")
        nc.vector.memset(eps128, EPS)
        onesf = cons.tile([128, 1], F32, tag="onesf")
        nc.vector.memset(onesf, 1.0 / 512.0)
        smats = [cons.tile([64, 144], BF16, tag=f"smat{i}", name=f"smat{i}")
                 for i in range(8)]
        for t in smats:
            nc.vector.memset(t, 0.0)

        for l in range(NL):
            # layer weights (attention set + rows)
            wq = wpA.tile([128, 4, 512], BF16, tag="wq")
            wk = wpA.tile([128, 4, 512], BF16, tag="wk")
            wv = wpA.tile([128, 4, 512], BF16, tag="wv")
            wo = wpA.tile([128, 4, 512], BF16, tag="wo")
            eb = wpA.tile([128, NH, 288], BF16, tag="expb")
            bq = wpA.tile([128, 4], F32, tag="bq")
            bk = wpA.tile([128, 4], F32, tag="bk")
            bo = wpA.tile([128, 4], F32, tag="bo")
            bv = wpA.tile([128, 512], BF16, tag="bvb")
            g1 = wpA.tile([128, 4], F32, tag="g1")
            b1 = wpA.tile([128, 4], F32, tag="b1")
            g2 = wpA.tile([128, 4], F32, tag="g2")
            b2 = wpA.tile([128, 4], F32, tag="b2")
            for nm, t in (("wq", wq), ("wk", wk), ("wv", wv), ("wo", wo),
                          ("expb", eb), ("bq", bq), ("bk", bk), ("bo_c", bo),
                          ("bvb", bv), ("g1", g1), ("b1", b1), ("g2", g2),
                          ("b2", b2)):
                nc.sync.dma_start(out=t, in_=cst[nm][l])
            # ffn weights: issued now, consumed after LN1 (overlaps attention)
            w1 = wpF.tile([128, 4, FF], BF16, tag="w1")
            w2 = wpF.tile([128, 16, 512], BF16, tag="w2")
            bf1 = wpF.tile([128, 16], F32, tag="bf1")
            bf2 = wpF.tile([128, 4], F32, tag="bf2")
            for nm, t in (("w1", w1), ("w2", w2), ("bf1", bf1), ("bf2_c", bf2)):
                nc.sync.dma_start(out=t, in_=cst[nm][l])

            # ---------------- per-region LN (stats already in ps_st) --------
            def ln_region(cs, wdt, ps_st, g, b, last=False):
                # rows: mean (bf16), mean^2, var, ln(var+eps), rstd=exp(-.5ln)
                srow = rowp.tile([1, 512], BF16, tag="srow", name="srow")[:, :wdt]
                nc.scalar.activation(out=srow, in_=ps_st[0:1, :wdt],
                                     func=AF.Copy)
                m2 = rowp.tile([1, 512], F32, tag="m2row", name="m2")[:, :wdt]
                nc.vector.tensor_mul(m2, srow, srow)
                var = rowp.tile([1, 512], F32, tag="varrow", name="var")[:, :wdt]
                nc.vector.tensor_sub(var, ps_st[32:33, :wdt], m2)
                lv = rowp.tile([1, 512], F32, tag="lvrow", name="lv")[:, :wdt]
                nc.scalar.activation(out=lv, in_=var, func=AF.Ln,
                                     bias=eps128[0:1, :])
                rrow = rowp.tile([1, 512], BF16, tag="rrow", name="rrow")[:, :wdt]
                nc.scalar.activation(out=rrow, in_=lv, func=AF.Exp, scale=-0.5)
                # broadcast rows to 128 partitions on the PE
                ps_bm = psaux.tile([128, 512], F32, tag="aux", name="ps_bm")[:, :wdt]
                nc.tensor.matmul(ps_bm, lhsT=ones1, rhs=srow,
                                 start=True, stop=True)
                ps_br = psaux.tile([128, 512], F32, tag="aux", name="ps_br")[:, :wdt]
                nc.tensor.matmul(ps_br, lhsT=ones1, rhs=rrow,
                                 start=True, stop=True)
                xc = X[:, :, cs:cs + wdt]
                nc.vector.tensor_sub(xc, xc, _insdim_ap(ps_bm, 0, 4))
                nc.vector.tensor_mul(xc, xc, _insdim_ap(ps_br, 0, 4))
                ob = None
                if not trivial_gb:
                    if last:
                        ob = sqp.tile([128, 4, 512], BF16, tag="xsq2",
                                      name="ob")
                    for mc in range(4):
                        dst = ob[:, mc, 0:wdt] if last else X[:, mc, cs:cs + wdt]
                        nc.scalar.activation(out=dst, in_=X[:, mc, cs:cs + wdt],
                                             func=AF.Identity,
                                             bias=b[:, mc:mc + 1],
                                             scale=g[:, mc:mc + 1])
                elif last:
                    ob = sqp.tile([128, 4, 512], BF16, tag="xsq2", name="ob")
                    nc.vector.tensor_copy(out=ob[:, :, 0:wdt], in_=xc)
                if last:
                    nc.sync.dma_start(out=d["out"][:, :, cs:cs + wdt],
                                      in_=ob[:, :, 0:wdt])

            def stats_mms(xsb_sl, xsq_sl, wdt):
                # mean at partition 0, mean-square at partition 32 (bf16 MMs)
                ps_st = psaux.tile([33, 512], F32, tag="aux")
                for kc in range(4):
                    nc.tensor.matmul(ps_st[0:1, :wdt], lhsT=ones,
                                     rhs=xsb_sl[:, kc, :],
                                     start=(kc == 0), stop=(kc == 3))
                for kc in range(4):
                    nc.tensor.matmul(ps_st[32:33, :wdt], lhsT=ones,
                                     rhs=xsq_sl[:, kc, :],
                                     start=(kc == 0), stop=(kc == 3))
                return ps_st

            # ---------------- per-pair attention emitter --------------------
            def att_pair(p):
                cs0 = p * 288
                xs = X[:, :, cs0:cs0 + 288]
                xb = xbp.tile([128, 4, 304], BF16, tag="xb")
                nc.gpsimd.tensor_copy(out=xb, in_=X[:, :, cs0:cs0 + 304])
                # tail tokens of both windows packed at cols {0:16, 32:48}
                xt = xbp.tile([128, 4, 64], BF16, tag="xt")
                nc.gpsimd.tensor_copy(
                    out=_insdim_ap(xt[:, :, 0:16], 32, 2, at=2),
                    in_=_insdim_ap(X[:, :, cs0 + 128:cs0 + 144], 144, 2, at=2))
                qw = winp.tile([128, 4, 288], BF16, tag="qw")
                kw = winp.tile([128, 4, 288], BF16, tag="kw")
                for mc in range(4):
                    pq = psmm.tile([128, 288], F32, tag="mm")
                    for kc in range(4):
                        nc.tensor.matmul(pq, lhsT=wq[:, kc, mc * 128:(mc + 1) * 128],
                                         rhs=xb[:, kc, 0:288], start=(kc == 0), stop=(kc == 3))
                    if trivial_bias:
                        nc.vector.tensor_copy(out=qw[:, mc, :], in_=pq)
                    else:
                        nc.scalar.activation(out=qw[:, mc, :], in_=pq, func=AF.Identity,
                                             bias=bq[:, mc:mc + 1])
                    pk = psmm.tile([128, 288], F32, tag="mm")
                    for kc in range(4):
                        nc.tensor.matmul(pk, lhsT=wk[:, kc, mc * 128:(mc + 1) * 128],
                                         rhs=xb[:, kc, 0:288], start=(kc == 0), stop=(kc == 3))
                    nc.scalar.activation(out=kw[:, mc, :], in_=pk, func=AF.Identity,
                                         bias=bk[:, mc:mc + 1])

                vws = []
                for wi in (0, 1):
                    vw1 = winp.tile([128, NH, 65], BF16, tag=f"vw1_{wi}")
                    off = wi * 144
                    pv1 = psmm.tile([128, 512], F32, tag="mm")
                    for kc in range(4):
                        nc.tensor.matmul(pv1, lhsT=xb[:, kc, off:off + 128],
                                         rhs=wv[:, kc, :], start=(kc == 0), stop=(kc == 3))
                    nc.vector.tensor_add(out=vw1[:, :, 0:64],
                                         in0=pv1.rearrange("p (h e) -> p h e", h=NH),
                                         in1=bv.rearrange("p (h e) -> p h e", h=NH))
                    nc.vector.memset(vw1[:, :, 64:65], 1.0)
                    vws.append(vw1)
                # merged tail-V for both windows: lhsT cols {128:160, 272:304}
                # -> out partitions A-tail 0:16, (garbage 16:32), B-tail 32:48
                vw2p = winp.tile([64, NH, 65], BF16, tag="vw2p")
                pv2 = psmm.tile([64, 512], F32, tag="mm")
                for kc in range(4):
                    nc.tensor.matmul(pv2, lhsT=xt[:, kc, :], rhs=wv[:, kc, :],
                                     start=(kc == 0), stop=(kc == 3))
                nc.vector.tensor_add(out=vw2p[:, :, 0:64],
                                     in0=pv2.rearrange("p (h e) -> p h e", h=NH),
                                     in1=bv[0:64].rearrange("p (h e) -> p h e", h=NH))
                nc.vector.memset(vw2p[:, :, 64:65], 1.0)

                ocm = winp.tile([128, 4, 288], BF16, tag="ocm")
                if skip_heads:
                    nc.vector.tensor_copy(out=ocm, in_=xs)

                # software-pipelined head loop: stage A (S-mm, exp, P-mul) runs
                # `LOOKAHEAD` heads in front of stage B (PV, den) and stage C
                # (per head-pair: e2 bcast-mm, recip, ocm scale), so the PE has
                # independent matmuls queued while act/DVE chew on earlier heads.
                heads = [(wi, h) for wi in (0, 1) for h in range(NH)]
                pts = {}
                psos = {}

                def stage_a(wi, h):
                    off = wi * 144
                    tb = 32 * wi
                    ro, tl = (h % 2) * 64, h // 2
                    ps_s = psmm.tile([128, 288], F32, tag="mm")
                    nc.tensor.matmul(ps_s[:, 0:144],
                                     lhsT=kw[ro:ro + 64, tl, off:off + 128],
                                     rhs=qw[ro:ro + 64, tl, off:off + 144],
                                     start=True, stop=True)
                    nc.tensor.matmul(ps_s[tb:tb + 16, 144:288],
                                     lhsT=kw[ro:ro + 64, tl, off + 128:off + 144],
                                     rhs=qw[ro:ro + 64, tl, off:off + 144],
                                     start=True, stop=True)
                    et = etp.tile([128, 288], BF16, tag="e")
                    nc.scalar.activation(out=et, in_=ps_s, func=AF.Exp)
                    pt = ep.tile([128, 288], BF16, tag="p")
                    nc.vector.tensor_mul(pt, et, eb[:, h, :])
                    pts[(wi, h)] = pt

                def stage_b(wi, h):
                    pt = pts.pop((wi, h))
                    vw1 = vws[wi]
                    smat = smats[wi * 4 + h // 2]
                    ps_o = psaux.tile([65, 144], F32, tag="aux")
                    nc.tensor.matmul(ps_o, lhsT=vw1[:, h, :], rhs=pt[:, 0:144],
                                     start=True, stop=False)
                    tb = 32 * wi
                    nc.tensor.matmul(ps_o, lhsT=vw2p[tb:tb + 16, h, :],
                                     rhs=pt[tb:tb + 16, 144:288],
                                     start=False, stop=True)
                    nc.scalar.activation(out=smat[32 * (h % 2):32 * (h % 2) + 1, :],
                                         in_=ps_o[64:65, 0:144], func=AF.Copy)
                    psos[(wi, h)] = ps_o
                    if h % 2 == 1:
                        stage_c(wi, h // 2, smat)

                def stage_c(wi, hpair, smat):
                    off = wi * 144
                    ps_sc = psmm.tile([128, 144], F32, tag="mm")
                    nc.tensor.matmul(ps_sc, lhsT=e2, rhs=smat, start=True, stop=True)
                    sc = scp.tile([128, 144], F32, tag="scsb")
                    nc.vector.reciprocal_approx_fast(out=sc, in_=ps_sc)
                    p0 = psos.pop((wi, 2 * hpair))
                    p1 = psos.pop((wi, 2 * hpair + 1))
                    nc.vector.tensor_mul(ocm[0:64, hpair, off:off + 144],
                                         p0[0:64, :], sc[0:64, :])
                    nc.vector.tensor_mul(ocm[64:128, hpair, off:off + 144],
                                         p1[0:64, :], sc[64:128, :])

                LOOKAHEAD = la
                for i, (wi, h) in enumerate(heads if not skip_heads else []):
                    stage_a(wi, h)
                    if i >= LOOKAHEAD:
                        stage_b(*heads[i - LOOKAHEAD])
                for j in (range(max(0, len(heads) - LOOKAHEAD), len(heads))
                          if not skip_heads else []):
                    stage_b(*heads[j])

                # O projection + residual -> X (pre-LN1), stats, LN1
                for mc in range(4):
                    po = psmm.tile([128, 288], F32, tag="mm")
                    for kc in range(4):
                        nc.tensor.matmul(po, lhsT=wo[:, kc, mc * 128:(mc + 1) * 128],
                                         rhs=ocm[:, kc, :], start=(kc == 0), stop=(kc == 3))
                    nc.vector.tensor_add(out=X[:, mc, cs0:cs0 + 288], in0=po,
                                         in1=X[:, mc, cs0:cs0 + 288])
                    if not trivial_bias:
                        nc.vector.tensor_add(out=X[:, mc, cs0:cs0 + 288],
                                             in0=X[:, mc, cs0:cs0 + 288],
                                             in1=bo[:, mc:mc + 1].broadcast_to([128, 288]))
                xsb = sqp.tile([128, 4, 288], BF16, tag="xsb", name="xsb")
                nc.gpsimd.tensor_copy(out=xsb, in_=xs)
                xsq = sqp.tile([128, 4, 288], BF16, tag="xsq")
                nc.gpsimd.tensor_mul(xsq, xsb, xsb)
                ps_st = stats_mms(xsb, xsq, 288)
                ln_region(cs0, 288, ps_st, g1, b1)

            # ---------------- FFN chunk emitter (incl. LN2) ----------------
            lastl = (l == NL - 1)

            def ffn_chunk(cc):
                cs = cc * 512
                xc = X[:, :, cs:cs + 512]
                xb2 = xb2p.tile([128, 4, 512], BF16, tag="xb2")
                nc.gpsimd.tensor_copy(out=xb2, in_=xc)
                hb = hp.tile([128, 16, 512], BF16, tag="hb")
                for fc in range(16):
                    ph = psmm.tile([128, 512], F32, tag="mm")
                    for kc in range(4):
                        nc.tensor.matmul(ph, lhsT=w1[:, kc, fc * 128:(fc + 1) * 128],
                                         rhs=xb2[:, kc, :], start=(kc == 0), stop=(kc == 3))
                    if fc % 2 == 0:
                        nc.scalar.activation(out=hb[:, fc, :], in_=ph, func=AF.Relu,
                                             bias=bf1[:, fc:fc + 1])
                    else:
                        nc.vector.tensor_scalar(
                            out=hb[:, fc, :], in0=ph, scalar1=bf1[:, fc:fc + 1],
                            scalar2=0.0, op0=mybir.AluOpType.add,
                            op1=mybir.AluOpType.max)
                for mc in range(4):
                    pf = psmm.tile([128, 512], F32, tag="mm")
                    for fc in range(16):
                        nc.tensor.matmul(pf, lhsT=w2[:, fc, mc * 128:(mc + 1) * 128],
                                         rhs=hb[:, fc, :], start=(fc == 0), stop=(fc == 15))
                    nc.vector.tensor_add(out=X[:, mc, cs:cs + 512], in0=pf,
                                         in1=X[:, mc, cs:cs + 512])
                    if not trivial_bias:
                        nc.vector.tensor_add(out=X[:, mc, cs:cs + 512],
                                             in0=X[:, mc, cs:cs + 512],
                                             in1=bf2[:, mc:mc + 1].broadcast_to([128, 512]))
                xsb = sqp.tile([128, 4, 512], BF16, tag="xsb2", name="xsb")
                nc.gpsimd.tensor_copy(out=xsb, in_=xc)
                xsq = sqp.tile([128, 4, 512], BF16, tag="xsq2")
                nc.gpsimd.tensor_mul(xsq, xsb, xsb)
                ps_st = stats_mms(xsb, xsq, 512)
                ln_region(cs, 512, ps_st, g2, b2, lastl)

            # ---------------- layer schedule -----------------------------
            if not skip_attn:
                nxt = 0
                for p in range(PAIRS):
                    att_pair(p)
                    if not skip_ffn:
                        while nxt < NCH and ((nxt + 1) * 512 <= (p + 1) * 288
                                             or p == PAIRS - 1):
                            ffn_chunk(nxt)
                            nxt += 1
            elif not skip_ffn:
                for cc in range(NCH):
                    ffn_chunk(cc)
            if skip_ffn and l == NL - 1:
                for cc in range(NCH):
                    cs = cc * 512
                    nc.sync.dma_start(out=d["out"][:, :, cs:cs + 512],
                                      in_=X[:, :, cs:cs + 512])

    return d


# ---------------------------------------------------------------------------
# Host-side packing + golden model
# ---------------------------------------------------------------------------

def rel_idx():
    coords = np.stack(np.meshgrid(np.arange(WS), np.arange(WS), indexing="ij"))
    flat = coords.reshape(2, -1)
    rel = (flat[:, :, None] - flat[:, None, :]).transpose(1, 2, 0).copy()
    rel[..., 0] += WS - 1
    rel[..., 1] += WS - 1
    rel[..., 0] *= 2 * WS - 1
    return rel.sum(-1)  # [N, N] int


def pack_weights(w, NL):
    """w: dict of reference arrays -> dict of const arrays (np)."""
    bf = ml_dtypes.bfloat16
    scale = HD ** -0.5
    ridx = rel_idx()
    out = {}

    def lhsT_pack(W, kchunks):  # [Cin, Cout] -> [128, kchunks, Cout]
        return np.ascontiguousarray(
            W.reshape(kchunks, 128, W.shape[1]).transpose(1, 0, 2)
        )

    wq = np.stack([lhsT_pack(w["Wq"][l] * scale, 4) for l in range(NL)])
    wk = np.stack([lhsT_pack(w["Wk"][l], 4) for l in range(NL)])
    wv = np.stack([lhsT_pack(w["Wv"][l], 4) for l in range(NL)])
    wo = np.stack([lhsT_pack(w["Wo"][l], 4) for l in range(NL)])
    w1 = np.stack([lhsT_pack(w["W1"][l], 4) for l in range(NL)])
    w2 = np.stack([lhsT_pack(w["W2"][l], 16) for l in range(NL)])
    for nm, arr in (("wq", wq), ("wk", wk), ("wv", wv), ("wo", wo),
                    ("w1", w1), ("w2", w2)):
        out[nm] = arr.astype(bf)

    expb = np.zeros((NL, 128, NH, 288), np.float32)
    for l in range(NL):
        bias = w["rpb"][l][ridx]            # [N(i), N(j), NH]
        ebT = np.exp(bias.transpose(2, 1, 0))  # [NH, j, i]
        expb[l, 0:128, :, 0:144] = ebT[:, 0:128, :].transpose(1, 0, 2)
        expb[l, 0:16, :, 144:288] = ebT[:, 128:144, :].transpose(1, 0, 2)
        expb[l, 32:48, :, 144:288] = ebT[:, 128:144, :].transpose(1, 0, 2)
    out["expb"] = expb.astype(bf)

    def percol(b):  # [NL, C] -> [NL, 128, 4]
        return np.ascontiguousarray(
            b.reshape(NL, 4, 128).transpose(0, 2, 1)).astype(np.float32)

    out["bq"] = percol(w["bq"] * scale)
    out["bk"] = percol(w["bk"])
    out["bo_c"] = percol(w["bo"])
    out["bf2_c"] = percol(w["bf2"])
    out["c_ones1"] = np.ones((1, 128), bf)
    e2 = np.zeros((64, 128), np.float32)
    e2[0, 0:64] = 1.0
    e2[32, 64:128] = 1.0
    out["c_e2"] = e2.astype(bf)
    out["g1"] = percol(w["g1"])
    out["b1"] = percol(w["b1"])
    out["g2"] = percol(w["g2"])
    out["b2"] = percol(w["b2"])
    out["bf1"] = np.ascontiguousarray(
        w["bf1"].reshape(NL, 16, 128).transpose(0, 2, 1)).astype(np.float32)
    out["bvb"] = np.broadcast_to(
        w["bv"].astype(bf)[:, None, :], (NL, 128, 512)).copy()
    out["c_ones"] = np.full((128, 1), 1.0 / 512.0, bf)
    return out


def golden_tm(x_tm, w, NL):
    """fp32 numpy reference on window-major token-major x [T, 512]."""
    T = x_tm.shape[0]
    NW = T // N
    ridx = rel_idx()
    x = x_tm.astype(np.float32)

    def ln(v, g, b):
        m = v.mean(-1, keepdims=True)
        s = v.var(-1, keepdims=True)
        return (v - m) / np.sqrt(s + EPS) * g + b

    for l in range(NL):
        xw = x.reshape(NW, N, C)
        q = (xw @ w["Wq"][l] + w["bq"][l]).reshape(NW, N, NH, HD).transpose(0, 2, 1, 3)
        k = (xw @ w["Wk"][l] + w["bk"][l]).reshape(NW, N, NH, HD).transpose(0, 2, 1, 3)
        v = (xw @ w["Wv"][l] + w["bv"][l]).reshape(NW, N, NH, HD).transpose(0, 2, 1, 3)
        bias = w["rpb"][l][ridx].transpose(2, 0, 1)
        attn = np.einsum("whid,whjd->whij", q, k) * (HD ** -0.5) + bias
        attn = attn - attn.max(-1, keepdims=True)
        p = np.exp(attn)
        p = p / p.sum(-1, keepdims=True)
        o = np.einsum("whij,whjd->whid", p, v).transpose(0, 2, 1, 3).reshape(NW, N, C)
        o = o @ w["Wo"][l] + w["bo"][l]
        x = ln(o.reshape(T, C) + x, w["g1"][l], w["b1"][l])
        h = np.maximum(x @ w["W1"][l] + w["bf1"][l], 0.0) @ w["W2"][l] + w["bf2"][l]
        x = ln(h + x, w["g2"][l], w["b2"][l])
    return x


# ---------------------------------------------------------------------------
# kernel() entry point: full inputs -> full output, 8-way batch data parallel
# ---------------------------------------------------------------------------

NCORES = 8
B_FULL = 64
H_RES = W_RES = 24
L_TOK = H_RES * W_RES
NW_FULL = (B_FULL // NCORES) * (H_RES // WS) * (W_RES // WS)   # 32 windows/core
NL_FULL = 3

_COMPILED = {}


def _pack_x_all(x):
    """[64, 576, 512] f32 -> [8, 128, 4, T] bf16 channel-major window-major."""
    b = x.reshape(NCORES, B_FULL // NCORES, 2, WS, 2, WS, 4, 128)
    v = b.transpose(0, 7, 6, 1, 2, 4, 3, 5)   # [core,128,4, b,hw,ww,hs,ws]
    return np.ascontiguousarray(v.reshape(NCORES, 128, 4, -1)
                                ).astype(ml_dtypes.bfloat16)


def _unpack_out_all(res_list):
    """list of [128, 4, T] -> [64, 576, 512] f32."""
    y = np.stack([r.astype(np.float32) for r in res_list])     # [8,128,4,T]
    bpc = B_FULL // NCORES
    v = y.reshape(NCORES, 128, 4, bpc, 2, 2, WS, WS)
    v = v.transpose(0, 3, 4, 6, 5, 7, 2, 1)   # [core,b,hw,hs,ww,ws,4,128]
    return np.ascontiguousarray(v.reshape(B_FULL, L_TOK, C))


def kernel(x, Wq, bq, Wk, bk, Wv, bv, Wo, bo, rpb,
           g1, b1, W1, bf1, W2, bf2, g2, b2):
    import hashlib
    from concourse.bass_utils import run_bass_kernel_spmd

    w = {"Wq": np.asarray(Wq, np.float32), "bq": np.asarray(bq, np.float32),
         "Wk": np.asarray(Wk, np.float32), "bk": np.asarray(bk, np.float32),
         "Wv": np.asarray(Wv, np.float32), "bv": np.asarray(bv, np.float32),
         "Wo": np.asarray(Wo, np.float32), "bo": np.asarray(bo, np.float32),
         "rpb": np.asarray(rpb, np.float32),
         "g1": np.asarray(g1, np.float32), "b1": np.asarray(b1, np.float32),
         "W1": np.asarray(W1, np.float32), "bf1": np.asarray(bf1, np.float32),
         "W2": np.asarray(W2, np.float32), "bf2": np.asarray(bf2, np.float32),
         "g2": np.asarray(g2, np.float32), "b2": np.asarray(b2, np.float32)}
    hsh = hashlib.blake2b(
        b"".join(np.ascontiguousarray(v).tobytes() for v in w.values()),
        digest_size=16).hexdigest()
    if _COMPILED.get("hash") != hsh:
        packed = pack_weights(w, NL_FULL)
        trivial_gb = bool(np.all(w["g1"] == 1) and np.all(w["b1"] == 0)
                          and np.all(w["g2"] == 1) and np.all(w["b2"] == 0))
        trivial_bias = bool(all(np.all(w[k] == 0)
                                for k in ("bq", "bk", "bv", "bo", "bf1", "bf2")))
        nc = bacc.Bacc("TRN2", target_bir_lowering=False, debug=False)
        build(nc, NW_FULL, NL_FULL, packed,
              trivial_gb=trivial_gb, trivial_bias=trivial_bias)
        nc.compile()
        _COMPILED.update(hash=hsh, nc=nc)

    xp = _pack_x_all(np.asarray(x, np.float32))
    in_maps = [{"x": xp[i]} for i in range(NCORES)]
    res = run_bass_kernel_spmd(_COMPILED["nc"], in_maps, list(range(NCORES)))
    return _unpack_out_all([res.results[i]["out"] for i in range(NCORES)])



# revision 20
# speedup vs baseline: 1.0032x; 1.0032x over previous
"""Swin-style window-attention encoder as a Bass/Tile kernel for TRN2 — v3.

Key design vs v1:
- Residual master X lives in SBUF as FP32 [128, 4, T+16] (channel-major) —
  the residual stream never rounds to bf16 (bf16 master measured 2.2e-2 max
  rel err, over the 2e-2 budget; f32 master 7.8e-3). Matmul inputs are
  bf16 copies cast on the (otherwise idle) GPSIMD engine; per-token mean
  stats contract the f32 master directly (f32 matmul, tiny N).
- Weights are baked into the NEFF as inline consts — per-launch IO is just
  x (bf16 in) and out (bf16).
- Per-layer phase batching: [attention (Exp table)] -> [LN1 rows (Sqrt) +
  apply] -> [FFN (Relu, no table load)] -> [LN2 rows + apply]. 2 activation
  table loads per layer instead of ~64.
- Attention softmax denominators: collected per head into smat rows (act
  Copy), broadcast to 128 partitions via one e2 matmul, ONE fat [128,144]
  DVE reciprocal per head-pair (no 1-lane recips).
- LN row math on [128,36] shuffled layout (SBUF->SBUF strided DMA), not
  1-lane [1,T] ops.
- O-proj / QK-proj / stats at window-pair (288 tokens) granularity; FFN at
  512-token chunks.
"""
from contextlib import ExitStack

import numpy as np
import ml_dtypes

import concourse.bass as bass
import concourse.bacc as bacc
import concourse.tile as tile
import concourse.mybir as mybir

F32 = mybir.dt.float32
F32R = mybir.dt.float32r
BF16 = mybir.dt.bfloat16
AF = mybir.ActivationFunctionType

WS = 12
N = WS * WS          # 144 tokens per window
C = 512
NH = 8
HD = 64
FF = 2048
EPS = 1e-5


def _insdim_ap(row_ap, stride, num, at=1):
    """Insert a dim of (stride, num) at position `at` of the AP (default:
    right after the partition dim). stride=0 -> broadcast; else gather."""
    dims = [list(d) for d in row_ap.ap]
    return bass.AP(
        tensor=row_ap.tensor,
        offset=row_ap.offset,
        ap=dims[:at] + [[stride, num]] + dims[at:],
    )


def _bcast_ap(row_ap, parts):
    return _insdim_ap(row_ap, 0, parts)


def build(nc: bass.Bass, NW: int, NL: int, w: dict,
          skip_attn=False, skip_ffn=False, skip_heads=False,
          pb=(4, 4), winb=2, epb=3, sqb=1, hbb=1, scb=2, la=3, xbb=2,
          trivial_gb=False, trivial_bias=False):
    """w: packed numpy weight dict (see pack_weights)."""
    T = NW * N
    PAIRS = NW // 2
    NCH = T // 512
    assert T % 512 == 0

    d = {}
    d["x"] = nc.dram_tensor("x", [128, 4, T], BF16, kind="ExternalInput").ap()
    d["out"] = nc.dram_tensor("out", [128, 4, T], BF16, kind="ExternalOutput").ap()
    cst = {nm: nc.inline_tensor(arr, name=nm).ap() for nm, arr in w.items()}

    with tile.TileContext(nc) as tc, ExitStack() as ctx:
        P = lambda name, bufs, **kw: ctx.enter_context(
            tc.tile_pool(name=name, bufs=bufs, **kw)
        )
        xp = P("xmaster", 1)
        cons = P("consts", 1)
        wpA = P("wtsA", 1)     # attention-phase weights
        wpF = P("wtsF", 1)     # ffn-phase weights
        winp = P("win", winb)  # per-pair working tiles
        ep = P("eptiles", epb)  # P tiles
        etp = P("ettiles", 2)   # exp tiles (short-lived)
        sqp = P("sqtiles", sqb)  # squared-x tiles for stats
        scp = P("sctiles", scb)  # recip rows [128,144]
        rowp = P("rows", 2)    # LN stat rows (short-lived, per region)
        hp = P("hbuf", hbb)
        xbp = P("xbcast", 2)
        xb2p = P("xb2cast", 2)
        psmm = P("psmm", pb[0], space="PSUM")
        psaux = P("psaux", 3, space="PSUM")

        # ---- persistent tiles ----
        X = xp.tile([128, 4, T + 16], F32, tag="X")
        for tq in range(NCH):
            xin = sqp.tile([128, 4, 512], BF16, tag="xsq2")
            nc.sync.dma_start(out=xin,
                              in_=d["x"][:, :, tq * 512:(tq + 1) * 512])
            nc.vector.tensor_copy(out=X[:, :, tq * 512:(tq + 1) * 512], in_=xin)
        ones = cons.tile([128, 1], BF16, tag="ones")       # value 1/512
        nc.sync.dma_start(out=ones, in_=cst["c_ones"])
        ones1 = cons.tile([1, 128], BF16, tag="ones1")     # bcast lhsT (1.0)
        nc.sync.dma_start(out=ones1, in_=cst["c_ones1"])
        e2 = cons.tile([64, 128], BF16, tag="e2")
        nc.sync.dma_start(out=e2, in_=cst["c_e2"])
        eps128 = cons.tile([128, 1], F32, tag="eps128")
        nc.vector.memset(eps128, EPS)
        onesf = cons.tile([128, 1], F32, tag="onesf")
        nc.vector.memset(onesf, 1.0 / 512.0)
        smats = [cons.tile([64, 144], BF16, tag=f"smat{i}", name=f"smat{i}")
                 for i in range(8)]
        for t in smats:
            nc.vector.memset(t, 0.0)

        for l in range(NL):
            # layer weights (attention set + rows)
            wq = wpA.tile([128, 4, 512], BF16, tag="wq")
            wk = wpA.tile([128, 4, 512], BF16, tag="wk")
            wv = wpA.tile([128, 4, 512], BF16, tag="wv")
            wo = wpA.tile([128, 4, 512], BF16, tag="wo")
            eb = wpA.tile([128, NH, 288], BF16, tag="expb")
            bq = wpA.tile([128, 4], F32, tag="bq")
            bk = wpA.tile([128, 4], F32, tag="bk")
            bo = wpA.tile([128, 4], F32, tag="bo")
            bv = wpA.tile([128, 512], BF16, tag="bvb")
            g1 = wpA.tile([128, 4], F32, tag="g1")
            b1 = wpA.tile([128, 4], F32, tag="b1")
            g2 = wpA.tile([128, 4], F32, tag="g2")
            b2 = wpA.tile([128, 4], F32, tag="b2")
            for nm, t in (("wq", wq), ("wk", wk), ("wv", wv), ("wo", wo),
                          ("expb", eb), ("bq", bq), ("bk", bk), ("bo_c", bo),
                          ("bvb", bv), ("g1", g1), ("b1", b1), ("g2", g2),
                          ("b2", b2)):
                nc.sync.dma_start(out=t, in_=cst[nm][l])
            # ffn weights: issued now, consumed after LN1 (overlaps attention)
            w1 = wpF.tile([128, 4, FF], BF16, tag="w1")
            w2 = wpF.tile([128, 16, 512], BF16, tag="w2")
            bf1 = wpF.tile([128, 16], F32, tag="bf1")
            bf2 = wpF.tile([128, 4], F32, tag="bf2")
            for nm, t in (("w1", w1), ("w2", w2), ("bf1", bf1), ("bf2_c", bf2)):
                nc.sync.dma_start(out=t, in_=cst[nm][l])

            # ---------------- per-region LN (stats already in ps_st) --------
            def ln_region(cs, wdt, ps_st, g, b, last=False):
                # rows: mean (bf16), mean^2, var, ln(var+eps), rstd=exp(-.5ln)
                srow = rowp.tile([1, 512], BF16, tag="srow", name="srow")[:, :wdt]
                nc.scalar.activation(out=srow, in_=ps_st[0:1, :wdt],
                                     func=AF.Copy)
                m2 = rowp.tile([1, 512], F32, tag="m2row", name="m2")[:, :wdt]
                nc.scalar.activation(out=m2, in_=ps_st[0:1, :wdt], func=AF.Square)
                nc.vector.tensor_sub(m2, ps_st[32:33, :wdt], m2)   # var, in place
                nc.scalar.activation(out=m2, in_=m2, func=AF.Ln,
                                     bias=eps128[0:1, :])          # ln(var+eps)
                rrow = rowp.tile([1, 512], BF16, tag="rrow", name="rrow")[:, :wdt]
                nc.scalar.activation(out=rrow, in_=m2, func=AF.Exp, scale=-0.5)
                # broadcast rows to 128 partitions on the PE; stage to SBUF
                # bf16 immediately so the PSUM bank frees fast (tag "bc"
                # bufs=1 -> bm/br serialize through one bank)
                ps_bm = psaux.tile([128, 512], F32, tag="bc", name="ps_bm",
                                   bufs=1)[:, :wdt]
                nc.tensor.matmul(ps_bm, lhsT=ones1, rhs=srow,
                                 start=True, stop=True)
                bm = rowp.tile([128, 512], BF16, tag="bmsb", name="bm")[:, :wdt]
                nc.vector.tensor_copy(out=bm, in_=ps_bm)
                ps_br = psaux.tile([128, 512], F32, tag="bc", name="ps_br",
                                   bufs=1)[:, :wdt]
                nc.tensor.matmul(ps_br, lhsT=ones1, rhs=rrow,
                                 start=True, stop=True)
                br = rowp.tile([128, 512], BF16, tag="brsb", name="br")[:, :wdt]
                nc.vector.tensor_copy(out=br, in_=ps_br)
                xc = X[:, :, cs:cs + wdt]
                nc.vector.tensor_sub(xc, xc, _insdim_ap(bm, 0, 4))
                nc.vector.tensor_mul(xc, xc, _insdim_ap(br, 0, 4))
                ob = None
                if not trivial_gb:
                    if last:
                        ob = sqp.tile([128, 4, 512], BF16, tag="xsq2",
                                      name="ob")
                    for mc in range(4):
                        dst = ob[:, mc, 0:wdt] if last else X[:, mc, cs:cs + wdt]
                        nc.scalar.activation(out=dst, in_=X[:, mc, cs:cs + wdt],
                                             func=AF.Identity,
                                             bias=b[:, mc:mc + 1],
                                             scale=g[:, mc:mc + 1])
                elif last:
                    ob = sqp.tile([128, 4, 512], BF16, tag="xsq2", name="ob")
                    nc.vector.tensor_copy(out=ob[:, :, 0:wdt], in_=xc)
                if last:
                    nc.sync.dma_start(out=d["out"][:, :, cs:cs + wdt],
                                      in_=ob[:, :, 0:wdt])

            def stats_mms(xs_sl, xsq_sl, wdt):
                # mean at partition 0 (f32 MM), mean-square at partition 32
                # (bf16 MM). Lives in the fast-draining "mm" ring.
                ps_st = psmm.tile([33, 512], F32, tag="mm", name="ps_st")
                for kc in range(4):
                    nc.tensor.matmul(ps_st[0:1, :wdt], lhsT=onesf,
                                     rhs=xs_sl[:, kc, :],
                                     start=(kc == 0), stop=(kc == 3))
                for kc in range(4):
                    nc.tensor.matmul(ps_st[32:33, :wdt], lhsT=ones,
                                     rhs=xsq_sl[:, kc, :],
                                     start=(kc == 0), stop=(kc == 3))
                return ps_st

            # ---------------- per-pair attention emitter --------------------
            def att_pair(p):
                cs0 = p * 288
                xs = X[:, :, cs0:cs0 + 288]
                xb = xbp.tile([128, 4, 304], BF16, tag="xb")
                nc.gpsimd.tensor_copy(out=xb, in_=X[:, :, cs0:cs0 + 304])
                # tail tokens of both windows packed at cols {0:16, 32:48}
                xt = xbp.tile([128, 4, 64], BF16, tag="xt")
                nc.gpsimd.tensor_copy(
                    out=_insdim_ap(xt[:, :, 0:16], 32, 2, at=2),
                    in_=_insdim_ap(X[:, :, cs0 + 128:cs0 + 144], 144, 2, at=2))
                qw = winp.tile([128, 4, 288], BF16, tag="qw")
                kw = winp.tile([128, 4, 288], BF16, tag="kw")
                for mc in range(4):
                    pq = psmm.tile([128, 288], F32, tag="mm")
                    for kc in range(4):
                        nc.tensor.matmul(pq, lhsT=wq[:, kc, mc * 128:(mc + 1) * 128],
                                         rhs=xb[:, kc, 0:288], start=(kc == 0), stop=(kc == 3))
                    if trivial_bias:
                        nc.vector.tensor_copy(out=qw[:, mc, :], in_=pq)
                    else:
                        nc.scalar.activation(out=qw[:, mc, :], in_=pq, func=AF.Identity,
                                             bias=bq[:, mc:mc + 1])
                    pk = psmm.tile([128, 288], F32, tag="mm")
                    for kc in range(4):
                        nc.tensor.matmul(pk, lhsT=wk[:, kc, mc * 128:(mc + 1) * 128],
                                         rhs=xb[:, kc, 0:288], start=(kc == 0), stop=(kc == 3))
                    nc.scalar.activation(out=kw[:, mc, :], in_=pk, func=AF.Identity,
                                         bias=bk[:, mc:mc + 1])

                vws = []
                for wi in (0, 1):
                    vw1 = winp.tile([128, NH, 65], BF16, tag=f"vw1_{wi}")
                    off = wi * 144
                    pv1 = psmm.tile([128, 512], F32, tag="mm")
                    for kc in range(4):
                        nc.tensor.matmul(pv1, lhsT=xb[:, kc, off:off + 128],
                                         rhs=wv[:, kc, :], start=(kc == 0), stop=(kc == 3))
                    nc.vector.tensor_add(out=vw1[:, :, 0:64],
                                         in0=pv1.rearrange("p (h e) -> p h e", h=NH),
                                         in1=bv.rearrange("p (h e) -> p h e", h=NH))
                    nc.vector.memset(vw1[:, :, 64:65], 1.0)
                    vws.append(vw1)
                # merged tail-V for both windows: lhsT cols {128:160, 272:304}
                # -> out partitions A-tail 0:16, (garbage 16:32), B-tail 32:48
                vw2p = winp.tile([64, NH, 65], BF16, tag="vw2p")
                pv2 = psmm.tile([64, 512], F32, tag="mm")
                for kc in range(4):
                    nc.tensor.matmul(pv2, lhsT=xt[:, kc, :], rhs=wv[:, kc, :],
                                     start=(kc == 0), stop=(kc == 3))
                nc.vector.tensor_add(out=vw2p[:, :, 0:64],
                                     in0=pv2.rearrange("p (h e) -> p h e", h=NH),
                                     in1=bv[0:64].rearrange("p (h e) -> p h e", h=NH))
                nc.vector.memset(vw2p[:, :, 64:65], 1.0)

                ocm = winp.tile([128, 4, 288], BF16, tag="ocm")
                if skip_heads:
                    nc.vector.tensor_copy(out=ocm, in_=xs)

                # software-pipelined head loop: stage A (S-mm, exp, P-mul) runs
                # `LOOKAHEAD` heads in front of stage B (PV, den) and stage C
                # (per head-pair: e2 bcast-mm, recip, ocm scale), so the PE has
                # independent matmuls queued while act/DVE chew on earlier heads.
                heads = [(wi, h) for wi in (0, 1) for h in range(NH)]
                pts = {}
                psos = {}

                def stage_a(wi, h):
                    off = wi * 144
                    tb = 32 * wi
                    ro, tl = (h % 2) * 64, h // 2
                    ps_s = psmm.tile([128, 288], F32, tag="mm")
                    nc.tensor.matmul(ps_s[:, 0:144],
                                     lhsT=kw[ro:ro + 64, tl, off:off + 128],
                                     rhs=qw[ro:ro + 64, tl, off:off + 144],
                                     start=True, stop=True)
                    nc.tensor.matmul(ps_s[tb:tb + 16, 144:288],
                                     lhsT=kw[ro:ro + 64, tl, off + 128:off + 144],
                                     rhs=qw[ro:ro + 64, tl, off:off + 144],
                                     start=True, stop=True)
                    et = etp.tile([128, 288], BF16, tag="e")
                    nc.scalar.activation(out=et, in_=ps_s, func=AF.Exp)
                    pt = ep.tile([128, 288], BF16, tag="p")
                    nc.vector.tensor_mul(pt, et, eb[:, h, :])
                    pts[(wi, h)] = pt

                def stage_b(wi, h):
                    pt = pts.pop((wi, h))
                    vw1 = vws[wi]
                    smat = smats[wi * 4 + h // 2]
                    ps_o = psaux.tile([65, 144], F32, tag="aux")
                    nc.tensor.matmul(ps_o, lhsT=vw1[:, h, :], rhs=pt[:, 0:144],
                                     start=True, stop=False)
                    tb = 32 * wi
                    nc.tensor.matmul(ps_o, lhsT=vw2p[tb:tb + 16, h, :],
                                     rhs=pt[tb:tb + 16, 144:288],
                                     start=False, stop=True)
                    nc.scalar.activation(out=smat[32 * (h % 2):32 * (h % 2) + 1, :],
                                         in_=ps_o[64:65, 0:144], func=AF.Copy)
                    psos[(wi, h)] = ps_o
                    if h % 2 == 1:
                        stage_c(wi, h // 2, smat)

                def stage_c(wi, hpair, smat):
                    off = wi * 144
                    ps_sc = psmm.tile([128, 144], F32, tag="mm")
                    nc.tensor.matmul(ps_sc, lhsT=e2, rhs=smat, start=True, stop=True)
                    sc = scp.tile([128, 144], F32, tag="scsb")
                    nc.vector.reciprocal_approx_fast(out=sc, in_=ps_sc)
                    p0 = psos.pop((wi, 2 * hpair))
                    p1 = psos.pop((wi, 2 * hpair + 1))
                    nc.vector.tensor_mul(ocm[0:64, hpair, off:off + 144],
                                         p0[0:64, :], sc[0:64, :])
                    nc.vector.tensor_mul(ocm[64:128, hpair, off:off + 144],
                                         p1[0:64, :], sc[64:128, :])

                LOOKAHEAD = la
                for i, (wi, h) in enumerate(heads if not skip_heads else []):
                    stage_a(wi, h)
                    if i >= LOOKAHEAD:
                        stage_b(*heads[i - LOOKAHEAD])
                for j in (range(max(0, len(heads) - LOOKAHEAD), len(heads))
                          if not skip_heads else []):
                    stage_b(*heads[j])

                # O projection + residual -> X (pre-LN1), stats, LN1
                for mc in range(4):
                    po = psmm.tile([128, 288], F32, tag="mm")
                    for kc in range(4):
                        nc.tensor.matmul(po, lhsT=wo[:, kc, mc * 128:(mc + 1) * 128],
                                         rhs=ocm[:, kc, :], start=(kc == 0), stop=(kc == 3))
                    nc.vector.tensor_add(out=X[:, mc, cs0:cs0 + 288], in0=po,
                                         in1=X[:, mc, cs0:cs0 + 288])
                    if not trivial_bias:
                        nc.vector.tensor_add(out=X[:, mc, cs0:cs0 + 288],
                                             in0=X[:, mc, cs0:cs0 + 288],
                                             in1=bo[:, mc:mc + 1].broadcast_to([128, 288]))
                xsq = sqp.tile([128, 4, 288], BF16, tag="xsq")
                nc.gpsimd.tensor_mul(xsq, xs, xs)
                ps_st = stats_mms(xs, xsq, 288)
                ln_region(cs0, 288, ps_st, g1, b1)

            # ---------------- FFN chunk emitter (incl. LN2) ----------------
            lastl = (l == NL - 1)

            def ffn_chunk(cc):
                cs = cc * 512
                xc = X[:, :, cs:cs + 512]
                xb2 = xb2p.tile([128, 4, 512], BF16, tag="xb2")
                nc.gpsimd.tensor_copy(out=xb2, in_=xc)
                hb = hp.tile([128, 16, 512], BF16, tag="hb")
                for fc in range(16):
                    ph = psmm.tile([128, 512], F32, tag="mm")
                    for kc in range(4):
                        nc.tensor.matmul(ph, lhsT=w1[:, kc, fc * 128:(fc + 1) * 128],
                                         rhs=xb2[:, kc, :], start=(kc == 0), stop=(kc == 3))
                    if fc % 2 == 0:
                        nc.scalar.activation(out=hb[:, fc, :], in_=ph, func=AF.Relu,
                                             bias=bf1[:, fc:fc + 1])
                    else:
                        nc.vector.tensor_scalar(
                            out=hb[:, fc, :], in0=ph, scalar1=bf1[:, fc:fc + 1],
                            scalar2=0.0, op0=mybir.AluOpType.add,
                            op1=mybir.AluOpType.max)
                for mc in range(4):
                    pf = psmm.tile([128, 512], F32, tag="mm")
                    for fc in range(16):
                        nc.tensor.matmul(pf, lhsT=w2[:, fc, mc * 128:(mc + 1) * 128],
                                         rhs=hb[:, fc, :], start=(fc == 0), stop=(fc == 15))
                    nc.vector.tensor_add(out=X[:, mc, cs:cs + 512], in0=pf,
                                         in1=X[:, mc, cs:cs + 512])
                    if not trivial_bias:
                        nc.vector.tensor_add(out=X[:, mc, cs:cs + 512],
                                             in0=X[:, mc, cs:cs + 512],
                                             in1=bf2[:, mc:mc + 1].broadcast_to([128, 512]))
                xsq = sqp.tile([128, 4, 512], BF16, tag="xsq2")
                nc.gpsimd.tensor_mul(xsq, xc, xc)
                ps_st = stats_mms(xc, xsq, 512)
                ln_region(cs, 512, ps_st, g2, b2, lastl)

            # ---------------- layer schedule -----------------------------
            if not skip_attn:
                nxt = 0
                for p in range(PAIRS):
                    att_pair(p)
                    if not skip_ffn:
                        while nxt < NCH and ((nxt + 1) * 512 <= (p + 1) * 288
                                             or p == PAIRS - 1):
                            ffn_chunk(nxt)
                            nxt += 1
            elif not skip_ffn:
                for cc in range(NCH):
                    ffn_chunk(cc)
            if skip_ffn and l == NL - 1:
                for cc in range(NCH):
                    cs = cc * 512
                    nc.sync.dma_start(out=d["out"][:, :, cs:cs + 512],
                                      in_=X[:, :, cs:cs + 512])

    return d


# ---------------------------------------------------------------------------
# Host-side packing + golden model
# ---------------------------------------------------------------------------

def rel_idx():
    coords = np.stack(np.meshgrid(np.arange(WS), np.arange(WS), indexing="ij"))
    flat = coords.reshape(2, -1)
    rel = (flat[:, :, None] - flat[:, None, :]).transpose(1, 2, 0).copy()
    rel[..., 0] += WS - 1
    rel[..., 1] += WS - 1
    rel[..., 0] *= 2 * WS - 1
    return rel.sum(-1)  # [N, N] int


def pack_weights(w, NL):
    """w: dict of reference arrays -> dict of const arrays (np)."""
    bf = ml_dtypes.bfloat16
    scale = HD ** -0.5
    ridx = rel_idx()
    out = {}

    def lhsT_pack(W, kchunks):  # [Cin, Cout] -> [128, kchunks, Cout]
        return np.ascontiguousarray(
            W.reshape(kchunks, 128, W.shape[1]).transpose(1, 0, 2)
        )

    wq = np.stack([lhsT_pack(w["Wq"][l] * scale, 4) for l in range(NL)])
    wk = np.stack([lhsT_pack(w["Wk"][l], 4) for l in range(NL)])
    wv = np.stack([lhsT_pack(w["Wv"][l], 4) for l in range(NL)])
    wo = np.stack([lhsT_pack(w["Wo"][l], 4) for l in range(NL)])
    w1 = np.stack([lhsT_pack(w["W1"][l], 4) for l in range(NL)])
    w2 = np.stack([lhsT_pack(w["W2"][l], 16) for l in range(NL)])
    for nm, arr in (("wq", wq), ("wk", wk), ("wv", wv), ("wo", wo),
                    ("w1", w1), ("w2", w2)):
        out[nm] = arr.astype(bf)

    expb = np.zeros((NL, 128, NH, 288), np.float32)
    for l in range(NL):
        bias = w["rpb"][l][ridx]            # [N(i), N(j), NH]
        ebT = np.exp(bias.transpose(2, 1, 0))  # [NH, j, i]
        expb[l, 0:128, :, 0:144] = ebT[:, 0:128, :].transpose(1, 0, 2)
        expb[l, 0:16, :, 144:288] = ebT[:, 128:144, :].transpose(1, 0, 2)
        expb[l, 32:48, :, 144:288] = ebT[:, 128:144, :].transpose(1, 0, 2)
    out["expb"] = expb.astype(bf)

    def percol(b):  # [NL, C] -> [NL, 128, 4]
        return np.ascontiguousarray(
            b.reshape(NL, 4, 128).transpose(0, 2, 1)).astype(np.float32)

    out["bq"] = percol(w["bq"] * scale)
    out["bk"] = percol(w["bk"])
    out["bo_c"] = percol(w["bo"])
    out["bf2_c"] = percol(w["bf2"])
    out["c_ones1"] = np.ones((1, 128), bf)
    e2 = np.zeros((64, 128), np.float32)
    e2[0, 0:64] = 1.0
    e2[32, 64:128] = 1.0
    out["c_e2"] = e2.astype(bf)
    out["g1"] = percol(w["g1"])
    out["b1"] = percol(w["b1"])
    out["g2"] = percol(w["g2"])
    out["b2"] = percol(w["b2"])
    out["bf1"] = np.ascontiguousarray(
        w["bf1"].reshape(NL, 16, 128).transpose(0, 2, 1)).astype(np.float32)
    out["bvb"] = np.broadcast_to(
        w["bv"].astype(bf)[:, None, :], (NL, 128, 512)).copy()
    out["c_ones"] = np.full((128, 1), 1.0 / 512.0, bf)
    return out


def golden_tm(x_tm, w, NL):
    """fp32 numpy reference on window-major token-major x [T, 512]."""
    T = x_tm.shape[0]
    NW = T // N
    ridx = rel_idx()
    x = x_tm.astype(np.float32)

    def ln(v, g, b):
        m = v.mean(-1, keepdims=True)
        s = v.var(-1, keepdims=True)
        return (v - m) / np.sqrt(s + EPS) * g + b

    for l in range(NL):
        xw = x.reshape(NW, N, C)
        q = (xw @ w["Wq"][l] + w["bq"][l]).reshape(NW, N, NH, HD).transpose(0, 2, 1, 3)
        k = (xw @ w["Wk"][l] + w["bk"][l]).reshape(NW, N, NH, HD).transpose(0, 2, 1, 3)
        v = (xw @ w["Wv"][l] + w["bv"][l]).reshape(NW, N, NH, HD).transpose(0, 2, 1, 3)
        bias = w["rpb"][l][ridx].transpose(2, 0, 1)
        attn = np.einsum("whid,whjd->whij", q, k) * (HD ** -0.5) + bias
        attn = attn - attn.max(-1, keepdims=True)
        p = np.exp(attn)
        p = p / p.sum(-1, keepdims=True)
        o = np.einsum("whij,whjd->whid", p, v).transpose(0, 2, 1, 3).reshape(NW, N, C)
        o = o @ w["Wo"][l] + w["bo"][l]
        x = ln(o.reshape(T, C) + x, w["g1"][l], w["b1"][l])
        h = np.maximum(x @ w["W1"][l] + w["bf1"][l], 0.0) @ w["W2"][l] + w["bf2"][l]
        x = ln(h + x, w["g2"][l], w["b2"][l])
    return x


# ---------------------------------------------------------------------------
# kernel() entry point: full inputs -> full output, 8-way batch data parallel
# ---------------------------------------------------------------------------

NCORES = 8
B_FULL = 64
H_RES = W_RES = 24
L_TOK = H_RES * W_RES
NW_FULL = (B_FULL // NCORES) * (H_RES // WS) * (W_RES // WS)   # 32 windows/core
NL_FULL = 3

_COMPILED = {}


def _pack_x_all(x):
    """[64, 576, 512] f32 -> [8, 128, 4, T] bf16 channel-major window-major."""
    b = x.reshape(NCORES, B_FULL // NCORES, 2, WS, 2, WS, 4, 128)
    v = b.transpose(0, 7, 6, 1, 2, 4, 3, 5)   # [core,128,4, b,hw,ww,hs,ws]
    return np.ascontiguousarray(v.reshape(NCORES, 128, 4, -1)
                                ).astype(ml_dtypes.bfloat16)


def _unpack_out_all(res_list):
    """list of [128, 4, T] -> [64, 576, 512] f32."""
    y = np.stack([r.astype(np.float32) for r in res_list])     # [8,128,4,T]
    bpc = B_FULL // NCORES
    v = y.reshape(NCORES, 128, 4, bpc, 2, 2, WS, WS)
    v = v.transpose(0, 3, 4, 6, 5, 7, 2, 1)   # [core,b,hw,hs,ww,ws,4,128]
    return np.ascontiguousarray(v.reshape(B_FULL, L_TOK, C))


def kernel(x, Wq, bq, Wk, bk, Wv, bv, Wo, bo, rpb,
           g1, b1, W1, bf1, W2, bf2, g2, b2):
    import hashlib
    from concourse.bass_utils import run_bass_kernel_spmd

    w = {"Wq": np.asarray(Wq, np.float32), "bq": np.asarray(bq, np.float32),
         "Wk": np.asarray(Wk, np.float32), "bk": np.asarray(bk, np.float32),
         "Wv": np.asarray(Wv, np.float32), "bv": np.asarray(bv, np.float32),
         "Wo": np.asarray(Wo, np.float32), "bo": np.asarray(bo, np.float32),
         "rpb": np.asarray(rpb, np.float32),
         "g1": np.asarray(g1, np.float32), "b1": np.asarray(b1, np.float32),
         "W1": np.asarray(W1, np.float32), "bf1": np.asarray(bf1, np.float32),
         "W2": np.asarray(W2, np.float32), "bf2": np.asarray(bf2, np.float32),
         "g2": np.asarray(g2, np.float32), "b2": np.asarray(b2, np.float32)}
    hsh = hashlib.blake2b(
        b"".join(np.ascontiguousarray(v).tobytes() for v in w.values()),
        digest_size=16).hexdigest()
    if _COMPILED.get("hash") != hsh:
        packed = pack_weights(w, NL_FULL)
        trivial_gb = bool(np.all(w["g1"] == 1) and np.all(w["b1"] == 0)
                          and np.all(w["g2"] == 1) and np.all(w["b2"] == 0))
        trivial_bias = bool(all(np.all(w[k] == 0)
                                for k in ("bq", "bk", "bv", "bo", "bf1", "bf2")))
        nc = bacc.Bacc("TRN2", target_bir_lowering=False, debug=False)
        build(nc, NW_FULL, NL_FULL, packed,
              trivial_gb=trivial_gb, trivial_bias=trivial_bias)
        nc.compile()
        _COMPILED.update(hash=hsh, nc=nc)

    xp = _pack_x_all(np.asarray(x, np.float32))
    in_maps = [{"x": xp[i]} for i in range(NCORES)]
    res = run_bass_kernel_spmd(_COMPILED["nc"], in_maps, list(range(NCORES)))
    return _unpack_out_all([res.results[i]["out"] for i in range(NCORES)])



# revision 21
# speedup vs baseline: 1.0180x; 1.0148x over previous
"""Swin-style window-attention encoder as a Bass/Tile kernel for TRN2 — v3.

Key design vs v1:
- Residual master X lives in SBUF as FP32 [128, 4, T+16] (channel-major) —
  the residual stream never rounds to bf16 (bf16 master measured 2.2e-2 max
  rel err, over the 2e-2 budget; f32 master 7.8e-3). Matmul inputs are
  bf16 copies cast on the (otherwise idle) GPSIMD engine; per-token mean
  stats contract the f32 master directly (f32 matmul, tiny N).
- Weights are baked into the NEFF as inline consts — per-launch IO is just
  x (bf16 in) and out (bf16).
- Per-layer phase batching: [attention (Exp table)] -> [LN1 rows (Sqrt) +
  apply] -> [FFN (Relu, no table load)] -> [LN2 rows + apply]. 2 activation
  table loads per layer instead of ~64.
- Attention softmax denominators: collected per head into smat rows (act
  Copy), broadcast to 128 partitions via one e2 matmul, ONE fat [128,144]
  DVE reciprocal per head-pair (no 1-lane recips).
- LN row math on [128,36] shuffled layout (SBUF->SBUF strided DMA), not
  1-lane [1,T] ops.
- O-proj / QK-proj / stats at window-pair (288 tokens) granularity; FFN at
  512-token chunks.
"""
from contextlib import ExitStack

import numpy as np
import ml_dtypes

import concourse.bass as bass
import concourse.bacc as bacc
import concourse.tile as tile
import concourse.mybir as mybir

F32 = mybir.dt.float32
F32R = mybir.dt.float32r
BF16 = mybir.dt.bfloat16
AF = mybir.ActivationFunctionType

WS = 12
N = WS * WS          # 144 tokens per window
C = 512
NH = 8
HD = 64
FF = 2048
EPS = 1e-5


def _insdim_ap(row_ap, stride, num, at=1):
    """Insert a dim of (stride, num) at position `at` of the AP (default:
    right after the partition dim). stride=0 -> broadcast; else gather."""
    dims = [list(d) for d in row_ap.ap]
    return bass.AP(
        tensor=row_ap.tensor,
        offset=row_ap.offset,
        ap=dims[:at] + [[stride, num]] + dims[at:],
    )


def _bcast_ap(row_ap, parts):
    return _insdim_ap(row_ap, 0, parts)


def build(nc: bass.Bass, NW: int, NL: int, w: dict,
          skip_attn=False, skip_ffn=False, skip_heads=False,
          pb=(4, 4), winb=2, epb=3, sqb=1, hbb=1, scb=2, la=3, xbb=2,
          trivial_gb=False, trivial_bias=False):
    """w: packed numpy weight dict (see pack_weights)."""
    T = NW * N
    PAIRS = NW // 2
    NCH = T // 512
    assert T % 512 == 0

    d = {}
    d["x"] = nc.dram_tensor("x", [128, 4, T], BF16, kind="ExternalInput").ap()
    d["out"] = nc.dram_tensor("out", [128, 4, T], BF16, kind="ExternalOutput").ap()
    cst = {nm: nc.inline_tensor(arr, name=nm).ap() for nm, arr in w.items()}

    with tile.TileContext(nc) as tc, ExitStack() as ctx:
        P = lambda name, bufs, **kw: ctx.enter_context(
            tc.tile_pool(name=name, bufs=bufs, **kw)
        )
        xp = P("xmaster", 1)
        cons = P("consts", 1)
        wpA = P("wtsA", 1)     # attention-phase weights
        wpF = P("wtsF", 1)     # ffn-phase weights
        winp = P("win", winb)  # per-pair working tiles
        ep = P("eptiles", epb)  # P tiles
        etp = P("ettiles", 2)   # exp tiles (short-lived)
        sqp = P("sqtiles", sqb)  # squared-x tiles for stats
        scp = P("sctiles", scb)  # recip rows [128,144]
        rowp = P("rows", 2)    # LN stat rows (short-lived, per region)
        hp = P("hbuf", hbb)
        xbp = P("xbcast", 2)
        xb2p = P("xb2cast", 2)
        psmm = P("psmm", pb[0], space="PSUM")
        psaux = P("psaux", 3, space="PSUM")

        # ---- persistent tiles ----
        X = xp.tile([128, 4, T + 16], F32, tag="X")
        for tq in range(NCH):
            xin = sqp.tile([128, 4, 512], BF16, tag="xsq2")
            nc.sync.dma_start(out=xin,
                              in_=d["x"][:, :, tq * 512:(tq + 1) * 512])
            nc.vector.tensor_copy(out=X[:, :, tq * 512:(tq + 1) * 512], in_=xin)
        ones = cons.tile([128, 1], BF16, tag="ones")       # value 1/512
        nc.sync.dma_start(out=ones, in_=cst["c_ones"])
        ones1 = cons.tile([1, 128], BF16, tag="ones1")     # bcast lhsT (1.0)
        nc.sync.dma_start(out=ones1, in_=cst["c_ones1"])
        e2 = cons.tile([64, 128], BF16, tag="e2")
        nc.sync.dma_start(out=e2, in_=cst["c_e2"])
        eps128 = cons.tile([128, 1], F32, tag="eps128")
        nc.vector.memset(eps128, EPS)
        onesf = cons.tile([128, 1], F32, tag="onesf")
        nc.vector.memset(onesf, 1.0 / 512.0)
        smats = [cons.tile([64, 144], BF16, tag=f"smat{i}", name=f"smat{i}")
                 for i in range(8)]
        for t in smats:
            nc.vector.memset(t, 0.0)

        for l in range(NL):
            # layer weights (attention set + rows)
            wq = wpA.tile([128, 4, 512], BF16, tag="wq")
            wk = wpA.tile([128, 4, 512], BF16, tag="wk")
            wv = wpA.tile([128, 4, 512], BF16, tag="wv")
            wo = wpA.tile([128, 4, 512], BF16, tag="wo")
            eb = wpA.tile([128, NH, 288], BF16, tag="expb")
            bq = wpA.tile([128, 4], F32, tag="bq")
            bk = wpA.tile([128, 4], F32, tag="bk")
            bo = wpA.tile([128, 4], F32, tag="bo")
            bv = wpA.tile([128, 512], BF16, tag="bvb")
            g1 = wpA.tile([128, 4], F32, tag="g1")
            b1 = wpA.tile([128, 4], F32, tag="b1")
            g2 = wpA.tile([128, 4], F32, tag="g2")
            b2 = wpA.tile([128, 4], F32, tag="b2")
            for nm, t in (("wq", wq), ("wk", wk), ("wv", wv), ("wo", wo),
                          ("expb", eb), ("bq", bq), ("bk", bk), ("bo_c", bo),
                          ("bvb", bv), ("g1", g1), ("b1", b1), ("g2", g2),
                          ("b2", b2)):
                nc.sync.dma_start(out=t, in_=cst[nm][l])
            # ffn weights: issued now, consumed after LN1 (overlaps attention)
            w1 = wpF.tile([128, 4, FF], BF16, tag="w1")
            w2 = wpF.tile([128, 16, 512], BF16, tag="w2")
            bf1 = wpF.tile([128, 16], F32, tag="bf1")
            bf2 = wpF.tile([128, 4], F32, tag="bf2")
            for nm, t in (("w1", w1), ("w2", w2), ("bf1", bf1), ("bf2_c", bf2)):
                nc.sync.dma_start(out=t, in_=cst[nm][l])

            # ---------------- per-region LN (stats already in ps_st) --------
            def ln_region(cs, wdt, ps_st, g, b, last=False):
                # rows: mean (bf16), mean^2, var, ln(var+eps), rstd=exp(-.5ln)
                srow = rowp.tile([1, 512], BF16, tag="srow", name="srow")[:, :wdt]
                nc.scalar.activation(out=srow, in_=ps_st[0:1, :wdt],
                                     func=AF.Copy)
                m2 = rowp.tile([1, 512], F32, tag="m2row", name="m2")[:, :wdt]
                nc.vector.tensor_mul(m2, srow, srow)
                nc.vector.tensor_sub(m2, ps_st[32:33, :wdt], m2)   # var, in place
                nc.scalar.activation(out=m2, in_=m2, func=AF.Ln,
                                     bias=eps128[0:1, :])          # ln(var+eps)
                rrow = rowp.tile([1, 512], BF16, tag="rrow", name="rrow")[:, :wdt]
                nc.scalar.activation(out=rrow, in_=m2, func=AF.Exp, scale=-0.5)
                # broadcast rows to 128 partitions on the PE; stage to SBUF
                # bf16 immediately so the PSUM bank frees fast (tag "bc"
                # bufs=1 -> bm/br serialize through one bank)
                ps_bm = psaux.tile([128, 512], F32, tag="bc", name="ps_bm",
                                   bufs=1)[:, :wdt]
                nc.tensor.matmul(ps_bm, lhsT=ones1, rhs=srow,
                                 start=True, stop=True)
                bm = rowp.tile([128, 512], BF16, tag="bmsb", name="bm")[:, :wdt]
                nc.vector.tensor_copy(out=bm, in_=ps_bm)
                ps_br = psaux.tile([128, 512], F32, tag="bc", name="ps_br",
                                   bufs=1)[:, :wdt]
                nc.tensor.matmul(ps_br, lhsT=ones1, rhs=rrow,
                                 start=True, stop=True)
                br = rowp.tile([128, 512], BF16, tag="brsb", name="br")[:, :wdt]
                nc.vector.tensor_copy(out=br, in_=ps_br)
                xc = X[:, :, cs:cs + wdt]
                nc.vector.tensor_sub(xc, xc, _insdim_ap(bm, 0, 4))
                nc.vector.tensor_mul(xc, xc, _insdim_ap(br, 0, 4))
                ob = None
                if not trivial_gb:
                    if last:
                        ob = sqp.tile([128, 4, 512], BF16, tag="xsq2",
                                      name="ob")
                    for mc in range(4):
                        dst = ob[:, mc, 0:wdt] if last else X[:, mc, cs:cs + wdt]
                        nc.scalar.activation(out=dst, in_=X[:, mc, cs:cs + wdt],
                                             func=AF.Identity,
                                             bias=b[:, mc:mc + 1],
                                             scale=g[:, mc:mc + 1])
                elif last:
                    ob = sqp.tile([128, 4, 512], BF16, tag="xsq2", name="ob")
                    nc.vector.tensor_copy(out=ob[:, :, 0:wdt], in_=xc)
                if last:
                    nc.sync.dma_start(out=d["out"][:, :, cs:cs + wdt],
                                      in_=ob[:, :, 0:wdt])

            def stats_mms(xs_sl, xsq_sl, wdt):
                # mean at partition 0 (f32 MM), mean-square at partition 32
                # (bf16 MM). Lives in the fast-draining "mm" ring.
                ps_st = psmm.tile([33, 512], F32, tag="mm", name="ps_st")
                for kc in range(4):
                    nc.tensor.matmul(ps_st[0:1, :wdt], lhsT=onesf,
                                     rhs=xs_sl[:, kc, :],
                                     start=(kc == 0), stop=(kc == 3))
                for kc in range(4):
                    nc.tensor.matmul(ps_st[32:33, :wdt], lhsT=ones,
                                     rhs=xsq_sl[:, kc, :],
                                     start=(kc == 0), stop=(kc == 3))
                return ps_st

            # ---------------- per-pair attention emitter --------------------
            cast_cache = {}

            def cast_pair(p):
                cs0 = p * 288
                xb = xbp.tile([128, 4, 304], BF16, tag="xb")
                nc.gpsimd.tensor_copy(out=xb, in_=X[:, :, cs0:cs0 + 304])
                # tail tokens of both windows packed at cols {0:16, 32:48}
                xt = xbp.tile([128, 4, 64], BF16, tag="xt")
                nc.gpsimd.tensor_copy(
                    out=_insdim_ap(xt[:, :, 0:16], 32, 2, at=2),
                    in_=_insdim_ap(X[:, :, cs0 + 128:cs0 + 144], 144, 2, at=2))
                cast_cache[p] = (xb, xt)

            def att_pair(p):
                cs0 = p * 288
                xs = X[:, :, cs0:cs0 + 288]
                xb, xt = cast_cache.pop(p)
                qw = winp.tile([128, 4, 288], BF16, tag="qw")
                kw = winp.tile([128, 4, 288], BF16, tag="kw")
                for mc in range(4):
                    pq = psmm.tile([128, 288], F32, tag="mm")
                    for kc in range(4):
                        nc.tensor.matmul(pq, lhsT=wq[:, kc, mc * 128:(mc + 1) * 128],
                                         rhs=xb[:, kc, 0:288], start=(kc == 0), stop=(kc == 3))
                    if trivial_bias:
                        nc.vector.tensor_copy(out=qw[:, mc, :], in_=pq)
                    else:
                        nc.scalar.activation(out=qw[:, mc, :], in_=pq, func=AF.Identity,
                                             bias=bq[:, mc:mc + 1])
                    pk = psmm.tile([128, 288], F32, tag="mm")
                    for kc in range(4):
                        nc.tensor.matmul(pk, lhsT=wk[:, kc, mc * 128:(mc + 1) * 128],
                                         rhs=xb[:, kc, 0:288], start=(kc == 0), stop=(kc == 3))
                    nc.scalar.activation(out=kw[:, mc, :], in_=pk, func=AF.Identity,
                                         bias=bk[:, mc:mc + 1])

                vws = []
                for wi in (0, 1):
                    vw1 = winp.tile([128, NH, 65], BF16, tag=f"vw1_{wi}")
                    off = wi * 144
                    pv1 = psmm.tile([128, 512], F32, tag="mm")
                    for kc in range(4):
                        nc.tensor.matmul(pv1, lhsT=xb[:, kc, off:off + 128],
                                         rhs=wv[:, kc, :], start=(kc == 0), stop=(kc == 3))
                    nc.vector.tensor_add(out=vw1[:, :, 0:64],
                                         in0=pv1.rearrange("p (h e) -> p h e", h=NH),
                                         in1=bv.rearrange("p (h e) -> p h e", h=NH))
                    nc.vector.memset(vw1[:, :, 64:65], 1.0)
                    vws.append(vw1)
                # merged tail-V for both windows: lhsT cols {128:160, 272:304}
                # -> out partitions A-tail 0:16, (garbage 16:32), B-tail 32:48
                vw2p = winp.tile([64, NH, 65], BF16, tag="vw2p")
                pv2 = psmm.tile([64, 512], F32, tag="mm")
                for kc in range(4):
                    nc.tensor.matmul(pv2, lhsT=xt[:, kc, :], rhs=wv[:, kc, :],
                                     start=(kc == 0), stop=(kc == 3))
                nc.vector.tensor_add(out=vw2p[:, :, 0:64],
                                     in0=pv2.rearrange("p (h e) -> p h e", h=NH),
                                     in1=bv[0:64].rearrange("p (h e) -> p h e", h=NH))
                nc.vector.memset(vw2p[:, :, 64:65], 1.0)

                ocm = winp.tile([128, 4, 288], BF16, tag="ocm")
                if skip_heads:
                    nc.vector.tensor_copy(out=ocm, in_=xs)

                # software-pipelined head loop: stage A (S-mm, exp, P-mul) runs
                # `LOOKAHEAD` heads in front of stage B (PV, den) and stage C
                # (per head-pair: e2 bcast-mm, recip, ocm scale), so the PE has
                # independent matmuls queued while act/DVE chew on earlier heads.
                heads = [(wi, h) for wi in (0, 1) for h in range(NH)]
                pts = {}
                psos = {}

                def stage_a(wi, h):
                    off = wi * 144
                    tb = 32 * wi
                    ro, tl = (h % 2) * 64, h // 2
                    ps_s = psmm.tile([128, 288], F32, tag="mm")
                    nc.tensor.matmul(ps_s[:, 0:144],
                                     lhsT=kw[ro:ro + 64, tl, off:off + 128],
                                     rhs=qw[ro:ro + 64, tl, off:off + 144],
                                     start=True, stop=True)
                    nc.tensor.matmul(ps_s[tb:tb + 16, 144:288],
                                     lhsT=kw[ro:ro + 64, tl, off + 128:off + 144],
                                     rhs=qw[ro:ro + 64, tl, off:off + 144],
                                     start=True, stop=True)
                    et = etp.tile([128, 288], BF16, tag="e")
                    nc.scalar.activation(out=et, in_=ps_s, func=AF.Exp)
                    pt = ep.tile([128, 288], BF16, tag="p")
                    nc.vector.tensor_mul(pt, et, eb[:, h, :])
                    pts[(wi, h)] = pt

                def stage_b(wi, h):
                    pt = pts.pop((wi, h))
                    vw1 = vws[wi]
                    smat = smats[wi * 4 + h // 2]
                    ps_o = psaux.tile([65, 144], F32, tag="aux")
                    nc.tensor.matmul(ps_o, lhsT=vw1[:, h, :], rhs=pt[:, 0:144],
                                     start=True, stop=False)
                    tb = 32 * wi
                    nc.tensor.matmul(ps_o, lhsT=vw2p[tb:tb + 16, h, :],
                                     rhs=pt[tb:tb + 16, 144:288],
                                     start=False, stop=True)
                    nc.scalar.activation(out=smat[32 * (h % 2):32 * (h % 2) + 1, :],
                                         in_=ps_o[64:65, 0:144], func=AF.Copy)
                    psos[(wi, h)] = ps_o
                    if h % 2 == 1:
                        stage_c(wi, h // 2, smat)

                def stage_c(wi, hpair, smat):
                    off = wi * 144
                    ps_sc = psmm.tile([128, 144], F32, tag="mm")
                    nc.tensor.matmul(ps_sc, lhsT=e2, rhs=smat, start=True, stop=True)
                    sc = scp.tile([128, 144], F32, tag="scsb")
                    nc.vector.reciprocal_approx_fast(out=sc, in_=ps_sc)
                    p0 = psos.pop((wi, 2 * hpair))
                    p1 = psos.pop((wi, 2 * hpair + 1))
                    nc.vector.tensor_mul(ocm[0:64, hpair, off:off + 144],
                                         p0[0:64, :], sc[0:64, :])
                    nc.vector.tensor_mul(ocm[64:128, hpair, off:off + 144],
                                         p1[0:64, :], sc[64:128, :])

                LOOKAHEAD = la
                for i, (wi, h) in enumerate(heads if not skip_heads else []):
                    stage_a(wi, h)
                    if i >= LOOKAHEAD:
                        stage_b(*heads[i - LOOKAHEAD])
                for j in (range(max(0, len(heads) - LOOKAHEAD), len(heads))
                          if not skip_heads else []):
                    stage_b(*heads[j])

                # O projection + residual -> X (pre-LN1), stats, LN1
                for mc in range(4):
                    po = psmm.tile([128, 288], F32, tag="mm")
                    for kc in range(4):
                        nc.tensor.matmul(po, lhsT=wo[:, kc, mc * 128:(mc + 1) * 128],
                                         rhs=ocm[:, kc, :], start=(kc == 0), stop=(kc == 3))
                    nc.vector.tensor_add(out=X[:, mc, cs0:cs0 + 288], in0=po,
                                         in1=X[:, mc, cs0:cs0 + 288])
                    if not trivial_bias:
                        nc.vector.tensor_add(out=X[:, mc, cs0:cs0 + 288],
                                             in0=X[:, mc, cs0:cs0 + 288],
                                             in1=bo[:, mc:mc + 1].broadcast_to([128, 288]))
                xsq = sqp.tile([128, 4, 288], BF16, tag="xsq")
                nc.gpsimd.tensor_mul(xsq, xs, xs)
                ps_st = stats_mms(xs, xsq, 288)
                ln_region(cs0, 288, ps_st, g1, b1)

            # ---------------- FFN chunk emitter (incl. LN2) ----------------
            lastl = (l == NL - 1)

            def ffn_chunk(cc):
                cs = cc * 512
                xc = X[:, :, cs:cs + 512]
                xb2 = xb2p.tile([128, 4, 512], BF16, tag="xb2")
                for kc in range(4):
                    nc.gpsimd.tensor_copy(out=xb2[:, kc, :], in_=xc[:, kc, :])
                hb = hp.tile([128, 16, 512], BF16, tag="hb")
                for fc in range(16):
                    ph = psmm.tile([128, 512], F32, tag="mm")
                    for kc in range(4):
                        nc.tensor.matmul(ph, lhsT=w1[:, kc, fc * 128:(fc + 1) * 128],
                                         rhs=xb2[:, kc, :], start=(kc == 0), stop=(kc == 3))
                    if fc % 2 == 0:
                        nc.scalar.activation(out=hb[:, fc, :], in_=ph, func=AF.Relu,
                                             bias=bf1[:, fc:fc + 1])
                    else:
                        nc.vector.tensor_scalar(
                            out=hb[:, fc, :], in0=ph, scalar1=bf1[:, fc:fc + 1],
                            scalar2=0.0, op0=mybir.AluOpType.add,
                            op1=mybir.AluOpType.max)
                for mc in range(4):
                    pf = psmm.tile([128, 512], F32, tag="mm")
                    for fc in range(16):
                        nc.tensor.matmul(pf, lhsT=w2[:, fc, mc * 128:(mc + 1) * 128],
                                         rhs=hb[:, fc, :], start=(fc == 0), stop=(fc == 15))
                    nc.vector.tensor_add(out=X[:, mc, cs:cs + 512], in0=pf,
                                         in1=X[:, mc, cs:cs + 512])
                    if not trivial_bias:
                        nc.vector.tensor_add(out=X[:, mc, cs:cs + 512],
                                             in0=X[:, mc, cs:cs + 512],
                                             in1=bf2[:, mc:mc + 1].broadcast_to([128, 512]))
                xsq = sqp.tile([128, 4, 512], BF16, tag="xsq2")
                nc.gpsimd.tensor_mul(xsq, xc, xc)
                ps_st = stats_mms(xc, xsq, 512)
                ln_region(cs, 512, ps_st, g2, b2, lastl)

            # ---------------- layer schedule -----------------------------
            if not skip_attn:
                nxt = 0
                cast_pair(0)
                for p in range(PAIRS):
                    if p + 1 < PAIRS:
                        cast_pair(p + 1)
                    att_pair(p)
                    if not skip_ffn:
                        while nxt < NCH and ((nxt + 1) * 512 <= (p + 1) * 288
                                             or p == PAIRS - 1):
                            ffn_chunk(nxt)
                            nxt += 1
            elif not skip_ffn:
                for cc in range(NCH):
                    ffn_chunk(cc)
            if skip_ffn and l == NL - 1:
                for cc in range(NCH):
                    cs = cc * 512
                    nc.sync.dma_start(out=d["out"][:, :, cs:cs + 512],
                                      in_=X[:, :, cs:cs + 512])

    return d


# ---------------------------------------------------------------------------
# Host-side packing + golden model
# ---------------------------------------------------------------------------

def rel_idx():
    coords = np.stack(np.meshgrid(np.arange(WS), np.arange(WS), indexing="ij"))
    flat = coords.reshape(2, -1)
    rel = (flat[:, :, None] - flat[:, None, :]).transpose(1, 2, 0).copy()
    rel[..., 0] += WS - 1
    rel[..., 1] += WS - 1
    rel[..., 0] *= 2 * WS - 1
    return rel.sum(-1)  # [N, N] int


def pack_weights(w, NL):
    """w: dict of reference arrays -> dict of const arrays (np)."""
    bf = ml_dtypes.bfloat16
    scale = HD ** -0.5
    ridx = rel_idx()
    out = {}

    def lhsT_pack(W, kchunks):  # [Cin, Cout] -> [128, kchunks, Cout]
        return np.ascontiguousarray(
            W.reshape(kchunks, 128, W.shape[1]).transpose(1, 0, 2)
        )

    wq = np.stack([lhsT_pack(w["Wq"][l] * scale, 4) for l in range(NL)])
    wk = np.stack([lhsT_pack(w["Wk"][l], 4) for l in range(NL)])
    wv = np.stack([lhsT_pack(w["Wv"][l], 4) for l in range(NL)])
    wo = np.stack([lhsT_pack(w["Wo"][l], 4) for l in range(NL)])
    w1 = np.stack([lhsT_pack(w["W1"][l], 4) for l in range(NL)])
    w2 = np.stack([lhsT_pack(w["W2"][l], 16) for l in range(NL)])
    for nm, arr in (("wq", wq), ("wk", wk), ("wv", wv), ("wo", wo),
                    ("w1", w1), ("w2", w2)):
        out[nm] = arr.astype(bf)

    expb = np.zeros((NL, 128, NH, 288), np.float32)
    for l in range(NL):
        bias = w["rpb"][l][ridx]            # [N(i), N(j), NH]
        ebT = np.exp(bias.transpose(2, 1, 0))  # [NH, j, i]
        expb[l, 0:128, :, 0:144] = ebT[:, 0:128, :].transpose(1, 0, 2)
        expb[l, 0:16, :, 144:288] = ebT[:, 128:144, :].transpose(1, 0, 2)
        expb[l, 32:48, :, 144:288] = ebT[:, 128:144, :].transpose(1, 0, 2)
    out["expb"] = expb.astype(bf)

    def percol(b):  # [NL, C] -> [NL, 128, 4]
        return np.ascontiguousarray(
            b.reshape(NL, 4, 128).transpose(0, 2, 1)).astype(np.float32)

    out["bq"] = percol(w["bq"] * scale)
    out["bk"] = percol(w["bk"])
    out["bo_c"] = percol(w["bo"])
    out["bf2_c"] = percol(w["bf2"])
    out["c_ones1"] = np.ones((1, 128), bf)
    e2 = np.zeros((64, 128), np.float32)
    e2[0, 0:64] = 1.0
    e2[32, 64:128] = 1.0
    out["c_e2"] = e2.astype(bf)
    out["g1"] = percol(w["g1"])
    out["b1"] = percol(w["b1"])
    out["g2"] = percol(w["g2"])
    out["b2"] = percol(w["b2"])
    out["bf1"] = np.ascontiguousarray(
        w["bf1"].reshape(NL, 16, 128).transpose(0, 2, 1)).astype(np.float32)
    out["bvb"] = np.broadcast_to(
        w["bv"].astype(bf)[:, None, :], (NL, 128, 512)).copy()
    out["c_ones"] = np.full((128, 1), 1.0 / 512.0, bf)
    return out


def golden_tm(x_tm, w, NL):
    """fp32 numpy reference on window-major token-major x [T, 512]."""
    T = x_tm.shape[0]
    NW = T // N
    ridx = rel_idx()
    x = x_tm.astype(np.float32)

    def ln(v, g, b):
        m = v.mean(-1, keepdims=True)
        s = v.var(-1, keepdims=True)
        return (v - m) / np.sqrt(s + EPS) * g + b

    for l in range(NL):
        xw = x.reshape(NW, N, C)
        q = (xw @ w["Wq"][l] + w["bq"][l]).reshape(NW, N, NH, HD).transpose(0, 2, 1, 3)
        k = (xw @ w["Wk"][l] + w["bk"][l]).reshape(NW, N, NH, HD).transpose(0, 2, 1, 3)
        v = (xw @ w["Wv"][l] + w["bv"][l]).reshape(NW, N, NH, HD).transpose(0, 2, 1, 3)
        bias = w["rpb"][l][ridx].transpose(2, 0, 1)
        attn = np.einsum("whid,whjd->whij", q, k) * (HD ** -0.5) + bias
        attn = attn - attn.max(-1, keepdims=True)
        p = np.exp(attn)
        p = p / p.sum(-1, keepdims=True)
        o = np.einsum("whij,whjd->whid", p, v).transpose(0, 2, 1, 3).reshape(NW, N, C)
        o = o @ w["Wo"][l] + w["bo"][l]
        x = ln(o.reshape(T, C) + x, w["g1"][l], w["b1"][l])
        h = np.maximum(x @ w["W1"][l] + w["bf1"][l], 0.0) @ w["W2"][l] + w["bf2"][l]
        x = ln(h + x, w["g2"][l], w["b2"][l])
    return x


# ---------------------------------------------------------------------------
# kernel() entry point: full inputs -> full output, 8-way batch data parallel
# ---------------------------------------------------------------------------

NCORES = 8
B_FULL = 64
H_RES = W_RES = 24
L_TOK = H_RES * W_RES
NW_FULL = (B_FULL // NCORES) * (H_RES // WS) * (W_RES // WS)   # 32 windows/core
NL_FULL = 3

_COMPILED = {}


def _pack_x_all(x):
    """[64, 576, 512] f32 -> [8, 128, 4, T] bf16 channel-major window-major."""
    b = x.reshape(NCORES, B_FULL // NCORES, 2, WS, 2, WS, 4, 128)
    v = b.transpose(0, 7, 6, 1, 2, 4, 3, 5)   # [core,128,4, b,hw,ww,hs,ws]
    return np.ascontiguousarray(v.reshape(NCORES, 128, 4, -1)
                                ).astype(ml_dtypes.bfloat16)


def _unpack_out_all(res_list):
    """list of [128, 4, T] -> [64, 576, 512] f32."""
    y = np.stack([r.astype(np.float32) for r in res_list])     # [8,128,4,T]
    bpc = B_FULL // NCORES
    v = y.reshape(NCORES, 128, 4, bpc, 2, 2, WS, WS)
    v = v.transpose(0, 3, 4, 6, 5, 7, 2, 1)   # [core,b,hw,hs,ww,ws,4,128]
    return np.ascontiguousarray(v.reshape(B_FULL, L_TOK, C))


def kernel(x, Wq, bq, Wk, bk, Wv, bv, Wo, bo, rpb,
           g1, b1, W1, bf1, W2, bf2, g2, b2):
    import hashlib
    from concourse.bass_utils import run_bass_kernel_spmd

    w = {"Wq": np.asarray(Wq, np.float32), "bq": np.asarray(bq, np.float32),
         "Wk": np.asarray(Wk, np.float32), "bk": np.asarray(bk, np.float32),
         "Wv": np.asarray(Wv, np.float32), "bv": np.asarray(bv, np.float32),
         "Wo": np.asarray(Wo, np.float32), "bo": np.asarray(bo, np.float32),
         "rpb": np.asarray(rpb, np.float32),
         "g1": np.asarray(g1, np.float32), "b1": np.asarray(b1, np.float32),
         "W1": np.asarray(W1, np.float32), "bf1": np.asarray(bf1, np.float32),
         "W2": np.asarray(W2, np.float32), "bf2": np.asarray(bf2, np.float32),
         "g2": np.asarray(g2, np.float32), "b2": np.asarray(b2, np.float32)}
    hsh = hashlib.blake2b(
        b"".join(np.ascontiguousarray(v).tobytes() for v in w.values()),
        digest_size=16).hexdigest()
    if _COMPILED.get("hash") != hsh:
        packed = pack_weights(w, NL_FULL)
        trivial_gb = bool(np.all(w["g1"] == 1) and np.all(w["b1"] == 0)
                          and np.all(w["g2"] == 1) and np.all(w["b2"] == 0))
        trivial_bias = bool(all(np.all(w[k] == 0)
                                for k in ("bq", "bk", "bv", "bo", "bf1", "bf2")))
        nc = bacc.Bacc("TRN2", target_bir_lowering=False, debug=False)
        build(nc, NW_FULL, NL_FULL, packed,
              trivial_gb=trivial_gb, trivial_bias=trivial_bias)
        nc.compile()
        _COMPILED.update(hash=hsh, nc=nc)

    xp = _pack_x_all(np.asarray(x, np.float32))
    in_maps = [{"x": xp[i]} for i in range(NCORES)]
    res = run_bass_kernel_spmd(_COMPILED["nc"], in_maps, list(range(NCORES)))
    return _unpack_out_all([res.results[i]["out"] for i in range(NCORES)])



# revision 22
# speedup vs baseline: 1.0495x; 1.0309x over previous
"""Swin-style window-attention encoder as a Bass/Tile kernel for TRN2 — v3.

Key design vs v1:
- Residual master X lives in SBUF as FP32 [128, 4, T+16] (channel-major) —
  the residual stream never rounds to bf16 (bf16 master measured 2.2e-2 max
  rel err, over the 2e-2 budget; f32 master 7.8e-3). Matmul inputs are
  bf16 copies cast on the (otherwise idle) GPSIMD engine; per-token mean
  stats contract the f32 master directly (f32 matmul, tiny N).
- Weights are baked into the NEFF as inline consts — per-launch IO is just
  x (bf16 in) and out (bf16).
- Per-layer phase batching: [attention (Exp table)] -> [LN1 rows (Sqrt) +
  apply] -> [FFN (Relu, no table load)] -> [LN2 rows + apply]. 2 activation
  table loads per layer instead of ~64.
- Attention softmax denominators: collected per head into smat rows (act
  Copy), broadcast to 128 partitions via one e2 matmul, ONE fat [128,144]
  DVE reciprocal per head-pair (no 1-lane recips).
- LN row math on [128,36] shuffled layout (SBUF->SBUF strided DMA), not
  1-lane [1,T] ops.
- O-proj / QK-proj / stats at window-pair (288 tokens) granularity; FFN at
  512-token chunks.
"""
from contextlib import ExitStack

import numpy as np
import ml_dtypes

import concourse.bass as bass
import concourse.bacc as bacc
import concourse.tile as tile
import concourse.mybir as mybir

F32 = mybir.dt.float32
F32R = mybir.dt.float32r
BF16 = mybir.dt.bfloat16
AF = mybir.ActivationFunctionType


def _patch_act_tables():
    """Make the act-table-load pass resolve Exp AND Ln to the combined
    `natural_log_exp_and_others` set (it otherwise greedily alternates
    between `exp_and_others` and `natural_log`, reloading tables at every
    attention<->LN boundary, ~150 loads/kernel). We hide exp/ln from every
    other set in the table list the pass consults; set IDs (list order)
    are unchanged, so walrus still emits the right act.json entries."""
    import concourse.hw_specs as hw_specs

    if getattr(bacc, "_ant_act_tables_patched", False):
        return
    orig = hw_specs.get_activation_tables

    def patched(arch):
        tabs = orig(arch)
        exp, ln = AF.Exp, AF.Ln
        if "natural_log_exp_and_others" in tabs:
            for name, fns in tabs.items():
                if name != "natural_log_exp_and_others":
                    fns.discard(exp)
                    fns.discard(ln)
        return tabs

    bacc.get_activation_tables = patched
    bacc._ant_act_tables_patched = True


_patch_act_tables()

WS = 12
N = WS * WS          # 144 tokens per window
C = 512
NH = 8
HD = 64
FF = 2048
EPS = 1e-5


def _insdim_ap(row_ap, stride, num, at=1):
    """Insert a dim of (stride, num) at position `at` of the AP (default:
    right after the partition dim). stride=0 -> broadcast; else gather."""
    dims = [list(d) for d in row_ap.ap]
    return bass.AP(
        tensor=row_ap.tensor,
        offset=row_ap.offset,
        ap=dims[:at] + [[stride, num]] + dims[at:],
    )


def _bcast_ap(row_ap, parts):
    return _insdim_ap(row_ap, 0, parts)


def build(nc: bass.Bass, NW: int, NL: int, w: dict,
          skip_attn=False, skip_ffn=False, skip_heads=False,
          pb=(4, 4), winb=2, epb=3, sqb=1, hbb=1, scb=2, la=3, xbb=2,
          trivial_gb=False, trivial_bias=False):
    """w: packed numpy weight dict (see pack_weights)."""
    T = NW * N
    PAIRS = NW // 2
    NCH = T // 512
    assert T % 512 == 0

    d = {}
    d["x"] = nc.dram_tensor("x", [128, 4, T], BF16, kind="ExternalInput").ap()
    d["out"] = nc.dram_tensor("out", [128, 4, T], BF16, kind="ExternalOutput").ap()
    cst = {nm: nc.inline_tensor(arr, name=nm).ap() for nm, arr in w.items()}

    with tile.TileContext(nc) as tc, ExitStack() as ctx:
        P = lambda name, bufs, **kw: ctx.enter_context(
            tc.tile_pool(name=name, bufs=bufs, **kw)
        )
        xp = P("xmaster", 1)
        cons = P("consts", 1)
        wpA = P("wtsA", 1)     # attention-phase weights
        wpF = P("wtsF", 1)     # ffn-phase weights
        winp = P("win", winb)  # per-pair working tiles
        ep = P("eptiles", epb)  # P tiles
        etp = P("ettiles", 2)   # exp tiles (short-lived)
        sqp = P("sqtiles", sqb)  # squared-x tiles for stats
        scp = P("sctiles", scb)  # recip rows [128,144]
        rowp = P("rows", 2)    # LN stat rows (short-lived, per region)
        hp = P("hbuf", hbb)
        xbp = P("xbcast", 2)
        xb2p = P("xb2cast", 2)
        psmm = P("psmm", pb[0], space="PSUM")
        psaux = P("psaux", 3, space="PSUM")

        # ---- persistent tiles ----
        X = xp.tile([128, 4, T + 16], F32, tag="X")
        for tq in range(NCH):
            xin = sqp.tile([128, 4, 512], BF16, tag="xsq2")
            nc.sync.dma_start(out=xin,
                              in_=d["x"][:, :, tq * 512:(tq + 1) * 512])
            nc.vector.tensor_copy(out=X[:, :, tq * 512:(tq + 1) * 512], in_=xin)
        ones = cons.tile([128, 1], BF16, tag="ones")       # value 1/512
        nc.sync.dma_start(out=ones, in_=cst["c_ones"])
        ones1 = cons.tile([1, 128], BF16, tag="ones1")     # bcast lhsT (1.0)
        nc.sync.dma_start(out=ones1, in_=cst["c_ones1"])
        e2 = cons.tile([64, 128], BF16, tag="e2")
        nc.sync.dma_start(out=e2, in_=cst["c_e2"])
        eps128 = cons.tile([128, 1], F32, tag="eps128")
        nc.vector.memset(eps128, EPS)
        onesf = cons.tile([128, 1], F32, tag="onesf")
        nc.vector.memset(onesf, 1.0 / 512.0)
        smats = [cons.tile([64, 144], BF16, tag=f"smat{i}", name=f"smat{i}")
                 for i in range(8)]
        for t in smats:
            nc.vector.memset(t, 0.0)

        for l in range(NL):
            # layer weights (attention set + rows)
            wq = wpA.tile([128, 4, 512], BF16, tag="wq")
            wk = wpA.tile([128, 4, 512], BF16, tag="wk")
            wv = wpA.tile([128, 4, 512], BF16, tag="wv")
            wo = wpA.tile([128, 4, 512], BF16, tag="wo")
            eb = wpA.tile([128, NH, 288], BF16, tag="expb")
            bq = wpA.tile([128, 4], F32, tag="bq")
            bk = wpA.tile([128, 4], F32, tag="bk")
            bo = wpA.tile([128, 4], F32, tag="bo")
            bv = wpA.tile([128, 512], BF16, tag="bvb")
            g1 = wpA.tile([128, 4], F32, tag="g1")
            b1 = wpA.tile([128, 4], F32, tag="b1")
            g2 = wpA.tile([128, 4], F32, tag="g2")
            b2 = wpA.tile([128, 4], F32, tag="b2")
            for nm, t in (("wq", wq), ("wk", wk), ("wv", wv), ("wo", wo),
                          ("expb", eb), ("bq", bq), ("bk", bk), ("bo_c", bo),
                          ("bvb", bv), ("g1", g1), ("b1", b1), ("g2", g2),
                          ("b2", b2)):
                nc.sync.dma_start(out=t, in_=cst[nm][l])
            # ffn weights: issued now, consumed after LN1 (overlaps attention)
            w1 = wpF.tile([128, 4, FF], BF16, tag="w1")
            w2 = wpF.tile([128, 16, 512], BF16, tag="w2")
            bf1 = wpF.tile([128, 16], F32, tag="bf1")
            bf2 = wpF.tile([128, 4], F32, tag="bf2")
            for nm, t in (("w1", w1), ("w2", w2), ("bf1", bf1), ("bf2_c", bf2)):
                nc.sync.dma_start(out=t, in_=cst[nm][l])

            # ---------------- per-region LN (stats already in ps_st) --------
            def ln_region(cs, wdt, ps_st, g, b, last=False):
                # rows: mean (bf16), mean^2, var, ln(var+eps), rstd=exp(-.5ln)
                srow = rowp.tile([1, 512], BF16, tag="srow", name="srow")[:, :wdt]
                nc.scalar.activation(out=srow, in_=ps_st[0:1, :wdt],
                                     func=AF.Copy)
                m2 = rowp.tile([1, 512], F32, tag="m2row", name="m2")[:, :wdt]
                nc.vector.tensor_mul(m2, srow, srow)
                nc.vector.tensor_sub(m2, ps_st[32:33, :wdt], m2)   # var, in place
                nc.scalar.activation(out=m2, in_=m2, func=AF.Ln,
                                     bias=eps128[0:1, :])          # ln(var+eps)
                rrow = rowp.tile([1, 512], BF16, tag="rrow", name="rrow")[:, :wdt]
                nc.scalar.activation(out=rrow, in_=m2, func=AF.Exp, scale=-0.5)
                # broadcast rows to 128 partitions on the PE; stage to SBUF
                # bf16 immediately so the PSUM bank frees fast (tag "bc"
                # bufs=1 -> bm/br serialize through one bank)
                ps_bm = psaux.tile([128, 512], F32, tag="bc", name="ps_bm",
                                   bufs=1)[:, :wdt]
                nc.tensor.matmul(ps_bm, lhsT=ones1, rhs=srow,
                                 start=True, stop=True)
                bm = rowp.tile([128, 512], BF16, tag="bmsb", name="bm")[:, :wdt]
                nc.vector.tensor_copy(out=bm, in_=ps_bm)
                ps_br = psaux.tile([128, 512], F32, tag="bc", name="ps_br",
                                   bufs=1)[:, :wdt]
                nc.tensor.matmul(ps_br, lhsT=ones1, rhs=rrow,
                                 start=True, stop=True)
                br = rowp.tile([128, 512], BF16, tag="brsb", name="br")[:, :wdt]
                nc.vector.tensor_copy(out=br, in_=ps_br)
                xc = X[:, :, cs:cs + wdt]
                nc.vector.tensor_sub(xc, xc, _insdim_ap(bm, 0, 4))
                nc.vector.tensor_mul(xc, xc, _insdim_ap(br, 0, 4))
                ob = None
                if not trivial_gb:
                    if last:
                        ob = sqp.tile([128, 4, 512], BF16, tag="xsq2",
                                      name="ob")
                    for mc in range(4):
                        dst = ob[:, mc, 0:wdt] if last else X[:, mc, cs:cs + wdt]
                        nc.scalar.activation(out=dst, in_=X[:, mc, cs:cs + wdt],
                                             func=AF.Identity,
                                             bias=b[:, mc:mc + 1],
                                             scale=g[:, mc:mc + 1])
                elif last:
                    ob = sqp.tile([128, 4, 512], BF16, tag="xsq2", name="ob")
                    nc.vector.tensor_copy(out=ob[:, :, 0:wdt], in_=xc)
                if last:
                    nc.sync.dma_start(out=d["out"][:, :, cs:cs + wdt],
                                      in_=ob[:, :, 0:wdt])

            def stats_mms(xs_sl, xsq_sl, wdt):
                # mean at partition 0 (f32 MM), mean-square at partition 32
                # (bf16 MM). Lives in the fast-draining "mm" ring.
                ps_st = psmm.tile([33, 512], F32, tag="mm", name="ps_st")
                for kc in range(4):
                    nc.tensor.matmul(ps_st[0:1, :wdt], lhsT=onesf,
                                     rhs=xs_sl[:, kc, :],
                                     start=(kc == 0), stop=(kc == 3))
                for kc in range(4):
                    nc.tensor.matmul(ps_st[32:33, :wdt], lhsT=ones,
                                     rhs=xsq_sl[:, kc, :],
                                     start=(kc == 0), stop=(kc == 3))
                return ps_st

            # ---------------- per-pair attention emitter --------------------
            cast_cache = {}

            def cast_pair(p):
                cs0 = p * 288
                xb = xbp.tile([128, 4, 304], BF16, tag="xb")
                nc.gpsimd.tensor_copy(out=xb, in_=X[:, :, cs0:cs0 + 304])
                # tail tokens of both windows packed at cols {0:16, 32:48}
                xt = xbp.tile([128, 4, 64], BF16, tag="xt")
                nc.gpsimd.tensor_copy(
                    out=_insdim_ap(xt[:, :, 0:16], 32, 2, at=2),
                    in_=_insdim_ap(X[:, :, cs0 + 128:cs0 + 144], 144, 2, at=2))
                cast_cache[p] = (xb, xt)

            def att_pair(p):
                cs0 = p * 288
                xs = X[:, :, cs0:cs0 + 288]
                xb, xt = cast_cache.pop(p)
                qw = winp.tile([128, 4, 288], BF16, tag="qw")
                kw = winp.tile([128, 4, 288], BF16, tag="kw")
                for mc in range(4):
                    pq = psmm.tile([128, 288], F32, tag="mm")
                    for kc in range(4):
                        nc.tensor.matmul(pq, lhsT=wq[:, kc, mc * 128:(mc + 1) * 128],
                                         rhs=xb[:, kc, 0:288], start=(kc == 0), stop=(kc == 3))
                    if trivial_bias:
                        nc.vector.tensor_copy(out=qw[:, mc, :], in_=pq)
                    else:
                        nc.scalar.activation(out=qw[:, mc, :], in_=pq, func=AF.Identity,
                                             bias=bq[:, mc:mc + 1])
                    pk = psmm.tile([128, 288], F32, tag="mm")
                    for kc in range(4):
                        nc.tensor.matmul(pk, lhsT=wk[:, kc, mc * 128:(mc + 1) * 128],
                                         rhs=xb[:, kc, 0:288], start=(kc == 0), stop=(kc == 3))
                    nc.scalar.activation(out=kw[:, mc, :], in_=pk, func=AF.Identity,
                                         bias=bk[:, mc:mc + 1])

                vws = []
                for wi in (0, 1):
                    vw1 = winp.tile([128, NH, 65], BF16, tag=f"vw1_{wi}")
                    off = wi * 144
                    pv1 = psmm.tile([128, 512], F32, tag="mm")
                    for kc in range(4):
                        nc.tensor.matmul(pv1, lhsT=xb[:, kc, off:off + 128],
                                         rhs=wv[:, kc, :], start=(kc == 0), stop=(kc == 3))
                    nc.vector.tensor_add(out=vw1[:, :, 0:64],
                                         in0=pv1.rearrange("p (h e) -> p h e", h=NH),
                                         in1=bv.rearrange("p (h e) -> p h e", h=NH))
                    nc.vector.memset(vw1[:, :, 64:65], 1.0)
                    vws.append(vw1)
                # merged tail-V for both windows: lhsT cols {128:160, 272:304}
                # -> out partitions A-tail 0:16, (garbage 16:32), B-tail 32:48
                vw2p = winp.tile([64, NH, 65], BF16, tag="vw2p")
                pv2 = psmm.tile([64, 512], F32, tag="mm")
                for kc in range(4):
                    nc.tensor.matmul(pv2, lhsT=xt[:, kc, :], rhs=wv[:, kc, :],
                                     start=(kc == 0), stop=(kc == 3))
                nc.vector.tensor_add(out=vw2p[:, :, 0:64],
                                     in0=pv2.rearrange("p (h e) -> p h e", h=NH),
                                     in1=bv[0:64].rearrange("p (h e) -> p h e", h=NH))
                nc.vector.memset(vw2p[:, :, 64:65], 1.0)

                ocm = winp.tile([128, 4, 288], BF16, tag="ocm")
                if skip_heads:
                    nc.vector.tensor_copy(out=ocm, in_=xs)

                # software-pipelined head loop: stage A (S-mm, exp, P-mul) runs
                # `LOOKAHEAD` heads in front of stage B (PV, den) and stage C
                # (per head-pair: e2 bcast-mm, recip, ocm scale), so the PE has
                # independent matmuls queued while act/DVE chew on earlier heads.
                heads = [(wi, h) for wi in (0, 1) for h in range(NH)]
                pts = {}
                psos = {}

                def stage_a(wi, h):
                    off = wi * 144
                    tb = 32 * wi
                    ro, tl = (h % 2) * 64, h // 2
                    ps_s = psmm.tile([128, 288], F32, tag="mm")
                    nc.tensor.matmul(ps_s[:, 0:144],
                                     lhsT=kw[ro:ro + 64, tl, off:off + 128],
                                     rhs=qw[ro:ro + 64, tl, off:off + 144],
                                     start=True, stop=True)
                    nc.tensor.matmul(ps_s[tb:tb + 16, 144:288],
                                     lhsT=kw[ro:ro + 64, tl, off + 128:off + 144],
                                     rhs=qw[ro:ro + 64, tl, off:off + 144],
                                     start=True, stop=True)
                    et = etp.tile([128, 288], BF16, tag="e")
                    nc.scalar.activation(out=et, in_=ps_s, func=AF.Exp)
                    pt = ep.tile([128, 288], BF16, tag="p")
                    nc.vector.tensor_mul(pt, et, eb[:, h, :])
                    pts[(wi, h)] = pt

                def stage_b(wi, h):
                    pt = pts.pop((wi, h))
                    vw1 = vws[wi]
                    smat = smats[wi * 4 + h // 2]
                    ps_o = psaux.tile([65, 144], F32, tag="aux")
                    nc.tensor.matmul(ps_o, lhsT=vw1[:, h, :], rhs=pt[:, 0:144],
                                     start=True, stop=False)
                    tb = 32 * wi
                    nc.tensor.matmul(ps_o, lhsT=vw2p[tb:tb + 16, h, :],
                                     rhs=pt[tb:tb + 16, 144:288],
                                     start=False, stop=True)
                    nc.scalar.activation(out=smat[32 * (h % 2):32 * (h % 2) + 1, :],
                                         in_=ps_o[64:65, 0:144], func=AF.Copy)
                    psos[(wi, h)] = ps_o
                    if h % 2 == 1:
                        stage_c(wi, h // 2, smat)

                def stage_c(wi, hpair, smat):
                    off = wi * 144
                    ps_sc = psmm.tile([128, 144], F32, tag="mm")
                    nc.tensor.matmul(ps_sc, lhsT=e2, rhs=smat, start=True, stop=True)
                    sc = scp.tile([128, 144], F32, tag="scsb")
                    nc.vector.reciprocal_approx_fast(out=sc, in_=ps_sc)
                    p0 = psos.pop((wi, 2 * hpair))
                    p1 = psos.pop((wi, 2 * hpair + 1))
                    nc.vector.tensor_mul(ocm[0:64, hpair, off:off + 144],
                                         p0[0:64, :], sc[0:64, :])
                    nc.vector.tensor_mul(ocm[64:128, hpair, off:off + 144],
                                         p1[0:64, :], sc[64:128, :])

                LOOKAHEAD = la
                for i, (wi, h) in enumerate(heads if not skip_heads else []):
                    stage_a(wi, h)
                    if i >= LOOKAHEAD:
                        stage_b(*heads[i - LOOKAHEAD])
                for j in (range(max(0, len(heads) - LOOKAHEAD), len(heads))
                          if not skip_heads else []):
                    stage_b(*heads[j])

                # O projection + residual -> X (pre-LN1), stats, LN1
                for mc in range(4):
                    po = psmm.tile([128, 288], F32, tag="mm")
                    for kc in range(4):
                        nc.tensor.matmul(po, lhsT=wo[:, kc, mc * 128:(mc + 1) * 128],
                                         rhs=ocm[:, kc, :], start=(kc == 0), stop=(kc == 3))
                    nc.vector.tensor_add(out=X[:, mc, cs0:cs0 + 288], in0=po,
                                         in1=X[:, mc, cs0:cs0 + 288])
                    if not trivial_bias:
                        nc.vector.tensor_add(out=X[:, mc, cs0:cs0 + 288],
                                             in0=X[:, mc, cs0:cs0 + 288],
                                             in1=bo[:, mc:mc + 1].broadcast_to([128, 288]))
                xsq = sqp.tile([128, 4, 288], BF16, tag="xsq")
                nc.gpsimd.tensor_mul(xsq, xs, xs)
                ps_st = stats_mms(xs, xsq, 288)
                ln_region(cs0, 288, ps_st, g1, b1)

            # ---------------- FFN chunk emitter (incl. LN2) ----------------
            lastl = (l == NL - 1)

            def ffn_chunk(cc):
                cs = cc * 512
                xc = X[:, :, cs:cs + 512]
                xb2 = xb2p.tile([128, 4, 512], BF16, tag="xb2")
                for kc in range(4):
                    nc.gpsimd.tensor_copy(out=xb2[:, kc, :], in_=xc[:, kc, :])
                hb = hp.tile([128, 16, 512], BF16, tag="hb")
                for fc in range(16):
                    ph = psmm.tile([128, 512], F32, tag="mm")
                    for kc in range(4):
                        nc.tensor.matmul(ph, lhsT=w1[:, kc, fc * 128:(fc + 1) * 128],
                                         rhs=xb2[:, kc, :], start=(kc == 0), stop=(kc == 3))
                    if fc % 2 == 0:
                        nc.scalar.activation(out=hb[:, fc, :], in_=ph, func=AF.Relu,
                                             bias=bf1[:, fc:fc + 1])
                    else:
                        nc.vector.tensor_scalar(
                            out=hb[:, fc, :], in0=ph, scalar1=bf1[:, fc:fc + 1],
                            scalar2=0.0, op0=mybir.AluOpType.add,
                            op1=mybir.AluOpType.max)
                for mc in range(4):
                    pf = psmm.tile([128, 512], F32, tag="mm")
                    for fc in range(16):
                        nc.tensor.matmul(pf, lhsT=w2[:, fc, mc * 128:(mc + 1) * 128],
                                         rhs=hb[:, fc, :], start=(fc == 0), stop=(fc == 15))
                    nc.vector.tensor_add(out=X[:, mc, cs:cs + 512], in0=pf,
                                         in1=X[:, mc, cs:cs + 512])
                    if not trivial_bias:
                        nc.vector.tensor_add(out=X[:, mc, cs:cs + 512],
                                             in0=X[:, mc, cs:cs + 512],
                                             in1=bf2[:, mc:mc + 1].broadcast_to([128, 512]))
                xsq = sqp.tile([128, 4, 512], BF16, tag="xsq2")
                nc.gpsimd.tensor_mul(xsq, xc, xc)
                ps_st = stats_mms(xc, xsq, 512)
                ln_region(cs, 512, ps_st, g2, b2, lastl)

            # ---------------- layer schedule -----------------------------
            if not skip_attn:
                nxt = 0
                cast_pair(0)
                for p in range(PAIRS):
                    if p + 1 < PAIRS:
                        cast_pair(p + 1)
                    att_pair(p)
                    if not skip_ffn:
                        while nxt < NCH and ((nxt + 1) * 512 <= (p + 1) * 288
                                             or p == PAIRS - 1):
                            ffn_chunk(nxt)
                            nxt += 1
            elif not skip_ffn:
                for cc in range(NCH):
                    ffn_chunk(cc)
            if skip_ffn and l == NL - 1:
                for cc in range(NCH):
                    cs = cc * 512
                    nc.sync.dma_start(out=d["out"][:, :, cs:cs + 512],
                                      in_=X[:, :, cs:cs + 512])

    return d


# ---------------------------------------------------------------------------
# Host-side packing + golden model
# ---------------------------------------------------------------------------

def rel_idx():
    coords = np.stack(np.meshgrid(np.arange(WS), np.arange(WS), indexing="ij"))
    flat = coords.reshape(2, -1)
    rel = (flat[:, :, None] - flat[:, None, :]).transpose(1, 2, 0).copy()
    rel[..., 0] += WS - 1
    rel[..., 1] += WS - 1
    rel[..., 0] *= 2 * WS - 1
    return rel.sum(-1)  # [N, N] int


def pack_weights(w, NL):
    """w: dict of reference arrays -> dict of const arrays (np)."""
    bf = ml_dtypes.bfloat16
    scale = HD ** -0.5
    ridx = rel_idx()
    out = {}

    def lhsT_pack(W, kchunks):  # [Cin, Cout] -> [128, kchunks, Cout]
        return np.ascontiguousarray(
            W.reshape(kchunks, 128, W.shape[1]).transpose(1, 0, 2)
        )

    wq = np.stack([lhsT_pack(w["Wq"][l] * scale, 4) for l in range(NL)])
    wk = np.stack([lhsT_pack(w["Wk"][l], 4) for l in range(NL)])
    wv = np.stack([lhsT_pack(w["Wv"][l], 4) for l in range(NL)])
    wo = np.stack([lhsT_pack(w["Wo"][l], 4) for l in range(NL)])
    w1 = np.stack([lhsT_pack(w["W1"][l], 4) for l in range(NL)])
    w2 = np.stack([lhsT_pack(w["W2"][l], 16) for l in range(NL)])
    for nm, arr in (("wq", wq), ("wk", wk), ("wv", wv), ("wo", wo),
                    ("w1", w1), ("w2", w2)):
        out[nm] = arr.astype(bf)

    expb = np.zeros((NL, 128, NH, 288), np.float32)
    for l in range(NL):
        bias = w["rpb"][l][ridx]            # [N(i), N(j), NH]
        ebT = np.exp(bias.transpose(2, 1, 0))  # [NH, j, i]
        expb[l, 0:128, :, 0:144] = ebT[:, 0:128, :].transpose(1, 0, 2)
        expb[l, 0:16, :, 144:288] = ebT[:, 128:144, :].transpose(1, 0, 2)
        expb[l, 32:48, :, 144:288] = ebT[:, 128:144, :].transpose(1, 0, 2)
    out["expb"] = expb.astype(bf)

    def percol(b):  # [NL, C] -> [NL, 128, 4]
        return np.ascontiguousarray(
            b.reshape(NL, 4, 128).transpose(0, 2, 1)).astype(np.float32)

    out["bq"] = percol(w["bq"] * scale)
    out["bk"] = percol(w["bk"])
    out["bo_c"] = percol(w["bo"])
    out["bf2_c"] = percol(w["bf2"])
    out["c_ones1"] = np.ones((1, 128), bf)
    e2 = np.zeros((64, 128), np.float32)
    e2[0, 0:64] = 1.0
    e2[32, 64:128] = 1.0
    out["c_e2"] = e2.astype(bf)
    out["g1"] = percol(w["g1"])
    out["b1"] = percol(w["b1"])
    out["g2"] = percol(w["g2"])
    out["b2"] = percol(w["b2"])
    out["bf1"] = np.ascontiguousarray(
        w["bf1"].reshape(NL, 16, 128).transpose(0, 2, 1)).astype(np.float32)
    out["bvb"] = np.broadcast_to(
        w["bv"].astype(bf)[:, None, :], (NL, 128, 512)).copy()
    out["c_ones"] = np.full((128, 1), 1.0 / 512.0, bf)
    return out


def golden_tm(x_tm, w, NL):
    """fp32 numpy reference on window-major token-major x [T, 512]."""
    T = x_tm.shape[0]
    NW = T // N
    ridx = rel_idx()
    x = x_tm.astype(np.float32)

    def ln(v, g, b):
        m = v.mean(-1, keepdims=True)
        s = v.var(-1, keepdims=True)
        return (v - m) / np.sqrt(s + EPS) * g + b

    for l in range(NL):
        xw = x.reshape(NW, N, C)
        q = (xw @ w["Wq"][l] + w["bq"][l]).reshape(NW, N, NH, HD).transpose(0, 2, 1, 3)
        k = (xw @ w["Wk"][l] + w["bk"][l]).reshape(NW, N, NH, HD).transpose(0, 2, 1, 3)
        v = (xw @ w["Wv"][l] + w["bv"][l]).reshape(NW, N, NH, HD).transpose(0, 2, 1, 3)
        bias = w["rpb"][l][ridx].transpose(2, 0, 1)
        attn = np.einsum("whid,whjd->whij", q, k) * (HD ** -0.5) + bias
        attn = attn - attn.max(-1, keepdims=True)
        p = np.exp(attn)
        p = p / p.sum(-1, keepdims=True)
        o = np.einsum("whij,whjd->whid", p, v).transpose(0, 2, 1, 3).reshape(NW, N, C)
        o = o @ w["Wo"][l] + w["bo"][l]
        x = ln(o.reshape(T, C) + x, w["g1"][l], w["b1"][l])
        h = np.maximum(x @ w["W1"][l] + w["bf1"][l], 0.0) @ w["W2"][l] + w["bf2"][l]
        x = ln(h + x, w["g2"][l], w["b2"][l])
    return x


# ---------------------------------------------------------------------------
# kernel() entry point: full inputs -> full output, 8-way batch data parallel
# ---------------------------------------------------------------------------

NCORES = 8
B_FULL = 64
H_RES = W_RES = 24
L_TOK = H_RES * W_RES
NW_FULL = (B_FULL // NCORES) * (H_RES // WS) * (W_RES // WS)   # 32 windows/core
NL_FULL = 3

_COMPILED = {}


def _pack_x_all(x):
    """[64, 576, 512] f32 -> [8, 128, 4, T] bf16 channel-major window-major."""
    b = x.reshape(NCORES, B_FULL // NCORES, 2, WS, 2, WS, 4, 128)
    v = b.transpose(0, 7, 6, 1, 2, 4, 3, 5)   # [core,128,4, b,hw,ww,hs,ws]
    return np.ascontiguousarray(v.reshape(NCORES, 128, 4, -1)
                                ).astype(ml_dtypes.bfloat16)


def _unpack_out_all(res_list):
    """list of [128, 4, T] -> [64, 576, 512] f32."""
    y = np.stack([r.astype(np.float32) for r in res_list])     # [8,128,4,T]
    bpc = B_FULL // NCORES
    v = y.reshape(NCORES, 128, 4, bpc, 2, 2, WS, WS)
    v = v.transpose(0, 3, 4, 6, 5, 7, 2, 1)   # [core,b,hw,hs,ww,ws,4,128]
    return np.ascontiguousarray(v.reshape(B_FULL, L_TOK, C))


def kernel(x, Wq, bq, Wk, bk, Wv, bv, Wo, bo, rpb,
           g1, b1, W1, bf1, W2, bf2, g2, b2):
    import hashlib
    from concourse.bass_utils import run_bass_kernel_spmd

    w = {"Wq": np.asarray(Wq, np.float32), "bq": np.asarray(bq, np.float32),
         "Wk": np.asarray(Wk, np.float32), "bk": np.asarray(bk, np.float32),
         "Wv": np.asarray(Wv, np.float32), "bv": np.asarray(bv, np.float32),
         "Wo": np.asarray(Wo, np.float32), "bo": np.asarray(bo, np.float32),
         "rpb": np.asarray(rpb, np.float32),
         "g1": np.asarray(g1, np.float32), "b1": np.asarray(b1, np.float32),
         "W1": np.asarray(W1, np.float32), "bf1": np.asarray(bf1, np.float32),
         "W2": np.asarray(W2, np.float32), "bf2": np.asarray(bf2, np.float32),
         "g2": np.asarray(g2, np.float32), "b2": np.asarray(b2, np.float32)}
    hsh = hashlib.blake2b(
        b"".join(np.ascontiguousarray(v).tobytes() for v in w.values()),
        digest_size=16).hexdigest()
    if _COMPILED.get("hash") != hsh:
        packed = pack_weights(w, NL_FULL)
        trivial_gb = bool(np.all(w["g1"] == 1) and np.all(w["b1"] == 0)
                          and np.all(w["g2"] == 1) and np.all(w["b2"] == 0))
        trivial_bias = bool(all(np.all(w[k] == 0)
                                for k in ("bq", "bk", "bv", "bo", "bf1", "bf2")))
        nc = bacc.Bacc("TRN2", target_bir_lowering=False, debug=False)
        build(nc, NW_FULL, NL_FULL, packed,
              trivial_gb=trivial_gb, trivial_bias=trivial_bias)
        nc.compile()
        _COMPILED.update(hash=hsh, nc=nc)

    xp = _pack_x_all(np.asarray(x, np.float32))
    in_maps = [{"x": xp[i]} for i in range(NCORES)]
    res = run_bass_kernel_spmd(_COMPILED["nc"], in_maps, list(range(NCORES)))
    return _unpack_out_all([res.results[i]["out"] for i in range(NCORES)])



# revision 23
# speedup vs baseline: 1.0510x; 1.0015x over previous
"""Swin-style window-attention encoder as a Bass/Tile kernel for TRN2 — v3.

Key design vs v1:
- Residual master X lives in SBUF as FP32 [128, 4, T+16] (channel-major) —
  the residual stream never rounds to bf16 (bf16 master measured 2.2e-2 max
  rel err, over the 2e-2 budget; f32 master 7.8e-3). Matmul inputs are
  bf16 copies cast on the (otherwise idle) GPSIMD engine; per-token mean
  stats contract the f32 master directly (f32 matmul, tiny N).
- Weights are baked into the NEFF as inline consts — per-launch IO is just
  x (bf16 in) and out (bf16).
- Per-layer phase batching: [attention (Exp table)] -> [LN1 rows (Sqrt) +
  apply] -> [FFN (Relu, no table load)] -> [LN2 rows + apply]. 2 activation
  table loads per layer instead of ~64.
- Attention softmax denominators: collected per head into smat rows (act
  Copy), broadcast to 128 partitions via one e2 matmul, ONE fat [128,144]
  DVE reciprocal per head-pair (no 1-lane recips).
- LN row math on [128,36] shuffled layout (SBUF->SBUF strided DMA), not
  1-lane [1,T] ops.
- O-proj / QK-proj / stats at window-pair (288 tokens) granularity; FFN at
  512-token chunks.
"""
from contextlib import ExitStack

import numpy as np
import ml_dtypes

import concourse.bass as bass
import concourse.bacc as bacc
import concourse.tile as tile
import concourse.mybir as mybir

F32 = mybir.dt.float32
F32R = mybir.dt.float32r
BF16 = mybir.dt.bfloat16
AF = mybir.ActivationFunctionType


def _patch_act_tables():
    """Make the act-table-load pass resolve Exp AND Ln to the combined
    `natural_log_exp_and_others` set (it otherwise greedily alternates
    between `exp_and_others` and `natural_log`, reloading tables at every
    attention<->LN boundary, ~150 loads/kernel). We hide exp/ln from every
    other set in the table list the pass consults; set IDs (list order)
    are unchanged, so walrus still emits the right act.json entries."""
    import concourse.hw_specs as hw_specs

    if getattr(bacc, "_ant_act_tables_patched", False):
        return
    orig = hw_specs.get_activation_tables

    def patched(arch):
        tabs = orig(arch)
        exp, ln = AF.Exp, AF.Ln
        if "natural_log_exp_and_others" in tabs:
            for name, fns in tabs.items():
                if name != "natural_log_exp_and_others":
                    fns.discard(exp)
                    fns.discard(ln)
        return tabs

    bacc.get_activation_tables = patched
    bacc._ant_act_tables_patched = True


_patch_act_tables()

WS = 12
N = WS * WS          # 144 tokens per window
C = 512
NH = 8
HD = 64
FF = 2048
EPS = 1e-5


def _insdim_ap(row_ap, stride, num, at=1):
    """Insert a dim of (stride, num) at position `at` of the AP (default:
    right after the partition dim). stride=0 -> broadcast; else gather."""
    dims = [list(d) for d in row_ap.ap]
    return bass.AP(
        tensor=row_ap.tensor,
        offset=row_ap.offset,
        ap=dims[:at] + [[stride, num]] + dims[at:],
    )


def _bcast_ap(row_ap, parts):
    return _insdim_ap(row_ap, 0, parts)


def build(nc: bass.Bass, NW: int, NL: int, w: dict,
          skip_attn=False, skip_ffn=False, skip_heads=False,
          pb=(4, 4), winb=2, epb=3, sqb=1, hbb=1, scb=2, la=3, xbb=2,
          trivial_gb=False, trivial_bias=False):
    """w: packed numpy weight dict (see pack_weights)."""
    T = NW * N
    PAIRS = NW // 2
    NCH = T // 512
    assert T % 512 == 0

    d = {}
    d["x"] = nc.dram_tensor("x", [128, 4, T], BF16, kind="ExternalInput").ap()
    d["out"] = nc.dram_tensor("out", [128, 4, T], BF16, kind="ExternalOutput").ap()
    cst = {nm: nc.inline_tensor(arr, name=nm).ap() for nm, arr in w.items()}

    with tile.TileContext(nc) as tc, ExitStack() as ctx:
        P = lambda name, bufs, **kw: ctx.enter_context(
            tc.tile_pool(name=name, bufs=bufs, **kw)
        )
        xp = P("xmaster", 1)
        cons = P("consts", 1)
        wpA = P("wtsA", 1)     # attention-phase weights
        wpF = P("wtsF", 1)     # ffn-phase weights
        winp = P("win", winb)  # per-pair working tiles
        ep = P("eptiles", epb)  # P tiles
        etp = P("ettiles", 2)   # exp tiles (short-lived)
        sqp = P("sqtiles", sqb)  # squared-x tiles for stats
        scp = P("sctiles", scb)  # recip rows [128,144]
        rowp = P("rows", 2)    # LN stat rows (short-lived, per region)
        hp = P("hbuf", hbb)
        xbp = P("xbcast", 2)
        xb2p = P("xb2cast", 2)
        psmm = P("psmm", pb[0], space="PSUM")
        psaux = P("psaux", 3, space="PSUM")

        # ---- persistent tiles ----
        X = xp.tile([128, 4, T + 16], F32, tag="X")
        for tq in range(NCH):
            xin = sqp.tile([128, 4, 512], BF16, tag="xsq2")
            nc.sync.dma_start(out=xin,
                              in_=d["x"][:, :, tq * 512:(tq + 1) * 512])
            nc.vector.tensor_copy(out=X[:, :, tq * 512:(tq + 1) * 512], in_=xin)
        ones = cons.tile([128, 1], BF16, tag="ones")       # value 1/512
        nc.sync.dma_start(out=ones, in_=cst["c_ones"])
        ones1 = cons.tile([1, 128], BF16, tag="ones1")     # bcast lhsT (1.0)
        nc.sync.dma_start(out=ones1, in_=cst["c_ones1"])
        e2 = cons.tile([64, 128], BF16, tag="e2")
        nc.sync.dma_start(out=e2, in_=cst["c_e2"])
        eps128 = cons.tile([128, 1], F32, tag="eps128")
        nc.vector.memset(eps128, EPS)
        onesf = cons.tile([128, 1], F32, tag="onesf")
        nc.vector.memset(onesf, 1.0 / 512.0)
        smats = [cons.tile([64, 144], BF16, tag=f"smat{i}", name=f"smat{i}")
                 for i in range(8)]
        for t in smats:
            nc.vector.memset(t, 0.0)

        for l in range(NL):
            # layer weights (attention set + rows)
            wq = wpA.tile([128, 4, 512], BF16, tag="wq")
            wk = wpA.tile([128, 4, 512], BF16, tag="wk")
            wv = wpA.tile([128, 4, 512], BF16, tag="wv")
            wo = wpA.tile([128, 4, 512], BF16, tag="wo")
            eb = wpA.tile([128, NH, 288], BF16, tag="expb")
            bq = wpA.tile([128, 4], F32, tag="bq")
            bk = wpA.tile([128, 4], F32, tag="bk")
            bo = wpA.tile([128, 4], F32, tag="bo")
            bv = wpA.tile([128, 512], BF16, tag="bvb")
            g1 = wpA.tile([128, 4], F32, tag="g1")
            b1 = wpA.tile([128, 4], F32, tag="b1")
            g2 = wpA.tile([128, 4], F32, tag="g2")
            b2 = wpA.tile([128, 4], F32, tag="b2")
            for nm, t in (("wq", wq), ("wk", wk), ("wv", wv), ("wo", wo),
                          ("expb", eb), ("bq", bq), ("bk", bk), ("bo_c", bo),
                          ("bvb", bv), ("g1", g1), ("b1", b1), ("g2", g2),
                          ("b2", b2)):
                nc.sync.dma_start(out=t, in_=cst[nm][l])
            # ffn weights: issued now, consumed after LN1 (overlaps attention)
            w1 = wpF.tile([128, 4, FF], BF16, tag="w1")
            w2 = wpF.tile([128, 16, 512], BF16, tag="w2")
            bf1 = wpF.tile([128, 16], F32, tag="bf1")
            bf2 = wpF.tile([128, 4], F32, tag="bf2")
            for nm, t in (("w1", w1), ("w2", w2), ("bf1", bf1), ("bf2_c", bf2)):
                nc.sync.dma_start(out=t, in_=cst[nm][l])

            # ---------------- per-region LN (stats already in ps_st) --------
            def ln_region(cs, wdt, ps_st, g, b, last=False):
                # rows: mean (bf16), mean^2, var, ln(var+eps), rstd=exp(-.5ln)
                srow = rowp.tile([1, 512], BF16, tag="srow", name="srow")[:, :wdt]
                nc.scalar.activation(out=srow, in_=ps_st[0:1, :wdt],
                                     func=AF.Copy)
                m2 = rowp.tile([1, 512], F32, tag="m2row", name="m2")[:, :wdt]
                nc.vector.tensor_mul(m2, srow, srow)
                nc.vector.tensor_sub(m2, ps_st[32:33, :wdt], m2)   # var, in place
                nc.scalar.activation(out=m2, in_=m2, func=AF.Ln,
                                     bias=eps128[0:1, :])          # ln(var+eps)
                rrow = rowp.tile([1, 512], BF16, tag="rrow", name="rrow")[:, :wdt]
                nc.scalar.activation(out=rrow, in_=m2, func=AF.Exp, scale=-0.5)
                # broadcast rows to 128 partitions on the PE; stage to SBUF
                # bf16 immediately so the PSUM bank frees fast (tag "bc"
                # bufs=1 -> bm/br serialize through one bank)
                ps_bm = psaux.tile([128, 512], F32, tag="bc", name="ps_bm",
                                   bufs=1)[:, :wdt]
                nc.tensor.matmul(ps_bm, lhsT=ones1, rhs=srow,
                                 start=True, stop=True)
                bm = rowp.tile([128, 512], BF16, tag="bmsb", name="bm")[:, :wdt]
                nc.vector.tensor_copy(out=bm, in_=ps_bm)
                ps_br = psaux.tile([128, 512], F32, tag="bc", name="ps_br",
                                   bufs=1)[:, :wdt]
                nc.tensor.matmul(ps_br, lhsT=ones1, rhs=rrow,
                                 start=True, stop=True)
                br = rowp.tile([128, 512], BF16, tag="brsb", name="br")[:, :wdt]
                nc.vector.tensor_copy(out=br, in_=ps_br)
                xc = X[:, :, cs:cs + wdt]
                nc.vector.tensor_sub(xc, xc, _insdim_ap(bm, 0, 4))
                nc.vector.tensor_mul(xc, xc, _insdim_ap(br, 0, 4))
                ob = None
                if not trivial_gb:
                    if last:
                        ob = sqp.tile([128, 4, 512], BF16, tag="xsq2",
                                      name="ob")
                    for mc in range(4):
                        dst = ob[:, mc, 0:wdt] if last else X[:, mc, cs:cs + wdt]
                        nc.scalar.activation(out=dst, in_=X[:, mc, cs:cs + wdt],
                                             func=AF.Identity,
                                             bias=b[:, mc:mc + 1],
                                             scale=g[:, mc:mc + 1])
                elif last:
                    ob = sqp.tile([128, 4, 512], BF16, tag="xsq2", name="ob")
                    nc.vector.tensor_copy(out=ob[:, :, 0:wdt], in_=xc)
                if last:
                    nc.sync.dma_start(out=d["out"][:, :, cs:cs + wdt],
                                      in_=ob[:, :, 0:wdt])

            def stats_mms(xs_sl, xsq_sl, wdt):
                # mean at partition 0 (f32 MM), mean-square at partition 32
                # (bf16 MM). Lives in the fast-draining "mm" ring.
                ps_st = psmm.tile([33, 512], F32, tag="mm", name="ps_st")
                for kc in range(4):
                    nc.tensor.matmul(ps_st[0:1, :wdt], lhsT=onesf,
                                     rhs=xs_sl[:, kc, :],
                                     start=(kc == 0), stop=(kc == 3))
                for kc in range(4):
                    nc.tensor.matmul(ps_st[32:33, :wdt], lhsT=ones,
                                     rhs=xsq_sl[:, kc, :],
                                     start=(kc == 0), stop=(kc == 3))
                return ps_st

            # ---------------- per-pair attention emitter --------------------
            cast_cache = {}

            def cast_pair(p):
                cs0 = p * 288
                xb = xbp.tile([128, 4, 304], BF16, tag="xb")
                nc.gpsimd.tensor_copy(out=xb, in_=X[:, :, cs0:cs0 + 304])
                # tail tokens of both windows packed at cols {0:16, 32:48}
                xt = xbp.tile([128, 4, 64], BF16, tag="xt")
                nc.gpsimd.tensor_copy(
                    out=_insdim_ap(xt[:, :, 0:16], 32, 2, at=2),
                    in_=_insdim_ap(X[:, :, cs0 + 128:cs0 + 144], 144, 2, at=2))
                cast_cache[p] = (xb, xt)

            def att_pair(p):
                cs0 = p * 288
                xs = X[:, :, cs0:cs0 + 288]
                xb, xt = cast_cache.pop(p)
                qw = winp.tile([128, 4, 288], BF16, tag="qw")
                kw = winp.tile([128, 4, 288], BF16, tag="kw")
                for mc in range(4):
                    pq = psmm.tile([128, 288], F32, tag="mm")
                    for kc in range(4):
                        nc.tensor.matmul(pq, lhsT=wq[:, kc, mc * 128:(mc + 1) * 128],
                                         rhs=xb[:, kc, 0:288], start=(kc == 0), stop=(kc == 3))
                    if trivial_bias:
                        nc.vector.tensor_copy(out=qw[:, mc, :], in_=pq)
                    else:
                        nc.scalar.activation(out=qw[:, mc, :], in_=pq, func=AF.Identity,
                                             bias=bq[:, mc:mc + 1])
                    pk = psmm.tile([128, 288], F32, tag="mm")
                    for kc in range(4):
                        nc.tensor.matmul(pk, lhsT=wk[:, kc, mc * 128:(mc + 1) * 128],
                                         rhs=xb[:, kc, 0:288], start=(kc == 0), stop=(kc == 3))
                    nc.scalar.activation(out=kw[:, mc, :], in_=pk, func=AF.Identity,
                                         bias=bk[:, mc:mc + 1])

                vws = []
                for wi in (0, 1):
                    vw1 = winp.tile([128, NH, 65], BF16, tag=f"vw1_{wi}")
                    off = wi * 144
                    pv1 = psmm.tile([128, 512], F32, tag="mm")
                    for kc in range(4):
                        nc.tensor.matmul(pv1, lhsT=xb[:, kc, off:off + 128],
                                         rhs=wv[:, kc, :], start=(kc == 0), stop=(kc == 3))
                    nc.vector.tensor_add(out=vw1[:, :, 0:64],
                                         in0=pv1.rearrange("p (h e) -> p h e", h=NH),
                                         in1=bv.rearrange("p (h e) -> p h e", h=NH))
                    nc.vector.memset(vw1[:, :, 64:65], 1.0)
                    vws.append(vw1)
                # merged tail-V for both windows: lhsT cols {128:160, 272:304}
                # -> out partitions A-tail 0:16, (garbage 16:32), B-tail 32:48
                vw2p = winp.tile([64, NH, 65], BF16, tag="vw2p")
                pv2 = psmm.tile([64, 512], F32, tag="mm")
                for kc in range(4):
                    nc.tensor.matmul(pv2, lhsT=xt[:, kc, :], rhs=wv[:, kc, :],
                                     start=(kc == 0), stop=(kc == 3))
                nc.vector.tensor_add(out=vw2p[:, :, 0:64],
                                     in0=pv2.rearrange("p (h e) -> p h e", h=NH),
                                     in1=bv[0:64].rearrange("p (h e) -> p h e", h=NH))
                nc.vector.memset(vw2p[:, :, 64:65], 1.0)

                ocm = winp.tile([128, 4, 288], BF16, tag="ocm")
                if skip_heads:
                    nc.vector.tensor_copy(out=ocm, in_=xs)

                # software-pipelined head loop: stage A (S-mm, exp, P-mul) runs
                # `LOOKAHEAD` heads in front of stage B (PV, den) and stage C
                # (per head-pair: e2 bcast-mm, recip, ocm scale), so the PE has
                # independent matmuls queued while act/DVE chew on earlier heads.
                heads = [(wi, h) for wi in (0, 1) for h in range(NH)]
                pts = {}
                psos = {}

                def stage_a(wi, h):
                    off = wi * 144
                    tb = 32 * wi
                    ro, tl = (h % 2) * 64, h // 2
                    ps_s = psmm.tile([128, 288], F32, tag="mm")
                    nc.tensor.matmul(ps_s[:, 0:144],
                                     lhsT=kw[ro:ro + 64, tl, off:off + 128],
                                     rhs=qw[ro:ro + 64, tl, off:off + 144],
                                     start=True, stop=True)
                    nc.tensor.matmul(ps_s[tb:tb + 16, 144:288],
                                     lhsT=kw[ro:ro + 64, tl, off + 128:off + 144],
                                     rhs=qw[ro:ro + 64, tl, off:off + 144],
                                     start=True, stop=True)
                    et = etp.tile([128, 288], BF16, tag="e")
                    nc.scalar.activation(out=et, in_=ps_s, func=AF.Exp)
                    pt = ep.tile([128, 288], BF16, tag="p")
                    nc.vector.tensor_mul(pt, et, eb[:, h, :])
                    pts[(wi, h)] = pt

                def stage_b(wi, h):
                    pt = pts.pop((wi, h))
                    vw1 = vws[wi]
                    smat = smats[wi * 4 + h // 2]
                    ps_o = psaux.tile([65, 144], F32, tag="aux")
                    nc.tensor.matmul(ps_o, lhsT=vw1[:, h, :], rhs=pt[:, 0:144],
                                     start=True, stop=False)
                    tb = 32 * wi
                    nc.tensor.matmul(ps_o, lhsT=vw2p[tb:tb + 16, h, :],
                                     rhs=pt[tb:tb + 16, 144:288],
                                     start=False, stop=True)
                    nc.scalar.activation(out=smat[32 * (h % 2):32 * (h % 2) + 1, :],
                                         in_=ps_o[64:65, 0:144], func=AF.Copy)
                    psos[(wi, h)] = ps_o
                    if h % 2 == 1:
                        stage_c(wi, h // 2, smat)

                def stage_c(wi, hpair, smat):
                    off = wi * 144
                    ps_sc = psmm.tile([128, 144], F32, tag="mm")
                    nc.tensor.matmul(ps_sc, lhsT=e2, rhs=smat, start=True, stop=True)
                    sc = scp.tile([128, 144], F32, tag="scsb")
                    nc.vector.reciprocal_approx_fast(out=sc, in_=ps_sc)
                    p0 = psos.pop((wi, 2 * hpair))
                    p1 = psos.pop((wi, 2 * hpair + 1))
                    nc.vector.tensor_mul(ocm[0:64, hpair, off:off + 144],
                                         p0[0:64, :], sc[0:64, :])
                    nc.vector.tensor_mul(ocm[64:128, hpair, off:off + 144],
                                         p1[0:64, :], sc[64:128, :])

                LOOKAHEAD = la
                for i, (wi, h) in enumerate(heads if not skip_heads else []):
                    stage_a(wi, h)
                    if i >= LOOKAHEAD:
                        stage_b(*heads[i - LOOKAHEAD])
                for j in (range(max(0, len(heads) - LOOKAHEAD), len(heads))
                          if not skip_heads else []):
                    stage_b(*heads[j])

                # O projection + residual -> X (pre-LN1), stats, LN1
                for mc in range(4):
                    po = psmm.tile([128, 288], F32, tag="mm")
                    for kc in range(4):
                        nc.tensor.matmul(po, lhsT=wo[:, kc, mc * 128:(mc + 1) * 128],
                                         rhs=ocm[:, kc, :], start=(kc == 0), stop=(kc == 3))
                    nc.vector.tensor_add(out=X[:, mc, cs0:cs0 + 288], in0=po,
                                         in1=X[:, mc, cs0:cs0 + 288])
                    if not trivial_bias:
                        nc.vector.tensor_add(out=X[:, mc, cs0:cs0 + 288],
                                             in0=X[:, mc, cs0:cs0 + 288],
                                             in1=bo[:, mc:mc + 1].broadcast_to([128, 288]))
                xsq = sqp.tile([128, 4, 288], BF16, tag="xsq")
                nc.gpsimd.tensor_mul(xsq, xs, xs)
                ps_st = stats_mms(xs, xsq, 288)
                ln_region(cs0, 288, ps_st, g1, b1)

            # ---------------- FFN chunk emitter (incl. LN2) ----------------
            lastl = (l == NL - 1)

            def ffn_chunk(cc):
                cs = cc * 512
                xc = X[:, :, cs:cs + 512]
                xb2 = xb2p.tile([128, 4, 512], BF16, tag="xb2")
                for kc in range(4):
                    nc.gpsimd.tensor_copy(out=xb2[:, kc, :], in_=xc[:, kc, :])
                hb = hp.tile([128, 16, 512], BF16, tag="hb")
                for fc in range(16):
                    ph = psmm.tile([128, 512], F32, tag="mm")
                    for kc in range(4):
                        nc.tensor.matmul(ph, lhsT=w1[:, kc, fc * 128:(fc + 1) * 128],
                                         rhs=xb2[:, kc, :], start=(kc == 0), stop=(kc == 3))
                    if fc % 2 == 0:
                        nc.scalar.activation(out=hb[:, fc, :], in_=ph, func=AF.Relu,
                                             bias=bf1[:, fc:fc + 1])
                    else:
                        nc.vector.tensor_scalar(
                            out=hb[:, fc, :], in0=ph, scalar1=bf1[:, fc:fc + 1],
                            scalar2=0.0, op0=mybir.AluOpType.add,
                            op1=mybir.AluOpType.max)
                for mc in range(4):
                    pf = psmm.tile([128, 512], F32, tag="mm")
                    for fc in range(16):
                        nc.tensor.matmul(pf, lhsT=w2[:, fc, mc * 128:(mc + 1) * 128],
                                         rhs=hb[:, fc, :], start=(fc == 0), stop=(fc == 15))
                    nc.vector.tensor_add(out=X[:, mc, cs:cs + 512], in0=pf,
                                         in1=X[:, mc, cs:cs + 512])
                    if not trivial_bias:
                        nc.vector.tensor_add(out=X[:, mc, cs:cs + 512],
                                             in0=X[:, mc, cs:cs + 512],
                                             in1=bf2[:, mc:mc + 1].broadcast_to([128, 512]))
                xsq = sqp.tile([128, 4, 512], BF16, tag="xsq2")
                nc.gpsimd.tensor_mul(xsq, xc, xc)
                ps_st = stats_mms(xc, xsq, 512)
                ln_region(cs, 512, ps_st, g2, b2, lastl)

            # ---------------- layer schedule -----------------------------
            if not skip_attn:
                # ffn chunk c runs one pair AFTER its LN1 coverage completes,
                # so the LN1 row/bcast/apply chain and the xb2 casts hide
                # under the next pair's attention burst.
                nxt = 0
                cast_pair(0)
                for p in range(PAIRS):
                    if p + 1 < PAIRS:
                        cast_pair(p + 1)
                    if not skip_ffn:
                        while nxt < NCH and (nxt + 1) * 512 <= p * 288:
                            ffn_chunk(nxt)
                            nxt += 1
                    att_pair(p)
                if not skip_ffn:
                    while nxt < NCH:
                        ffn_chunk(nxt)
                        nxt += 1
            elif not skip_ffn:
                for cc in range(NCH):
                    ffn_chunk(cc)
            if skip_ffn and l == NL - 1:
                for cc in range(NCH):
                    cs = cc * 512
                    nc.sync.dma_start(out=d["out"][:, :, cs:cs + 512],
                                      in_=X[:, :, cs:cs + 512])

    return d


# ---------------------------------------------------------------------------
# Host-side packing + golden model
# ---------------------------------------------------------------------------

def rel_idx():
    coords = np.stack(np.meshgrid(np.arange(WS), np.arange(WS), indexing="ij"))
    flat = coords.reshape(2, -1)
    rel = (flat[:, :, None] - flat[:, None, :]).transpose(1, 2, 0).copy()
    rel[..., 0] += WS - 1
    rel[..., 1] += WS - 1
    rel[..., 0] *= 2 * WS - 1
    return rel.sum(-1)  # [N, N] int


def pack_weights(w, NL):
    """w: dict of reference arrays -> dict of const arrays (np)."""
    bf = ml_dtypes.bfloat16
    scale = HD ** -0.5
    ridx = rel_idx()
    out = {}

    def lhsT_pack(W, kchunks):  # [Cin, Cout] -> [128, kchunks, Cout]
        return np.ascontiguousarray(
            W.reshape(kchunks, 128, W.shape[1]).transpose(1, 0, 2)
        )

    wq = np.stack([lhsT_pack(w["Wq"][l] * scale, 4) for l in range(NL)])
    wk = np.stack([lhsT_pack(w["Wk"][l], 4) for l in range(NL)])
    wv = np.stack([lhsT_pack(w["Wv"][l], 4) for l in range(NL)])
    wo = np.stack([lhsT_pack(w["Wo"][l], 4) for l in range(NL)])
    w1 = np.stack([lhsT_pack(w["W1"][l], 4) for l in range(NL)])
    w2 = np.stack([lhsT_pack(w["W2"][l], 16) for l in range(NL)])
    for nm, arr in (("wq", wq), ("wk", wk), ("wv", wv), ("wo", wo),
                    ("w1", w1), ("w2", w2)):
        out[nm] = arr.astype(bf)

    expb = np.zeros((NL, 128, NH, 288), np.float32)
    for l in range(NL):
        bias = w["rpb"][l][ridx]            # [N(i), N(j), NH]
        ebT = np.exp(bias.transpose(2, 1, 0))  # [NH, j, i]
        expb[l, 0:128, :, 0:144] = ebT[:, 0:128, :].transpose(1, 0, 2)
        expb[l, 0:16, :, 144:288] = ebT[:, 128:144, :].transpose(1, 0, 2)
        expb[l, 32:48, :, 144:288] = ebT[:, 128:144, :].transpose(1, 0, 2)
    out["expb"] = expb.astype(bf)

    def percol(b):  # [NL, C] -> [NL, 128, 4]
        return np.ascontiguousarray(
            b.reshape(NL, 4, 128).transpose(0, 2, 1)).astype(np.float32)

    out["bq"] = percol(w["bq"] * scale)
    out["bk"] = percol(w["bk"])
    out["bo_c"] = percol(w["bo"])
    out["bf2_c"] = percol(w["bf2"])
    out["c_ones1"] = np.ones((1, 128), bf)
    e2 = np.zeros((64, 128), np.float32)
    e2[0, 0:64] = 1.0
    e2[32, 64:128] = 1.0
    out["c_e2"] = e2.astype(bf)
    out["g1"] = percol(w["g1"])
    out["b1"] = percol(w["b1"])
    out["g2"] = percol(w["g2"])
    out["b2"] = percol(w["b2"])
    out["bf1"] = np.ascontiguousarray(
        w["bf1"].reshape(NL, 16, 128).transpose(0, 2, 1)).astype(np.float32)
    out["bvb"] = np.broadcast_to(
        w["bv"].astype(bf)[:, None, :], (NL, 128, 512)).copy()
    out["c_ones"] = np.full((128, 1), 1.0 / 512.0, bf)
    return out


def golden_tm(x_tm, w, NL):
    """fp32 numpy reference on window-major token-major x [T, 512]."""
    T = x_tm.shape[0]
    NW = T // N
    ridx = rel_idx()
    x = x_tm.astype(np.float32)

    def ln(v, g, b):
        m = v.mean(-1, keepdims=True)
        s = v.var(-1, keepdims=True)
        return (v - m) / np.sqrt(s + EPS) * g + b

    for l in range(NL):
        xw = x.reshape(NW, N, C)
        q = (xw @ w["Wq"][l] + w["bq"][l]).reshape(NW, N, NH, HD).transpose(0, 2, 1, 3)
        k = (xw @ w["Wk"][l] + w["bk"][l]).reshape(NW, N, NH, HD).transpose(0, 2, 1, 3)
        v = (xw @ w["Wv"][l] + w["bv"][l]).reshape(NW, N, NH, HD).transpose(0, 2, 1, 3)
        bias = w["rpb"][l][ridx].transpose(2, 0, 1)
        attn = np.einsum("whid,whjd->whij", q, k) * (HD ** -0.5) + bias
        attn = attn - attn.max(-1, keepdims=True)
        p = np.exp(attn)
        p = p / p.sum(-1, keepdims=True)
        o = np.einsum("whij,whjd->whid", p, v).transpose(0, 2, 1, 3).reshape(NW, N, C)
        o = o @ w["Wo"][l] + w["bo"][l]
        x = ln(o.reshape(T, C) + x, w["g1"][l], w["b1"][l])
        h = np.maximum(x @ w["W1"][l] + w["bf1"][l], 0.0) @ w["W2"][l] + w["bf2"][l]
        x = ln(h + x, w["g2"][l], w["b2"][l])
    return x


# ---------------------------------------------------------------------------
# kernel() entry point: full inputs -> full output, 8-way batch data parallel
# ---------------------------------------------------------------------------

NCORES = 8
B_FULL = 64
H_RES = W_RES = 24
L_TOK = H_RES * W_RES
NW_FULL = (B_FULL // NCORES) * (H_RES // WS) * (W_RES // WS)   # 32 windows/core
NL_FULL = 3

_COMPILED = {}


def _pack_x_all(x):
    """[64, 576, 512] f32 -> [8, 128, 4, T] bf16 channel-major window-major."""
    b = x.reshape(NCORES, B_FULL // NCORES, 2, WS, 2, WS, 4, 128)
    v = b.transpose(0, 7, 6, 1, 2, 4, 3, 5)   # [core,128,4, b,hw,ww,hs,ws]
    return np.ascontiguousarray(v.reshape(NCORES, 128, 4, -1)
                                ).astype(ml_dtypes.bfloat16)


def _unpack_out_all(res_list):
    """list of [128, 4, T] -> [64, 576, 512] f32."""
    y = np.stack([r.astype(np.float32) for r in res_list])     # [8,128,4,T]
    bpc = B_FULL // NCORES
    v = y.reshape(NCORES, 128, 4, bpc, 2, 2, WS, WS)
    v = v.transpose(0, 3, 4, 6, 5, 7, 2, 1)   # [core,b,hw,hs,ww,ws,4,128]
    return np.ascontiguousarray(v.reshape(B_FULL, L_TOK, C))


def kernel(x, Wq, bq, Wk, bk, Wv, bv, Wo, bo, rpb,
           g1, b1, W1, bf1, W2, bf2, g2, b2):
    import hashlib
    from concourse.bass_utils import run_bass_kernel_spmd

    w = {"Wq": np.asarray(Wq, np.float32), "bq": np.asarray(bq, np.float32),
         "Wk": np.asarray(Wk, np.float32), "bk": np.asarray(bk, np.float32),
         "Wv": np.asarray(Wv, np.float32), "bv": np.asarray(bv, np.float32),
         "Wo": np.asarray(Wo, np.float32), "bo": np.asarray(bo, np.float32),
         "rpb": np.asarray(rpb, np.float32),
         "g1": np.asarray(g1, np.float32), "b1": np.asarray(b1, np.float32),
         "W1": np.asarray(W1, np.float32), "bf1": np.asarray(bf1, np.float32),
         "W2": np.asarray(W2, np.float32), "bf2": np.asarray(bf2, np.float32),
         "g2": np.asarray(g2, np.float32), "b2": np.asarray(b2, np.float32)}
    hsh = hashlib.blake2b(
        b"".join(np.ascontiguousarray(v).tobytes() for v in w.values()),
        digest_size=16).hexdigest()
    if _COMPILED.get("hash") != hsh:
        packed = pack_weights(w, NL_FULL)
        trivial_gb = bool(np.all(w["g1"] == 1) and np.all(w["b1"] == 0)
                          and np.all(w["g2"] == 1) and np.all(w["b2"] == 0))
        trivial_bias = bool(all(np.all(w[k] == 0)
                                for k in ("bq", "bk", "bv", "bo", "bf1", "bf2")))
        nc = bacc.Bacc("TRN2", target_bir_lowering=False, debug=False)
        build(nc, NW_FULL, NL_FULL, packed,
              trivial_gb=trivial_gb, trivial_bias=trivial_bias)
        nc.compile()
        _COMPILED.update(hash=hsh, nc=nc)

    xp = _pack_x_all(np.asarray(x, np.float32))
    in_maps = [{"x": xp[i]} for i in range(NCORES)]
    res = run_bass_kernel_spmd(_COMPILED["nc"], in_maps, list(range(NCORES)))
    return _unpack_out_all([res.results[i]["out"] for i in range(NCORES)])



# revision 25
# speedup vs baseline: 1.0533x; 1.0021x over previous
"""Swin-style window-attention encoder as a Bass/Tile kernel for TRN2 — v3.

Key design vs v1:
- Residual master X lives in SBUF as FP32 [128, 4, T+16] (channel-major) —
  the residual stream never rounds to bf16 (bf16 master measured 2.2e-2 max
  rel err, over the 2e-2 budget; f32 master 7.8e-3). Matmul inputs are
  bf16 copies cast on the (otherwise idle) GPSIMD engine; per-token mean
  stats contract the f32 master directly (f32 matmul, tiny N).
- Weights are baked into the NEFF as inline consts — per-launch IO is just
  x (bf16 in) and out (bf16).
- Per-layer phase batching: [attention (Exp table)] -> [LN1 rows (Sqrt) +
  apply] -> [FFN (Relu, no table load)] -> [LN2 rows + apply]. 2 activation
  table loads per layer instead of ~64.
- Attention softmax denominators: collected per head into smat rows (act
  Copy), broadcast to 128 partitions via one e2 matmul, ONE fat [128,144]
  DVE reciprocal per head-pair (no 1-lane recips).
- LN row math on [128,36] shuffled layout (SBUF->SBUF strided DMA), not
  1-lane [1,T] ops.
- O-proj / QK-proj / stats at window-pair (288 tokens) granularity; FFN at
  512-token chunks.
"""
from contextlib import ExitStack

import numpy as np
import ml_dtypes

import concourse.bass as bass
import concourse.bacc as bacc
import concourse.tile as tile
import concourse.mybir as mybir

F32 = mybir.dt.float32
F32R = mybir.dt.float32r
BF16 = mybir.dt.bfloat16
AF = mybir.ActivationFunctionType


def _patch_act_tables():
    """Make the act-table-load pass resolve Exp AND Ln to the combined
    `natural_log_exp_and_others` set (it otherwise greedily alternates
    between `exp_and_others` and `natural_log`, reloading tables at every
    attention<->LN boundary, ~150 loads/kernel). We hide exp/ln from every
    other set in the table list the pass consults; set IDs (list order)
    are unchanged, so walrus still emits the right act.json entries."""
    import concourse.hw_specs as hw_specs

    if getattr(bacc, "_ant_act_tables_patched", False):
        return
    orig = hw_specs.get_activation_tables

    def patched(arch):
        tabs = orig(arch)
        exp, ln = AF.Exp, AF.Ln
        if "natural_log_exp_and_others" in tabs:
            for name, fns in tabs.items():
                if name != "natural_log_exp_and_others":
                    fns.discard(exp)
                    fns.discard(ln)
        return tabs

    bacc.get_activation_tables = patched
    bacc._ant_act_tables_patched = True


_patch_act_tables()

WS = 12
N = WS * WS          # 144 tokens per window
C = 512
NH = 8
HD = 64
FF = 2048
EPS = 1e-5


def _insdim_ap(row_ap, stride, num, at=1):
    """Insert a dim of (stride, num) at position `at` of the AP (default:
    right after the partition dim). stride=0 -> broadcast; else gather."""
    dims = [list(d) for d in row_ap.ap]
    return bass.AP(
        tensor=row_ap.tensor,
        offset=row_ap.offset,
        ap=dims[:at] + [[stride, num]] + dims[at:],
    )


def _bcast_ap(row_ap, parts):
    return _insdim_ap(row_ap, 0, parts)


def build(nc: bass.Bass, NW: int, NL: int, w: dict,
          skip_attn=False, skip_ffn=False, skip_heads=False,
          pb=(4, 4), winb=2, epb=5, sqb=1, hbb=1, scb=2, la=4, xbb=2,
          trivial_gb=False, trivial_bias=False):
    """w: packed numpy weight dict (see pack_weights)."""
    T = NW * N
    PAIRS = NW // 2
    NCH = T // 512
    assert T % 512 == 0

    d = {}
    d["x"] = nc.dram_tensor("x", [128, 4, T], BF16, kind="ExternalInput").ap()
    d["out"] = nc.dram_tensor("out", [128, 4, T], BF16, kind="ExternalOutput").ap()
    cst = {nm: nc.inline_tensor(arr, name=nm).ap() for nm, arr in w.items()}

    with tile.TileContext(nc) as tc, ExitStack() as ctx:
        P = lambda name, bufs, **kw: ctx.enter_context(
            tc.tile_pool(name=name, bufs=bufs, **kw)
        )
        xp = P("xmaster", 1)
        cons = P("consts", 1)
        wpA = P("wtsA", 1)     # attention-phase weights
        wpF = P("wtsF", 1)     # ffn-phase weights
        winp = P("win", winb)  # per-pair working tiles
        ep = P("eptiles", epb)  # P tiles
        etp = P("ettiles", 2)   # exp tiles (short-lived)
        sqp = P("sqtiles", sqb)  # squared-x tiles for stats
        scp = P("sctiles", scb)  # recip rows [128,144]
        rowp = P("rows", 2)    # LN stat rows (short-lived, per region)
        hp = P("hbuf", hbb)
        xbp = P("xbcast", 2)
        xb2p = P("xb2cast", 2)
        psmm = P("psmm", pb[0], space="PSUM")
        psaux = P("psaux", 3, space="PSUM")

        # ---- persistent tiles ----
        X = xp.tile([128, 4, T + 16], F32, tag="X")
        for tq in range(NCH):
            xin = sqp.tile([128, 4, 512], BF16, tag="xsq2")
            nc.sync.dma_start(out=xin,
                              in_=d["x"][:, :, tq * 512:(tq + 1) * 512])
            nc.vector.tensor_copy(out=X[:, :, tq * 512:(tq + 1) * 512], in_=xin)
        ones = cons.tile([128, 1], BF16, tag="ones")       # value 1/512
        nc.sync.dma_start(out=ones, in_=cst["c_ones"])
        ones1 = cons.tile([1, 128], BF16, tag="ones1")     # bcast lhsT (1.0)
        nc.sync.dma_start(out=ones1, in_=cst["c_ones1"])
        e2 = cons.tile([64, 128], BF16, tag="e2")
        nc.sync.dma_start(out=e2, in_=cst["c_e2"])
        eps128 = cons.tile([128, 1], F32, tag="eps128")
        nc.vector.memset(eps128, EPS)
        onesf = cons.tile([128, 1], F32, tag="onesf")
        nc.vector.memset(onesf, 1.0 / 512.0)

        for l in range(NL):
            # layer weights (attention set + rows)
            wq = wpA.tile([128, 4, 512], BF16, tag="wq")
            wk = wpA.tile([128, 4, 512], BF16, tag="wk")
            wv = wpA.tile([128, 4, 512], BF16, tag="wv")
            wo = wpA.tile([128, 4, 512], BF16, tag="wo")
            eb = wpA.tile([128, NH, 288], BF16, tag="expb")
            bq = wpA.tile([128, 4], F32, tag="bq")
            bk = wpA.tile([128, 4], F32, tag="bk")
            bo = wpA.tile([128, 4], F32, tag="bo")
            bv = wpA.tile([128, 512], BF16, tag="bvb")
            g1 = wpA.tile([128, 4], F32, tag="g1")
            b1 = wpA.tile([128, 4], F32, tag="b1")
            g2 = wpA.tile([128, 4], F32, tag="g2")
            b2 = wpA.tile([128, 4], F32, tag="b2")
            for nm, t in (("wq", wq), ("wk", wk), ("wv", wv), ("wo", wo),
                          ("expb", eb), ("bq", bq), ("bk", bk), ("bo_c", bo),
                          ("bvb", bv), ("g1", g1), ("b1", b1), ("g2", g2),
                          ("b2", b2)):
                nc.sync.dma_start(out=t, in_=cst[nm][l])
            # ffn weights: issued now, consumed after LN1 (overlaps attention)
            w1 = wpF.tile([128, 4, FF], BF16, tag="w1")
            w2 = wpF.tile([128, 16, 512], BF16, tag="w2")
            bf1 = wpF.tile([128, 16], F32, tag="bf1")
            bf2 = wpF.tile([128, 4], F32, tag="bf2")
            for nm, t in (("w1", w1), ("w2", w2), ("bf1", bf1), ("bf2_c", bf2)):
                nc.sync.dma_start(out=t, in_=cst[nm][l])

            # ---------------- per-region LN (stats already in ps_st) --------
            def ln_region(cs, wdt, ps_st, g, b, last=False):
                # rows: mean (bf16), mean^2, var, ln(var+eps), rstd=exp(-.5ln)
                srow = rowp.tile([1, 512], BF16, tag="srow", name="srow")[:, :wdt]
                nc.scalar.activation(out=srow, in_=ps_st[0:1, :wdt],
                                     func=AF.Copy)
                m2 = rowp.tile([1, 512], F32, tag="m2row", name="m2")[:, :wdt]
                nc.vector.tensor_mul(m2, srow, srow)
                nc.vector.tensor_sub(m2, ps_st[32:33, :wdt], m2)   # var, in place
                nc.scalar.activation(out=m2, in_=m2, func=AF.Ln,
                                     bias=eps128[0:1, :])          # ln(var+eps)
                rrow = rowp.tile([1, 512], BF16, tag="rrow", name="rrow")[:, :wdt]
                nc.scalar.activation(out=rrow, in_=m2, func=AF.Exp, scale=-0.5)
                # broadcast rows to 128 partitions on the PE; stage to SBUF
                # bf16 immediately so the PSUM bank frees fast (tag "bc"
                # bufs=1 -> bm/br serialize through one bank)
                ps_bm = psaux.tile([128, 512], F32, tag="bc", name="ps_bm",
                                   bufs=1)[:, :wdt]
                nc.tensor.matmul(ps_bm, lhsT=ones1, rhs=srow,
                                 start=True, stop=True)
                bm = rowp.tile([128, 512], BF16, tag="bmsb", name="bm")[:, :wdt]
                nc.vector.tensor_copy(out=bm, in_=ps_bm)
                ps_br = psaux.tile([128, 512], F32, tag="bc", name="ps_br",
                                   bufs=1)[:, :wdt]
                nc.tensor.matmul(ps_br, lhsT=ones1, rhs=rrow,
                                 start=True, stop=True)
                br = rowp.tile([128, 512], BF16, tag="brsb", name="br")[:, :wdt]
                nc.vector.tensor_copy(out=br, in_=ps_br)
                xc = X[:, :, cs:cs + wdt]
                nc.vector.tensor_sub(xc, xc, _insdim_ap(bm, 0, 4))
                nc.vector.tensor_mul(xc, xc, _insdim_ap(br, 0, 4))
                ob = None
                if not trivial_gb:
                    if last:
                        ob = sqp.tile([128, 4, 512], BF16, tag="xsq2",
                                      name="ob")
                    for mc in range(4):
                        dst = ob[:, mc, 0:wdt] if last else X[:, mc, cs:cs + wdt]
                        nc.scalar.activation(out=dst, in_=X[:, mc, cs:cs + wdt],
                                             func=AF.Identity,
                                             bias=b[:, mc:mc + 1],
                                             scale=g[:, mc:mc + 1])
                elif last:
                    ob = sqp.tile([128, 4, 512], BF16, tag="xsq2", name="ob")
                    nc.vector.tensor_copy(out=ob[:, :, 0:wdt], in_=xc)
                if last:
                    nc.sync.dma_start(out=d["out"][:, :, cs:cs + wdt],
                                      in_=ob[:, :, 0:wdt])

            def stats_mms(xs_sl, xsq_sl, wdt):
                # mean at partition 0 (f32 MM), mean-square at partition 32
                # (bf16 MM). Lives in the fast-draining "mm" ring.
                ps_st = psmm.tile([33, 512], F32, tag="mm", name="ps_st")
                for kc in range(4):
                    nc.tensor.matmul(ps_st[0:1, :wdt], lhsT=onesf,
                                     rhs=xs_sl[:, kc, :],
                                     start=(kc == 0), stop=(kc == 3))
                for kc in range(4):
                    nc.tensor.matmul(ps_st[32:33, :wdt], lhsT=ones,
                                     rhs=xsq_sl[:, kc, :],
                                     start=(kc == 0), stop=(kc == 3))
                return ps_st

            # ---------------- per-pair attention emitter --------------------
            cast_cache = {}

            def cast_pair(p):
                cs0 = p * 288
                xb = xbp.tile([128, 4, 304], BF16, tag="xb")
                nc.gpsimd.tensor_copy(out=xb, in_=X[:, :, cs0:cs0 + 304])
                # tail tokens of both windows packed at cols {0:16, 32:48}
                xt = xbp.tile([128, 4, 64], BF16, tag="xt")
                nc.gpsimd.tensor_copy(
                    out=_insdim_ap(xt[:, :, 0:16], 32, 2, at=2),
                    in_=_insdim_ap(X[:, :, cs0 + 128:cs0 + 144], 144, 2, at=2))
                cast_cache[p] = (xb, xt)

            def att_pair(p):
                cs0 = p * 288
                xs = X[:, :, cs0:cs0 + 288]
                xb, xt = cast_cache.pop(p)
                qw = winp.tile([128, 4, 288], BF16, tag="qw")
                kw = winp.tile([128, 4, 288], BF16, tag="kw")
                for mc in range(4):
                    pq = psmm.tile([128, 288], F32, tag="mm")
                    for kc in range(4):
                        nc.tensor.matmul(pq, lhsT=wq[:, kc, mc * 128:(mc + 1) * 128],
                                         rhs=xb[:, kc, 0:288], start=(kc == 0), stop=(kc == 3))
                    if trivial_bias:
                        nc.vector.tensor_copy(out=qw[:, mc, :], in_=pq)
                    else:
                        nc.scalar.activation(out=qw[:, mc, :], in_=pq, func=AF.Identity,
                                             bias=bq[:, mc:mc + 1])
                    pk = psmm.tile([128, 288], F32, tag="mm")
                    for kc in range(4):
                        nc.tensor.matmul(pk, lhsT=wk[:, kc, mc * 128:(mc + 1) * 128],
                                         rhs=xb[:, kc, 0:288], start=(kc == 0), stop=(kc == 3))
                    nc.scalar.activation(out=kw[:, mc, :], in_=pk, func=AF.Identity,
                                         bias=bk[:, mc:mc + 1])

                vws = []
                for wi in (0, 1):
                    vw1 = winp.tile([128, NH, 65], BF16, tag=f"vw1_{wi}")
                    off = wi * 144
                    pv1 = psmm.tile([128, 512], F32, tag="mm")
                    for kc in range(4):
                        nc.tensor.matmul(pv1, lhsT=xb[:, kc, off:off + 128],
                                         rhs=wv[:, kc, :], start=(kc == 0), stop=(kc == 3))
                    nc.vector.tensor_add(out=vw1[:, :, 0:64],
                                         in0=pv1.rearrange("p (h e) -> p h e", h=NH),
                                         in1=bv.rearrange("p (h e) -> p h e", h=NH))
                    nc.vector.memset(vw1[:, :, 64:65], 1.0)
                    vws.append(vw1)
                # merged tail-V for both windows: lhsT cols {128:160, 272:304}
                # -> out partitions A-tail 0:16, (garbage 16:32), B-tail 32:48
                vw2p = winp.tile([64, NH, 65], BF16, tag="vw2p")
                pv2 = psmm.tile([64, 512], F32, tag="mm")
                for kc in range(4):
                    nc.tensor.matmul(pv2, lhsT=xt[:, kc, :], rhs=wv[:, kc, :],
                                     start=(kc == 0), stop=(kc == 3))
                nc.vector.tensor_add(out=vw2p[:, :, 0:64],
                                     in0=pv2.rearrange("p (h e) -> p h e", h=NH),
                                     in1=bv[0:64].rearrange("p (h e) -> p h e", h=NH))
                nc.vector.memset(vw2p[:, :, 64:65], 1.0)

                ocm = winp.tile([128, 4, 288], BF16, tag="ocm")
                if skip_heads:
                    nc.vector.tensor_copy(out=ocm, in_=xs)

                # software-pipelined head loop: stage A (S-mm, exp, P-mul) runs
                # `LOOKAHEAD` heads in front of stage B (PV, den) and stage C
                # (per head-pair: e2 bcast-mm, recip, ocm scale), so the PE has
                # independent matmuls queued while act/DVE chew on earlier heads.
                heads = [(wi, h) for wi in (0, 1) for h in range(NH)]
                pts = {}
                psos = {}

                def stage_a(wi, h):
                    off = wi * 144
                    tb = 32 * wi
                    ro, tl = (h % 2) * 64, h // 2
                    ps_s = psmm.tile([128, 288], F32, tag="mm")
                    nc.tensor.matmul(ps_s[:, 0:144],
                                     lhsT=kw[ro:ro + 64, tl, off:off + 128],
                                     rhs=qw[ro:ro + 64, tl, off:off + 144],
                                     start=True, stop=True)
                    nc.tensor.matmul(ps_s[tb:tb + 16, 144:288],
                                     lhsT=kw[ro:ro + 64, tl, off + 128:off + 144],
                                     rhs=qw[ro:ro + 64, tl, off:off + 144],
                                     start=True, stop=True)
                    et = etp.tile([128, 288], BF16, tag="e")
                    nc.scalar.activation(out=et, in_=ps_s, func=AF.Exp)
                    pt = ep.tile([128, 288], BF16, tag="p")
                    nc.vector.tensor_mul(pt, et, eb[:, h, :])
                    pts[(wi, h)] = pt

                def stage_b(wi, h):
                    pt = pts.pop((wi, h))
                    vw1 = vws[wi]
                    hp, hi = h // 2, h % 2
                    if hi == 0:
                        psos[(wi, hp)] = psaux.tile([65, 2, 144], F32,
                                                    tag="aux", name="ps_o2")
                    ps_o = psos[(wi, hp)]
                    nc.tensor.matmul(ps_o[:, hi, :], lhsT=vw1[:, h, :],
                                     rhs=pt[:, 0:144], start=True, stop=False)
                    tb = 32 * wi
                    nc.tensor.matmul(ps_o[:, hi, :],
                                     lhsT=vw2p[tb:tb + 16, h, :],
                                     rhs=pt[tb:tb + 16, 144:288],
                                     start=False, stop=True)
                    if hi == 1:
                        stage_c(wi, hp)

                def stage_c(wi, hpair):
                    off = wi * 144
                    ps_o = psos[(wi, hpair)]
                    # both heads' softmax denominators in one row copy
                    dp = rowp.tile([1, 288], BF16, tag="dpair", name="dp",
                                   bufs=4)
                    nc.scalar.activation(out=dp, in_=ps_o[64:65, 0:2, 0:144],
                                         func=AF.Copy)
                    ps_sc = psmm.tile([128, 144], F32, tag="mm")
                    nc.tensor.matmul(ps_sc[0:64, :], lhsT=ones1[0:1, 0:64],
                                     rhs=dp[0:1, 0:144], start=True, stop=True)
                    nc.tensor.matmul(ps_sc[64:128, :], lhsT=ones1[0:1, 0:64],
                                     rhs=dp[0:1, 144:288], start=True, stop=True)
                    sc = scp.tile([128, 144], F32, tag="scsb")
                    nc.vector.reciprocal_approx_fast(out=sc, in_=ps_sc)
                    p01 = psos.pop((wi, hpair))
                    nc.vector.tensor_mul(ocm[0:64, hpair, off:off + 144],
                                         p01[0:64, 0, :], sc[0:64, :])
                    nc.vector.tensor_mul(ocm[64:128, hpair, off:off + 144],
                                         p01[0:64, 1, :], sc[64:128, :])

                LOOKAHEAD = la
                for i, (wi, h) in enumerate(heads if not skip_heads else []):
                    stage_a(wi, h)
                    if i >= LOOKAHEAD:
                        stage_b(*heads[i - LOOKAHEAD])
                for j in (range(max(0, len(heads) - LOOKAHEAD), len(heads))
                          if not skip_heads else []):
                    stage_b(*heads[j])

                # O projection + residual -> X (pre-LN1), stats, LN1
                for mc in range(4):
                    po = psmm.tile([128, 288], F32, tag="mm")
                    for kc in range(4):
                        nc.tensor.matmul(po, lhsT=wo[:, kc, mc * 128:(mc + 1) * 128],
                                         rhs=ocm[:, kc, :], start=(kc == 0), stop=(kc == 3))
                    nc.vector.tensor_add(out=X[:, mc, cs0:cs0 + 288], in0=po,
                                         in1=X[:, mc, cs0:cs0 + 288])
                    if not trivial_bias:
                        nc.vector.tensor_add(out=X[:, mc, cs0:cs0 + 288],
                                             in0=X[:, mc, cs0:cs0 + 288],
                                             in1=bo[:, mc:mc + 1].broadcast_to([128, 288]))
                xsq = sqp.tile([128, 4, 288], BF16, tag="xsq")
                nc.gpsimd.tensor_mul(xsq, xs, xs)
                ps_st = stats_mms(xs, xsq, 288)
                ln_region(cs0, 288, ps_st, g1, b1)

            # ---------------- FFN chunk emitter (incl. LN2) ----------------
            lastl = (l == NL - 1)

            def ffn_chunk(cc):
                cs = cc * 512
                xc = X[:, :, cs:cs + 512]
                xb2 = xb2p.tile([128, 4, 512], BF16, tag="xb2")
                for kc in range(4):
                    nc.gpsimd.tensor_copy(out=xb2[:, kc, :], in_=xc[:, kc, :])
                hb = hp.tile([128, 16, 512], BF16, tag="hb")
                for fc in range(16):
                    ph = psmm.tile([128, 512], F32, tag="mm")
                    for kc in range(4):
                        nc.tensor.matmul(ph, lhsT=w1[:, kc, fc * 128:(fc + 1) * 128],
                                         rhs=xb2[:, kc, :], start=(kc == 0), stop=(kc == 3))
                    if fc % 2 == 0:
                        nc.scalar.activation(out=hb[:, fc, :], in_=ph, func=AF.Relu,
                                             bias=bf1[:, fc:fc + 1])
                    else:
                        nc.vector.tensor_scalar(
                            out=hb[:, fc, :], in0=ph, scalar1=bf1[:, fc:fc + 1],
                            scalar2=0.0, op0=mybir.AluOpType.add,
                            op1=mybir.AluOpType.max)
                for mc in range(4):
                    pf = psmm.tile([128, 512], F32, tag="mm")
                    for fc in range(16):
                        nc.tensor.matmul(pf, lhsT=w2[:, fc, mc * 128:(mc + 1) * 128],
                                         rhs=hb[:, fc, :], start=(fc == 0), stop=(fc == 15))
                    nc.vector.tensor_add(out=X[:, mc, cs:cs + 512], in0=pf,
                                         in1=X[:, mc, cs:cs + 512])
                    if not trivial_bias:
                        nc.vector.tensor_add(out=X[:, mc, cs:cs + 512],
                                             in0=X[:, mc, cs:cs + 512],
                                             in1=bf2[:, mc:mc + 1].broadcast_to([128, 512]))
                xsq = sqp.tile([128, 4, 512], BF16, tag="xsq2")
                nc.gpsimd.tensor_mul(xsq, xc, xc)
                ps_st = stats_mms(xc, xsq, 512)
                ln_region(cs, 512, ps_st, g2, b2, lastl)

            # ---------------- layer schedule -----------------------------
            if not skip_attn:
                # ffn chunk c runs one pair AFTER its LN1 coverage completes,
                # so the LN1 row/bcast/apply chain and the xb2 casts hide
                # under the next pair's attention burst.
                nxt = 0
                cast_pair(0)
                for p in range(PAIRS):
                    if p + 1 < PAIRS:
                        cast_pair(p + 1)
                    if not skip_ffn:
                        while nxt < NCH and (nxt + 1) * 512 <= p * 288:
                            ffn_chunk(nxt)
                            nxt += 1
                    att_pair(p)
                if not skip_ffn:
                    while nxt < NCH:
                        ffn_chunk(nxt)
                        nxt += 1
            elif not skip_ffn:
                for cc in range(NCH):
                    ffn_chunk(cc)
            if skip_ffn and l == NL - 1:
                for cc in range(NCH):
                    cs = cc * 512
                    nc.sync.dma_start(out=d["out"][:, :, cs:cs + 512],
                                      in_=X[:, :, cs:cs + 512])

    return d


# ---------------------------------------------------------------------------
# Host-side packing + golden model
# ---------------------------------------------------------------------------

def rel_idx():
    coords = np.stack(np.meshgrid(np.arange(WS), np.arange(WS), indexing="ij"))
    flat = coords.reshape(2, -1)
    rel = (flat[:, :, None] - flat[:, None, :]).transpose(1, 2, 0).copy()
    rel[..., 0] += WS - 1
    rel[..., 1] += WS - 1
    rel[..., 0] *= 2 * WS - 1
    return rel.sum(-1)  # [N, N] int


def pack_weights(w, NL):
    """w: dict of reference arrays -> dict of const arrays (np)."""
    bf = ml_dtypes.bfloat16
    scale = HD ** -0.5
    ridx = rel_idx()
    out = {}

    def lhsT_pack(W, kchunks):  # [Cin, Cout] -> [128, kchunks, Cout]
        return np.ascontiguousarray(
            W.reshape(kchunks, 128, W.shape[1]).transpose(1, 0, 2)
        )

    wq = np.stack([lhsT_pack(w["Wq"][l] * scale, 4) for l in range(NL)])
    wk = np.stack([lhsT_pack(w["Wk"][l], 4) for l in range(NL)])
    wv = np.stack([lhsT_pack(w["Wv"][l], 4) for l in range(NL)])
    wo = np.stack([lhsT_pack(w["Wo"][l], 4) for l in range(NL)])
    w1 = np.stack([lhsT_pack(w["W1"][l], 4) for l in range(NL)])
    w2 = np.stack([lhsT_pack(w["W2"][l], 16) for l in range(NL)])
    for nm, arr in (("wq", wq), ("wk", wk), ("wv", wv), ("wo", wo),
                    ("w1", w1), ("w2", w2)):
        out[nm] = arr.astype(bf)

    expb = np.zeros((NL, 128, NH, 288), np.float32)
    for l in range(NL):
        bias = w["rpb"][l][ridx]            # [N(i), N(j), NH]
        ebT = np.exp(bias.transpose(2, 1, 0))  # [NH, j, i]
        expb[l, 0:128, :, 0:144] = ebT[:, 0:128, :].transpose(1, 0, 2)
        expb[l, 0:16, :, 144:288] = ebT[:, 128:144, :].transpose(1, 0, 2)
        expb[l, 32:48, :, 144:288] = ebT[:, 128:144, :].transpose(1, 0, 2)
    out["expb"] = expb.astype(bf)

    def percol(b):  # [NL, C] -> [NL, 128, 4]
        return np.ascontiguousarray(
            b.reshape(NL, 4, 128).transpose(0, 2, 1)).astype(np.float32)

    out["bq"] = percol(w["bq"] * scale)
    out["bk"] = percol(w["bk"])
    out["bo_c"] = percol(w["bo"])
    out["bf2_c"] = percol(w["bf2"])
    out["c_ones1"] = np.ones((1, 128), bf)
    e2 = np.zeros((64, 128), np.float32)
    e2[0, 0:64] = 1.0
    e2[32, 64:128] = 1.0
    out["c_e2"] = e2.astype(bf)
    out["g1"] = percol(w["g1"])
    out["b1"] = percol(w["b1"])
    out["g2"] = percol(w["g2"])
    out["b2"] = percol(w["b2"])
    out["bf1"] = np.ascontiguousarray(
        w["bf1"].reshape(NL, 16, 128).transpose(0, 2, 1)).astype(np.float32)
    out["bvb"] = np.broadcast_to(
        w["bv"].astype(bf)[:, None, :], (NL, 128, 512)).copy()
    out["c_ones"] = np.full((128, 1), 1.0 / 512.0, bf)
    return out


def golden_tm(x_tm, w, NL):
    """fp32 numpy reference on window-major token-major x [T, 512]."""
    T = x_tm.shape[0]
    NW = T // N
    ridx = rel_idx()
    x = x_tm.astype(np.float32)

    def ln(v, g, b):
        m = v.mean(-1, keepdims=True)
        s = v.var(-1, keepdims=True)
        return (v - m) / np.sqrt(s + EPS) * g + b

    for l in range(NL):
        xw = x.reshape(NW, N, C)
        q = (xw @ w["Wq"][l] + w["bq"][l]).reshape(NW, N, NH, HD).transpose(0, 2, 1, 3)
        k = (xw @ w["Wk"][l] + w["bk"][l]).reshape(NW, N, NH, HD).transpose(0, 2, 1, 3)
        v = (xw @ w["Wv"][l] + w["bv"][l]).reshape(NW, N, NH, HD).transpose(0, 2, 1, 3)
        bias = w["rpb"][l][ridx].transpose(2, 0, 1)
        attn = np.einsum("whid,whjd->whij", q, k) * (HD ** -0.5) + bias
        attn = attn - attn.max(-1, keepdims=True)
        p = np.exp(attn)
        p = p / p.sum(-1, keepdims=True)
        o = np.einsum("whij,whjd->whid", p, v).transpose(0, 2, 1, 3).reshape(NW, N, C)
        o = o @ w["Wo"][l] + w["bo"][l]
        x = ln(o.reshape(T, C) + x, w["g1"][l], w["b1"][l])
        h = np.maximum(x @ w["W1"][l] + w["bf1"][l], 0.0) @ w["W2"][l] + w["bf2"][l]
        x = ln(h + x, w["g2"][l], w["b2"][l])
    return x


# ---------------------------------------------------------------------------
# kernel() entry point: full inputs -> full output, 8-way batch data parallel
# ---------------------------------------------------------------------------

NCORES = 8
B_FULL = 64
H_RES = W_RES = 24
L_TOK = H_RES * W_RES
NW_FULL = (B_FULL // NCORES) * (H_RES // WS) * (W_RES // WS)   # 32 windows/core
NL_FULL = 3

_COMPILED = {}


def _pack_x_all(x):
    """[64, 576, 512] f32 -> [8, 128, 4, T] bf16 channel-major window-major."""
    b = x.reshape(NCORES, B_FULL // NCORES, 2, WS, 2, WS, 4, 128)
    v = b.transpose(0, 7, 6, 1, 2, 4, 3, 5)   # [core,128,4, b,hw,ww,hs,ws]
    return np.ascontiguousarray(v.reshape(NCORES, 128, 4, -1)
                                ).astype(ml_dtypes.bfloat16)


def _unpack_out_all(res_list):
    """list of [128, 4, T] -> [64, 576, 512] f32."""
    y = np.stack([r.astype(np.float32) for r in res_list])     # [8,128,4,T]
    bpc = B_FULL // NCORES
    v = y.reshape(NCORES, 128, 4, bpc, 2, 2, WS, WS)
    v = v.transpose(0, 3, 4, 6, 5, 7, 2, 1)   # [core,b,hw,hs,ww,ws,4,128]
    return np.ascontiguousarray(v.reshape(B_FULL, L_TOK, C))


def kernel(x, Wq, bq, Wk, bk, Wv, bv, Wo, bo, rpb,
           g1, b1, W1, bf1, W2, bf2, g2, b2):
    import hashlib
    from concourse.bass_utils import run_bass_kernel_spmd

    w = {"Wq": np.asarray(Wq, np.float32), "bq": np.asarray(bq, np.float32),
         "Wk": np.asarray(Wk, np.float32), "bk": np.asarray(bk, np.float32),
         "Wv": np.asarray(Wv, np.float32), "bv": np.asarray(bv, np.float32),
         "Wo": np.asarray(Wo, np.float32), "bo": np.asarray(bo, np.float32),
         "rpb": np.asarray(rpb, np.float32),
         "g1": np.asarray(g1, np.float32), "b1": np.asarray(b1, np.float32),
         "W1": np.asarray(W1, np.float32), "bf1": np.asarray(bf1, np.float32),
         "W2": np.asarray(W2, np.float32), "bf2": np.asarray(bf2, np.float32),
         "g2": np.asarray(g2, np.float32), "b2": np.asarray(b2, np.float32)}
    hsh = hashlib.blake2b(
        b"".join(np.ascontiguousarray(v).tobytes() for v in w.values()),
        digest_size=16).hexdigest()
    if _COMPILED.get("hash") != hsh:
        packed = pack_weights(w, NL_FULL)
        trivial_gb = bool(np.all(w["g1"] == 1) and np.all(w["b1"] == 0)
                          and np.all(w["g2"] == 1) and np.all(w["b2"] == 0))
        trivial_bias = bool(all(np.all(w[k] == 0)
                                for k in ("bq", "bk", "bv", "bo", "bf1", "bf2")))
        nc = bacc.Bacc("TRN2", target_bir_lowering=False, debug=False)
        build(nc, NW_FULL, NL_FULL, packed,
              trivial_gb=trivial_gb, trivial_bias=trivial_bias)
        nc.compile()
        _COMPILED.update(hash=hsh, nc=nc)

    xp = _pack_x_all(np.asarray(x, np.float32))
    in_maps = [{"x": xp[i]} for i in range(NCORES)]
    res = run_bass_kernel_spmd(_COMPILED["nc"], in_maps, list(range(NCORES)))
    return _unpack_out_all([res.results[i]["out"] for i in range(NCORES)])



# revision 26
# speedup vs baseline: 1.0707x; 1.0165x over previous
"""Swin-style window-attention encoder as a Bass/Tile kernel for TRN2 — v3.

Key design vs v1:
- Residual master X lives in SBUF as FP32 [128, 4, T+16] (channel-major) —
  the residual stream never rounds to bf16 (bf16 master measured 2.2e-2 max
  rel err, over the 2e-2 budget; f32 master 7.8e-3). Matmul inputs are
  bf16 copies cast on the (otherwise idle) GPSIMD engine; per-token mean
  stats contract the f32 master directly (f32 matmul, tiny N).
- Weights are baked into the NEFF as inline consts — per-launch IO is just
  x (bf16 in) and out (bf16).
- Per-layer phase batching: [attention (Exp table)] -> [LN1 rows (Sqrt) +
  apply] -> [FFN (Relu, no table load)] -> [LN2 rows + apply]. 2 activation
  table loads per layer instead of ~64.
- Attention softmax denominators: collected per head into smat rows (act
  Copy), broadcast to 128 partitions via one e2 matmul, ONE fat [128,144]
  DVE reciprocal per head-pair (no 1-lane recips).
- LN row math on [128,36] shuffled layout (SBUF->SBUF strided DMA), not
  1-lane [1,T] ops.
- O-proj / QK-proj / stats at window-pair (288 tokens) granularity; FFN at
  512-token chunks.
"""
from contextlib import ExitStack

import numpy as np
import ml_dtypes

import concourse.bass as bass
import concourse.bacc as bacc
import concourse.tile as tile
import concourse.mybir as mybir

F32 = mybir.dt.float32
F32R = mybir.dt.float32r
BF16 = mybir.dt.bfloat16
AF = mybir.ActivationFunctionType


def _patch_act_tables():
    """Make the act-table-load pass resolve Exp AND Ln to the combined
    `natural_log_exp_and_others` set (it otherwise greedily alternates
    between `exp_and_others` and `natural_log`, reloading tables at every
    attention<->LN boundary, ~150 loads/kernel). We hide exp/ln from every
    other set in the table list the pass consults; set IDs (list order)
    are unchanged, so walrus still emits the right act.json entries."""
    import concourse.hw_specs as hw_specs

    if getattr(bacc, "_ant_act_tables_patched", False):
        return
    orig = hw_specs.get_activation_tables

    def patched(arch):
        tabs = orig(arch)
        exp, ln = AF.Exp, AF.Ln
        if "natural_log_exp_and_others" in tabs:
            for name, fns in tabs.items():
                if name != "natural_log_exp_and_others":
                    fns.discard(exp)
                    fns.discard(ln)
        return tabs

    bacc.get_activation_tables = patched
    bacc._ant_act_tables_patched = True


_patch_act_tables()

WS = 12
N = WS * WS          # 144 tokens per window
C = 512
NH = 8
HD = 64
FF = 2048
EPS = 1e-5


def _insdim_ap(row_ap, stride, num, at=1):
    """Insert a dim of (stride, num) at position `at` of the AP (default:
    right after the partition dim). stride=0 -> broadcast; else gather."""
    dims = [list(d) for d in row_ap.ap]
    return bass.AP(
        tensor=row_ap.tensor,
        offset=row_ap.offset,
        ap=dims[:at] + [[stride, num]] + dims[at:],
    )


def _bcast_ap(row_ap, parts):
    return _insdim_ap(row_ap, 0, parts)


def build(nc: bass.Bass, NW: int, NL: int, w: dict,
          skip_attn=False, skip_ffn=False, skip_heads=False,
          pb=(4, 4), winb=2, epb=5, sqb=1, hbb=1, scb=2, la=4, xbb=2,
          trivial_gb=False, trivial_bias=False):
    """w: packed numpy weight dict (see pack_weights)."""
    T = NW * N
    PAIRS = NW // 2
    NCH = T // 512
    assert T % 512 == 0

    d = {}
    d["x"] = nc.dram_tensor("x", [128, 4, T], BF16, kind="ExternalInput").ap()
    d["out"] = nc.dram_tensor("out", [128, 4, T], BF16, kind="ExternalOutput").ap()
    cst = {nm: nc.inline_tensor(arr, name=nm).ap() for nm, arr in w.items()}

    with tile.TileContext(nc) as tc, ExitStack() as ctx:
        P = lambda name, bufs, **kw: ctx.enter_context(
            tc.tile_pool(name=name, bufs=bufs, **kw)
        )
        xp = P("xmaster", 1)
        cons = P("consts", 1)
        wpA = P("wtsA", 1)     # attention-phase weights
        wpF = P("wtsF", 1)     # ffn-phase weights
        winp = P("win", winb)  # per-pair working tiles
        ep = P("eptiles", epb)  # P tiles
        etp = P("ettiles", 2)   # exp tiles (short-lived)
        sqp = P("sqtiles", sqb)  # squared-x tiles for stats
        scp = P("sctiles", scb)  # recip rows [128,144]
        rowp = P("rows", 2)    # LN stat rows (short-lived, per region)
        hp = P("hbuf", hbb)
        xbp = P("xbcast", 2)
        xb2p = P("xb2cast", 2)
        psmm = P("psmm", pb[0], space="PSUM")
        psaux = P("psaux", 3, space="PSUM")

        # ---- persistent tiles ----
        X = xp.tile([128, 4, T + 16], F32, tag="X")
        for tq in range(NCH):
            xin = sqp.tile([128, 4, 512], BF16, tag="xsq2")
            nc.sync.dma_start(out=xin,
                              in_=d["x"][:, :, tq * 512:(tq + 1) * 512])
            nc.vector.tensor_copy(out=X[:, :, tq * 512:(tq + 1) * 512], in_=xin)
        ones = cons.tile([128, 1], BF16, tag="ones")       # value 1/512
        nc.sync.dma_start(out=ones, in_=cst["c_ones"])
        ones1 = cons.tile([1, 128], BF16, tag="ones1")     # bcast lhsT (1.0)
        nc.sync.dma_start(out=ones1, in_=cst["c_ones1"])
        e2 = cons.tile([64, 128], BF16, tag="e2")
        nc.sync.dma_start(out=e2, in_=cst["c_e2"])
        eps128 = cons.tile([128, 1], F32, tag="eps128")
        nc.vector.memset(eps128, EPS)
        onesf = cons.tile([128, 1], F32, tag="onesf")
        nc.vector.memset(onesf, 1.0 / 512.0)

        for l in range(NL):
            # layer weights (attention set + rows)
            wq = wpA.tile([128, 4, 512], BF16, tag="wq")
            wk = wpA.tile([128, 4, 512], BF16, tag="wk")
            wv = wpA.tile([128, 4, 512], BF16, tag="wv")
            wo = wpA.tile([128, 4, 512], BF16, tag="wo")
            eb = wpA.tile([128, NH, 288], BF16, tag="expb")
            bq = wpA.tile([128, 4], F32, tag="bq")
            bk = wpA.tile([128, 4], F32, tag="bk")
            bo = wpA.tile([128, 4], F32, tag="bo")
            bv = wpA.tile([128, 512], BF16, tag="bvb")
            g1 = wpA.tile([128, 4], F32, tag="g1")
            b1 = wpA.tile([128, 4], F32, tag="b1")
            g2 = wpA.tile([128, 4], F32, tag="g2")
            b2 = wpA.tile([128, 4], F32, tag="b2")
            for nm, t in (("wq", wq), ("wk", wk), ("wv", wv), ("wo", wo),
                          ("expb", eb), ("bq", bq), ("bk", bk), ("bo_c", bo),
                          ("bvb", bv), ("g1", g1), ("b1", b1), ("g2", g2),
                          ("b2", b2)):
                nc.sync.dma_start(out=t, in_=cst[nm][l])
            # ffn weights: issued now, consumed after LN1 (overlaps attention)
            w1 = wpF.tile([128, 4, FF], BF16, tag="w1")
            w2 = wpF.tile([128, 16, 512], BF16, tag="w2")
            bf1 = wpF.tile([128, 16], F32, tag="bf1")
            bf2 = wpF.tile([128, 4], F32, tag="bf2")
            for nm, t in (("w1", w1), ("w2", w2), ("bf1", bf1), ("bf2_c", bf2)):
                nc.sync.dma_start(out=t, in_=cst[nm][l])

            # ---------------- per-region LN (stats already in ps_st) --------
            def ln_region(cs, wdt, ps_st, g, b, last=False):
                # rows: mean (bf16), mean^2, var, ln(var+eps), rstd=exp(-.5ln)
                srow = rowp.tile([1, 512], BF16, tag="srow", name="srow")[:, :wdt]
                nc.scalar.activation(out=srow, in_=ps_st[0:1, :wdt],
                                     func=AF.Copy)
                m2 = rowp.tile([1, 512], F32, tag="m2row", name="m2")[:, :wdt]
                nc.vector.tensor_mul(m2, srow, srow)
                nc.vector.tensor_sub(m2, ps_st[32:33, :wdt], m2)   # var, in place
                nc.scalar.activation(out=m2, in_=m2, func=AF.Ln,
                                     bias=eps128[0:1, :])          # ln(var+eps)
                rrow = rowp.tile([1, 512], BF16, tag="rrow", name="rrow")[:, :wdt]
                nc.scalar.activation(out=rrow, in_=m2, func=AF.Exp, scale=-0.5)
                # broadcast rows to 128 partitions on the PE; stage to SBUF
                # bf16 immediately so the PSUM bank frees fast (tag "bc"
                # bufs=1 -> bm/br serialize through one bank)
                ps_bm = psaux.tile([128, 512], F32, tag="bc", name="ps_bm",
                                   bufs=1)[:, :wdt]
                nc.tensor.matmul(ps_bm, lhsT=ones1, rhs=srow,
                                 start=True, stop=True)
                bm = rowp.tile([128, 512], BF16, tag="bmsb", name="bm")[:, :wdt]
                nc.vector.tensor_copy(out=bm, in_=ps_bm)
                ps_br = psaux.tile([128, 512], F32, tag="bc", name="ps_br",
                                   bufs=1)[:, :wdt]
                nc.tensor.matmul(ps_br, lhsT=ones1, rhs=rrow,
                                 start=True, stop=True)
                br = rowp.tile([128, 512], BF16, tag="brsb", name="br")[:, :wdt]
                nc.vector.tensor_copy(out=br, in_=ps_br)
                xc = X[:, :, cs:cs + wdt]
                nc.vector.tensor_sub(xc, xc, _insdim_ap(bm, 0, 4))
                nc.vector.tensor_mul(xc, xc, _insdim_ap(br, 0, 4))
                ob = None
                if not trivial_gb:
                    if last:
                        ob = sqp.tile([128, 4, 512], BF16, tag="xsq2",
                                      name="ob")
                    for mc in range(4):
                        dst = ob[:, mc, 0:wdt] if last else X[:, mc, cs:cs + wdt]
                        nc.scalar.activation(out=dst, in_=X[:, mc, cs:cs + wdt],
                                             func=AF.Identity,
                                             bias=b[:, mc:mc + 1],
                                             scale=g[:, mc:mc + 1])
                elif last:
                    ob = sqp.tile([128, 4, 512], BF16, tag="xsq2", name="ob")
                    nc.vector.tensor_copy(out=ob[:, :, 0:wdt], in_=xc)
                if last:
                    nc.sync.dma_start(out=d["out"][:, :, cs:cs + wdt],
                                      in_=ob[:, :, 0:wdt])

            def stats_mms(xs_sl, xsq_sl, wdt):
                # mean at partition 0 (f32 MM), mean-square at partition 32
                # (bf16 MM). Lives in the fast-draining "mm" ring.
                ps_st = psmm.tile([33, 512], F32, tag="mm", name="ps_st")
                for kc in range(4):
                    nc.tensor.matmul(ps_st[0:1, :wdt], lhsT=onesf,
                                     rhs=xs_sl[:, kc, :],
                                     start=(kc == 0), stop=(kc == 3))
                for kc in range(4):
                    nc.tensor.matmul(ps_st[32:33, :wdt], lhsT=ones,
                                     rhs=xsq_sl[:, kc, :],
                                     start=(kc == 0), stop=(kc == 3))
                return ps_st

            # ---------------- per-pair attention emitter --------------------
            cast_cache = {}

            def cast_pair(p):
                cs0 = p * 288
                xb = xbp.tile([128, 4, 304], BF16, tag="xb")
                nc.gpsimd.tensor_copy(out=xb, in_=X[:, :, cs0:cs0 + 304])
                # tail tokens of both windows packed at cols {0:16, 32:48}
                xt = xbp.tile([128, 4, 64], BF16, tag="xt")
                nc.gpsimd.tensor_copy(
                    out=_insdim_ap(xt[:, :, 0:16], 32, 2, at=2),
                    in_=_insdim_ap(X[:, :, cs0 + 128:cs0 + 144], 144, 2, at=2))
                cast_cache[p] = (xb, xt)

            def att_pair(p):
                cs0 = p * 288
                xs = X[:, :, cs0:cs0 + 288]
                xb, xt = cast_cache.pop(p)
                qw = winp.tile([128, 4, 288], BF16, tag="qw")
                kw = winp.tile([128, 4, 288], BF16, tag="kw")
                for mc in range(4):
                    pq = psmm.tile([128, 288], F32, tag="mm")
                    for kc in range(4):
                        nc.tensor.matmul(pq, lhsT=wq[:, kc, mc * 128:(mc + 1) * 128],
                                         rhs=xb[:, kc, 0:288], start=(kc == 0), stop=(kc == 3))
                    if trivial_bias:
                        nc.vector.tensor_copy(out=qw[:, mc, :], in_=pq)
                    else:
                        nc.scalar.activation(out=qw[:, mc, :], in_=pq, func=AF.Identity,
                                             bias=bq[:, mc:mc + 1])
                    pk = psmm.tile([128, 288], F32, tag="mm")
                    for kc in range(4):
                        nc.tensor.matmul(pk, lhsT=wk[:, kc, mc * 128:(mc + 1) * 128],
                                         rhs=xb[:, kc, 0:288], start=(kc == 0), stop=(kc == 3))
                    nc.scalar.activation(out=kw[:, mc, :], in_=pk, func=AF.Identity,
                                         bias=bk[:, mc:mc + 1])

                vws = []
                for wi in (0, 1):
                    vw1 = winp.tile([128, NH, 65], BF16, tag=f"vw1_{wi}")
                    off = wi * 144
                    pv1 = psmm.tile([128, 512], F32, tag="mm")
                    for kc in range(4):
                        nc.tensor.matmul(pv1, lhsT=xb[:, kc, off:off + 128],
                                         rhs=wv[:, kc, :], start=(kc == 0), stop=(kc == 3))
                    nc.vector.tensor_add(out=vw1[:, :, 0:64],
                                         in0=pv1.rearrange("p (h e) -> p h e", h=NH),
                                         in1=bv.rearrange("p (h e) -> p h e", h=NH))
                    nc.vector.memset(vw1[:, :, 64:65], 1.0)
                    vws.append(vw1)
                # merged tail-V for both windows: lhsT cols {128:160, 272:304}
                # -> out partitions A-tail 0:16, (garbage 16:32), B-tail 32:48
                vw2p = winp.tile([64, NH, 65], BF16, tag="vw2p")
                pv2 = psmm.tile([64, 512], F32, tag="mm")
                for kc in range(4):
                    nc.tensor.matmul(pv2, lhsT=xt[:, kc, :], rhs=wv[:, kc, :],
                                     start=(kc == 0), stop=(kc == 3))
                nc.vector.tensor_add(out=vw2p[:, :, 0:64],
                                     in0=pv2.rearrange("p (h e) -> p h e", h=NH),
                                     in1=bv[0:64].rearrange("p (h e) -> p h e", h=NH))
                nc.vector.memset(vw2p[:, :, 64:65], 1.0)

                ocm = winp.tile([128, 4, 288], BF16, tag="ocm")
                if skip_heads:
                    nc.vector.tensor_copy(out=ocm, in_=xs)

                # software-pipelined head loop: stage A (S-mm, exp, P-mul) runs
                # `LOOKAHEAD` heads in front of stage B (PV, den) and stage C
                # (per head-pair: e2 bcast-mm, recip, ocm scale), so the PE has
                # independent matmuls queued while act/DVE chew on earlier heads.
                heads = [(wi, h) for wi in (0, 1) for h in range(NH)]
                pts = {}
                psos = {}

                def stage_a(wi, h):
                    off = wi * 144
                    tb = 32 * wi
                    ro, tl = (h % 2) * 64, h // 2
                    ps_s = psmm.tile([128, 288], F32, tag="mm")
                    nc.tensor.matmul(ps_s[:, 0:144],
                                     lhsT=kw[ro:ro + 64, tl, off:off + 128],
                                     rhs=qw[ro:ro + 64, tl, off:off + 144],
                                     start=True, stop=True)
                    nc.tensor.matmul(ps_s[tb:tb + 16, 144:288],
                                     lhsT=kw[ro:ro + 64, tl, off + 128:off + 144],
                                     rhs=qw[ro:ro + 64, tl, off:off + 144],
                                     start=True, stop=True)
                    et = etp.tile([128, 288], BF16, tag="e")
                    nc.scalar.activation(out=et, in_=ps_s, func=AF.Exp)
                    pt = ep.tile([128, 288], BF16, tag="p")
                    nc.vector.tensor_mul(pt, et, eb[:, h, :])
                    pts[(wi, h)] = pt

                def stage_b(wi, h):
                    pt = pts.pop((wi, h))
                    vw1 = vws[wi]
                    hp, hi = h // 2, h % 2
                    if hi == 0:
                        psos[(wi, hp)] = psaux.tile([65, 2, 144], F32,
                                                    tag="aux", name="ps_o2")
                    ps_o = psos[(wi, hp)]
                    nc.tensor.matmul(ps_o[:, hi, :], lhsT=vw1[:, h, :],
                                     rhs=pt[:, 0:144], start=True, stop=False)
                    tb = 32 * wi
                    nc.tensor.matmul(ps_o[:, hi, :],
                                     lhsT=vw2p[tb:tb + 16, h, :],
                                     rhs=pt[tb:tb + 16, 144:288],
                                     start=False, stop=True)
                    if hi == 1:
                        stage_c(wi, hp)

                def stage_c(wi, hpair):
                    off = wi * 144
                    ps_o = psos[(wi, hpair)]
                    # both heads' softmax denominators in one row copy
                    dp = rowp.tile([1, 288], BF16, tag="dpair", name="dp",
                                   bufs=4)
                    nc.scalar.activation(out=dp, in_=ps_o[64:65, 0:2, 0:144],
                                         func=AF.Copy)
                    ps_sc = psmm.tile([128, 144], F32, tag="mm")
                    nc.tensor.matmul(ps_sc[0:64, :], lhsT=ones1[0:1, 0:64],
                                     rhs=dp[0:1, 0:144], start=True, stop=True)
                    nc.tensor.matmul(ps_sc[64:128, :], lhsT=ones1[0:1, 0:64],
                                     rhs=dp[0:1, 144:288], start=True, stop=True)
                    sc = scp.tile([128, 144], F32, tag="scsb")
                    nc.vector.reciprocal_approx_fast(out=sc, in_=ps_sc)
                    p01 = psos.pop((wi, hpair))
                    nc.vector.tensor_mul(ocm[0:64, hpair, off:off + 144],
                                         p01[0:64, 0, :], sc[0:64, :])
                    nc.vector.tensor_mul(ocm[64:128, hpair, off:off + 144],
                                         p01[0:64, 1, :], sc[64:128, :])

                LOOKAHEAD = la
                for i, (wi, h) in enumerate(heads if not skip_heads else []):
                    stage_a(wi, h)
                    if i >= LOOKAHEAD:
                        stage_b(*heads[i - LOOKAHEAD])
                for j in (range(max(0, len(heads) - LOOKAHEAD), len(heads))
                          if not skip_heads else []):
                    stage_b(*heads[j])

                # O projection + residual -> X (pre-LN1), stats, LN1
                for mc in range(4):
                    po = psmm.tile([128, 288], F32, tag="mm")
                    for kc in range(4):
                        nc.tensor.matmul(po, lhsT=wo[:, kc, mc * 128:(mc + 1) * 128],
                                         rhs=ocm[:, kc, :], start=(kc == 0), stop=(kc == 3))
                    nc.vector.tensor_add(out=X[:, mc, cs0:cs0 + 288], in0=po,
                                         in1=X[:, mc, cs0:cs0 + 288])
                    if not trivial_bias:
                        nc.vector.tensor_add(out=X[:, mc, cs0:cs0 + 288],
                                             in0=X[:, mc, cs0:cs0 + 288],
                                             in1=bo[:, mc:mc + 1].broadcast_to([128, 288]))
                xsq = sqp.tile([128, 4, 288], BF16, tag="xsq")
                nc.gpsimd.tensor_mul(xsq, xs, xs)
                ps_st = stats_mms(xs, xsq, 288)
                ln_region(cs0, 288, ps_st, g1, b1)

            # ---------------- FFN chunk emitter (incl. LN2) ----------------
            lastl = (l == NL - 1)

            def ffn_chunk(cc):
                cs = cc * 512
                xc = X[:, :, cs:cs + 512]
                xb2 = xb2p.tile([128, 4, 512], BF16, tag="xb2")
                for kc in range(4):
                    nc.gpsimd.tensor_copy(out=xb2[:, kc, :], in_=xc[:, kc, :])
                hb = hp.tile([128, 16, 512], BF16, tag="hb")
                for fc in range(16):
                    ph = psmm.tile([128, 512], F32, tag="mm")
                    for kc in range(4):
                        nc.tensor.matmul(ph, lhsT=w1[:, kc, fc * 128:(fc + 1) * 128],
                                         rhs=xb2[:, kc, :], start=(kc == 0), stop=(kc == 3))
                    if fc % 2 == 0:
                        nc.scalar.activation(out=hb[:, fc, :], in_=ph, func=AF.Relu,
                                             bias=bf1[:, fc:fc + 1])
                    else:
                        nc.vector.tensor_scalar(
                            out=hb[:, fc, :], in0=ph, scalar1=bf1[:, fc:fc + 1],
                            scalar2=0.0, op0=mybir.AluOpType.add,
                            op1=mybir.AluOpType.max)
                for mc in range(4):
                    pf = psmm.tile([128, 512], F32, tag="mm")
                    for fc in range(16):
                        nc.tensor.matmul(pf, lhsT=w2[:, fc, mc * 128:(mc + 1) * 128],
                                         rhs=hb[:, fc, :], start=(fc == 0), stop=(fc == 15))
                    nc.vector.tensor_add(out=X[:, mc, cs:cs + 512], in0=pf,
                                         in1=X[:, mc, cs:cs + 512])
                    if not trivial_bias:
                        nc.vector.tensor_add(out=X[:, mc, cs:cs + 512],
                                             in0=X[:, mc, cs:cs + 512],
                                             in1=bf2[:, mc:mc + 1].broadcast_to([128, 512]))
                xsq = sqp.tile([128, 4, 512], BF16, tag="xsq2")
                nc.gpsimd.tensor_mul(xsq, xc, xc)
                ps_st = stats_mms(xc, xsq, 512)
                ln_region(cs, 512, ps_st, g2, b2, lastl)

            # ---------------- layer schedule -----------------------------
            if not skip_attn:
                # ffn chunk c runs one pair AFTER its LN1 coverage completes,
                # so the LN1 row/bcast/apply chain and the xb2 casts hide
                # under the next pair's attention burst.
                nxt = 0
                cast_pair(0)
                for p in range(PAIRS):
                    if p + 1 < PAIRS:
                        cast_pair(p + 1)
                    if not skip_ffn:
                        # emit in batches of >=2 so the PE gets long warm
                        # N=512 streams (HAM stays at K=8/8)
                        avail = min(NCH, (p * 288) // 512) - nxt
                        if avail >= 2 or (p == PAIRS - 1 and avail > 0):
                            for _ in range(avail):
                                ffn_chunk(nxt)
                                nxt += 1
                    att_pair(p)
                if not skip_ffn:
                    while nxt < NCH:
                        ffn_chunk(nxt)
                        nxt += 1
            elif not skip_ffn:
                for cc in range(NCH):
                    ffn_chunk(cc)
            if skip_ffn and l == NL - 1:
                for cc in range(NCH):
                    cs = cc * 512
                    nc.sync.dma_start(out=d["out"][:, :, cs:cs + 512],
                                      in_=X[:, :, cs:cs + 512])

    return d


# ---------------------------------------------------------------------------
# Host-side packing + golden model
# ---------------------------------------------------------------------------

def rel_idx():
    coords = np.stack(np.meshgrid(np.arange(WS), np.arange(WS), indexing="ij"))
    flat = coords.reshape(2, -1)
    rel = (flat[:, :, None] - flat[:, None, :]).transpose(1, 2, 0).copy()
    rel[..., 0] += WS - 1
    rel[..., 1] += WS - 1
    rel[..., 0] *= 2 * WS - 1
    return rel.sum(-1)  # [N, N] int


def pack_weights(w, NL):
    """w: dict of reference arrays -> dict of const arrays (np)."""
    bf = ml_dtypes.bfloat16
    scale = HD ** -0.5
    ridx = rel_idx()
    out = {}

    def lhsT_pack(W, kchunks):  # [Cin, Cout] -> [128, kchunks, Cout]
        return np.ascontiguousarray(
            W.reshape(kchunks, 128, W.shape[1]).transpose(1, 0, 2)
        )

    wq = np.stack([lhsT_pack(w["Wq"][l] * scale, 4) for l in range(NL)])
    wk = np.stack([lhsT_pack(w["Wk"][l], 4) for l in range(NL)])
    wv = np.stack([lhsT_pack(w["Wv"][l], 4) for l in range(NL)])
    wo = np.stack([lhsT_pack(w["Wo"][l], 4) for l in range(NL)])
    w1 = np.stack([lhsT_pack(w["W1"][l], 4) for l in range(NL)])
    w2 = np.stack([lhsT_pack(w["W2"][l], 16) for l in range(NL)])
    for nm, arr in (("wq", wq), ("wk", wk), ("wv", wv), ("wo", wo),
                    ("w1", w1), ("w2", w2)):
        out[nm] = arr.astype(bf)

    expb = np.zeros((NL, 128, NH, 288), np.float32)
    for l in range(NL):
        bias = w["rpb"][l][ridx]            # [N(i), N(j), NH]
        ebT = np.exp(bias.transpose(2, 1, 0))  # [NH, j, i]
        expb[l, 0:128, :, 0:144] = ebT[:, 0:128, :].transpose(1, 0, 2)
        expb[l, 0:16, :, 144:288] = ebT[:, 128:144, :].transpose(1, 0, 2)
        expb[l, 32:48, :, 144:288] = ebT[:, 128:144, :].transpose(1, 0, 2)
    out["expb"] = expb.astype(bf)

    def percol(b):  # [NL, C] -> [NL, 128, 4]
        return np.ascontiguousarray(
            b.reshape(NL, 4, 128).transpose(0, 2, 1)).astype(np.float32)

    out["bq"] = percol(w["bq"] * scale)
    out["bk"] = percol(w["bk"])
    out["bo_c"] = percol(w["bo"])
    out["bf2_c"] = percol(w["bf2"])
    out["c_ones1"] = np.ones((1, 128), bf)
    e2 = np.zeros((64, 128), np.float32)
    e2[0, 0:64] = 1.0
    e2[32, 64:128] = 1.0
    out["c_e2"] = e2.astype(bf)
    out["g1"] = percol(w["g1"])
    out["b1"] = percol(w["b1"])
    out["g2"] = percol(w["g2"])
    out["b2"] = percol(w["b2"])
    out["bf1"] = np.ascontiguousarray(
        w["bf1"].reshape(NL, 16, 128).transpose(0, 2, 1)).astype(np.float32)
    out["bvb"] = np.broadcast_to(
        w["bv"].astype(bf)[:, None, :], (NL, 128, 512)).copy()
    out["c_ones"] = np.full((128, 1), 1.0 / 512.0, bf)
    return out


def golden_tm(x_tm, w, NL):
    """fp32 numpy reference on window-major token-major x [T, 512]."""
    T = x_tm.shape[0]
    NW = T // N
    ridx = rel_idx()
    x = x_tm.astype(np.float32)

    def ln(v, g, b):
        m = v.mean(-1, keepdims=True)
        s = v.var(-1, keepdims=True)
        return (v - m) / np.sqrt(s + EPS) * g + b

    for l in range(NL):
        xw = x.reshape(NW, N, C)
        q = (xw @ w["Wq"][l] + w["bq"][l]).reshape(NW, N, NH, HD).transpose(0, 2, 1, 3)
        k = (xw @ w["Wk"][l] + w["bk"][l]).reshape(NW, N, NH, HD).transpose(0, 2, 1, 3)
        v = (xw @ w["Wv"][l] + w["bv"][l]).reshape(NW, N, NH, HD).transpose(0, 2, 1, 3)
        bias = w["rpb"][l][ridx].transpose(2, 0, 1)
        attn = np.einsum("whid,whjd->whij", q, k) * (HD ** -0.5) + bias
        attn = attn - attn.max(-1, keepdims=True)
        p = np.exp(attn)
        p = p / p.sum(-1, keepdims=True)
        o = np.einsum("whij,whjd->whid", p, v).transpose(0, 2, 1, 3).reshape(NW, N, C)
        o = o @ w["Wo"][l] + w["bo"][l]
        x = ln(o.reshape(T, C) + x, w["g1"][l], w["b1"][l])
        h = np.maximum(x @ w["W1"][l] + w["bf1"][l], 0.0) @ w["W2"][l] + w["bf2"][l]
        x = ln(h + x, w["g2"][l], w["b2"][l])
    return x


# ---------------------------------------------------------------------------
# kernel() entry point: full inputs -> full output, 8-way batch data parallel
# ---------------------------------------------------------------------------

NCORES = 8
B_FULL = 64
H_RES = W_RES = 24
L_TOK = H_RES * W_RES
NW_FULL = (B_FULL // NCORES) * (H_RES // WS) * (W_RES // WS)   # 32 windows/core
NL_FULL = 3

_COMPILED = {}


def _pack_x_all(x):
    """[64, 576, 512] f32 -> [8, 128, 4, T] bf16 channel-major window-major."""
    b = x.reshape(NCORES, B_FULL // NCORES, 2, WS, 2, WS, 4, 128)
    v = b.transpose(0, 7, 6, 1, 2, 4, 3, 5)   # [core,128,4, b,hw,ww,hs,ws]
    return np.ascontiguousarray(v.reshape(NCORES, 128, 4, -1)
                                ).astype(ml_dtypes.bfloat16)


def _unpack_out_all(res_list):
    """list of [128, 4, T] -> [64, 576, 512] f32."""
    y = np.stack([r.astype(np.float32) for r in res_list])     # [8,128,4,T]
    bpc = B_FULL // NCORES
    v = y.reshape(NCORES, 128, 4, bpc, 2, 2, WS, WS)
    v = v.transpose(0, 3, 4, 6, 5, 7, 2, 1)   # [core,b,hw,hs,ww,ws,4,128]
    return np.ascontiguousarray(v.reshape(B_FULL, L_TOK, C))


def kernel(x, Wq, bq, Wk, bk, Wv, bv, Wo, bo, rpb,
           g1, b1, W1, bf1, W2, bf2, g2, b2):
    import hashlib
    from concourse.bass_utils import run_bass_kernel_spmd

    w = {"Wq": np.asarray(Wq, np.float32), "bq": np.asarray(bq, np.float32),
         "Wk": np.asarray(Wk, np.float32), "bk": np.asarray(bk, np.float32),
         "Wv": np.asarray(Wv, np.float32), "bv": np.asarray(bv, np.float32),
         "Wo": np.asarray(Wo, np.float32), "bo": np.asarray(bo, np.float32),
         "rpb": np.asarray(rpb, np.float32),
         "g1": np.asarray(g1, np.float32), "b1": np.asarray(b1, np.float32),
         "W1": np.asarray(W1, np.float32), "bf1": np.asarray(bf1, np.float32),
         "W2": np.asarray(W2, np.float32), "bf2": np.asarray(bf2, np.float32),
         "g2": np.asarray(g2, np.float32), "b2": np.asarray(b2, np.float32)}
    hsh = hashlib.blake2b(
        b"".join(np.ascontiguousarray(v).tobytes() for v in w.values()),
        digest_size=16).hexdigest()
    if _COMPILED.get("hash") != hsh:
        packed = pack_weights(w, NL_FULL)
        trivial_gb = bool(np.all(w["g1"] == 1) and np.all(w["b1"] == 0)
                          and np.all(w["g2"] == 1) and np.all(w["b2"] == 0))
        trivial_bias = bool(all(np.all(w[k] == 0)
                                for k in ("bq", "bk", "bv", "bo", "bf1", "bf2")))
        nc = bacc.Bacc("TRN2", target_bir_lowering=False, debug=False)
        build(nc, NW_FULL, NL_FULL, packed,
              trivial_gb=trivial_gb, trivial_bias=trivial_bias)
        nc.compile()
        _COMPILED.update(hash=hsh, nc=nc)

    xp = _pack_x_all(np.asarray(x, np.float32))
    in_maps = [{"x": xp[i]} for i in range(NCORES)]
    res = run_bass_kernel_spmd(_COMPILED["nc"], in_maps, list(range(NCORES)))
    return _unpack_out_all([res.results[i]["out"] for i in range(NCORES)])



# revision 27
# speedup vs baseline: 1.0826x; 1.0111x over previous
"""Swin-style window-attention encoder as a Bass/Tile kernel for TRN2 — v3.

Key design vs v1:
- Residual master X lives in SBUF as FP32 [128, 4, T+16] (channel-major) —
  the residual stream never rounds to bf16 (bf16 master measured 2.2e-2 max
  rel err, over the 2e-2 budget; f32 master 7.8e-3). Matmul inputs are
  bf16 copies cast on the (otherwise idle) GPSIMD engine; per-token mean
  stats contract the f32 master directly (f32 matmul, tiny N).
- Weights are baked into the NEFF as inline consts — per-launch IO is just
  x (bf16 in) and out (bf16).
- Per-layer phase batching: [attention (Exp table)] -> [LN1 rows (Sqrt) +
  apply] -> [FFN (Relu, no table load)] -> [LN2 rows + apply]. 2 activation
  table loads per layer instead of ~64.
- Attention softmax denominators: collected per head into smat rows (act
  Copy), broadcast to 128 partitions via one e2 matmul, ONE fat [128,144]
  DVE reciprocal per head-pair (no 1-lane recips).
- LN row math on [128,36] shuffled layout (SBUF->SBUF strided DMA), not
  1-lane [1,T] ops.
- O-proj / QK-proj / stats at window-pair (288 tokens) granularity; FFN at
  512-token chunks.
"""
from contextlib import ExitStack

import numpy as np
import ml_dtypes

import concourse.bass as bass
import concourse.bacc as bacc
import concourse.tile as tile
import concourse.mybir as mybir

F32 = mybir.dt.float32
F32R = mybir.dt.float32r
BF16 = mybir.dt.bfloat16
AF = mybir.ActivationFunctionType


def _patch_act_tables():
    """Make the act-table-load pass resolve Exp AND Ln to the combined
    `natural_log_exp_and_others` set (it otherwise greedily alternates
    between `exp_and_others` and `natural_log`, reloading tables at every
    attention<->LN boundary, ~150 loads/kernel). We hide exp/ln from every
    other set in the table list the pass consults; set IDs (list order)
    are unchanged, so walrus still emits the right act.json entries."""
    import concourse.hw_specs as hw_specs

    if getattr(bacc, "_ant_act_tables_patched", False):
        return
    orig = hw_specs.get_activation_tables

    def patched(arch):
        tabs = orig(arch)
        exp, ln = AF.Exp, AF.Ln
        if "natural_log_exp_and_others" in tabs:
            for name, fns in tabs.items():
                if name != "natural_log_exp_and_others":
                    fns.discard(exp)
                    fns.discard(ln)
        return tabs

    bacc.get_activation_tables = patched
    bacc._ant_act_tables_patched = True


_patch_act_tables()

WS = 12
N = WS * WS          # 144 tokens per window
C = 512
NH = 8
HD = 64
FF = 2048
EPS = 1e-5


def _insdim_ap(row_ap, stride, num, at=1):
    """Insert a dim of (stride, num) at position `at` of the AP (default:
    right after the partition dim). stride=0 -> broadcast; else gather."""
    dims = [list(d) for d in row_ap.ap]
    return bass.AP(
        tensor=row_ap.tensor,
        offset=row_ap.offset,
        ap=dims[:at] + [[stride, num]] + dims[at:],
    )


def _bcast_ap(row_ap, parts):
    return _insdim_ap(row_ap, 0, parts)


def build(nc: bass.Bass, NW: int, NL: int, w: dict,
          skip_attn=False, skip_ffn=False, skip_heads=False,
          pb=(4, 4), winb=2, epb=5, sqb=1, hbb=1, scb=2, la=4, xbb=2,
          trivial_gb=False, trivial_bias=False):
    """w: packed numpy weight dict (see pack_weights)."""
    T = NW * N
    PAIRS = NW // 2
    NCH = T // 512
    assert T % 512 == 0

    d = {}
    d["x"] = nc.dram_tensor("x", [128, 4, T], BF16, kind="ExternalInput").ap()
    d["out"] = nc.dram_tensor("out", [128, 4, T], BF16, kind="ExternalOutput").ap()
    cst = {nm: nc.inline_tensor(arr, name=nm).ap() for nm, arr in w.items()}

    with tile.TileContext(nc) as tc, ExitStack() as ctx:
        P = lambda name, bufs, **kw: ctx.enter_context(
            tc.tile_pool(name=name, bufs=bufs, **kw)
        )
        xp = P("xmaster", 1)
        cons = P("consts", 1)
        wpA = P("wtsA", 1)     # attention-phase weights
        wpF = P("wtsF", 1)     # ffn-phase weights
        winp = P("win", winb)  # per-pair working tiles
        ep = P("eptiles", epb)  # P tiles
        etp = P("ettiles", 2)   # exp tiles (short-lived)
        sqp = P("sqtiles", sqb)  # squared-x tiles for stats
        scp = P("sctiles", scb)  # recip rows [128,144]
        rowp = P("rows", 2)    # LN stat rows (short-lived, per region)
        hp = P("hbuf", hbb)
        xbp = P("xbcast", 2)
        xb2p = P("xb2cast", 2)
        psmm = P("psmm", pb[0], space="PSUM")
        psaux = P("psaux", 3, space="PSUM")

        # ---- persistent tiles ----
        X = xp.tile([128, 4, T + 16], F32, tag="X")
        for tq in range(NCH):
            xin = sqp.tile([128, 4, 512], BF16, tag="xsq2")
            nc.sync.dma_start(out=xin,
                              in_=d["x"][:, :, tq * 512:(tq + 1) * 512])
            nc.vector.tensor_copy(out=X[:, :, tq * 512:(tq + 1) * 512], in_=xin)
        ones = cons.tile([128, 1], BF16, tag="ones")       # value 1/512
        nc.sync.dma_start(out=ones, in_=cst["c_ones"])
        ones1 = cons.tile([1, 128], BF16, tag="ones1")     # bcast lhsT (1.0)
        nc.sync.dma_start(out=ones1, in_=cst["c_ones1"])
        e2 = cons.tile([64, 128], BF16, tag="e2")
        nc.sync.dma_start(out=e2, in_=cst["c_e2"])
        eps128 = cons.tile([128, 1], F32, tag="eps128")
        nc.vector.memset(eps128, EPS)
        onesf = cons.tile([128, 1], F32, tag="onesf")
        nc.vector.memset(onesf, 1.0 / 512.0)

        for l in range(NL):
            # layer weights (attention set + rows)
            wq = wpA.tile([128, 4, 512], BF16, tag="wq")
            wk = wpA.tile([128, 4, 512], BF16, tag="wk")
            wv = wpA.tile([128, 4, 512], BF16, tag="wv")
            wo = wpA.tile([128, 4, 512], BF16, tag="wo")
            eb = wpA.tile([128, NH, 288], BF16, tag="expb")
            bq = wpA.tile([128, 4], F32, tag="bq")
            bk = wpA.tile([128, 4], F32, tag="bk")
            bo = wpA.tile([128, 4], F32, tag="bo")
            bv = wpA.tile([128, 512], BF16, tag="bvb")
            g1 = wpA.tile([128, 4], F32, tag="g1")
            b1 = wpA.tile([128, 4], F32, tag="b1")
            g2 = wpA.tile([128, 4], F32, tag="g2")
            b2 = wpA.tile([128, 4], F32, tag="b2")
            for nm, t in (("wq", wq), ("wk", wk), ("wv", wv), ("wo", wo),
                          ("expb", eb), ("bq", bq), ("bk", bk), ("bo_c", bo),
                          ("bvb", bv), ("g1", g1), ("b1", b1), ("g2", g2),
                          ("b2", b2)):
                nc.sync.dma_start(out=t, in_=cst[nm][l])
            # ffn weights: issued now, consumed after LN1 (overlaps attention)
            w1 = wpF.tile([128, 4, FF], BF16, tag="w1")
            w2 = wpF.tile([128, 16, 512], BF16, tag="w2")
            bf1 = wpF.tile([128, 16], F32, tag="bf1")
            bf2 = wpF.tile([128, 4], F32, tag="bf2")
            for nm, t in (("w1", w1), ("w2", w2), ("bf1", bf1), ("bf2_c", bf2)):
                nc.sync.dma_start(out=t, in_=cst[nm][l])

            # ---------------- per-region LN (stats already in ps_st) --------
            def ln_region(cs, wdt, ps_st, g, b, last=False):
                # rows: mean (bf16), mean^2, var, ln(var+eps), rstd=exp(-.5ln)
                srow = rowp.tile([1, 512], BF16, tag="srow", name="srow")[:, :wdt]
                nc.scalar.activation(out=srow, in_=ps_st[0:1, :wdt],
                                     func=AF.Copy)
                m2 = rowp.tile([1, 512], F32, tag="m2row", name="m2")[:, :wdt]
                nc.vector.tensor_mul(m2, srow, srow)
                nc.vector.tensor_sub(m2, ps_st[32:33, :wdt], m2)   # var, in place
                nc.scalar.activation(out=m2, in_=m2, func=AF.Ln,
                                     bias=eps128[0:1, :])          # ln(var+eps)
                rrow = rowp.tile([1, 512], BF16, tag="rrow", name="rrow")[:, :wdt]
                nc.scalar.activation(out=rrow, in_=m2, func=AF.Exp, scale=-0.5)
                # broadcast rows to 128 partitions on the PE; stage to SBUF
                # bf16 immediately so the PSUM bank frees fast (tag "bc"
                # bufs=1 -> bm/br serialize through one bank)
                ps_bm = psaux.tile([128, 512], F32, tag="bc", name="ps_bm",
                                   bufs=1)[:, :wdt]
                nc.tensor.matmul(ps_bm, lhsT=ones1, rhs=srow,
                                 start=True, stop=True)
                bm = rowp.tile([128, 512], BF16, tag="bmsb", name="bm")[:, :wdt]
                nc.vector.tensor_copy(out=bm, in_=ps_bm)
                ps_br = psaux.tile([128, 512], F32, tag="bc", name="ps_br",
                                   bufs=1)[:, :wdt]
                nc.tensor.matmul(ps_br, lhsT=ones1, rhs=rrow,
                                 start=True, stop=True)
                br = rowp.tile([128, 512], BF16, tag="brsb", name="br")[:, :wdt]
                nc.vector.tensor_copy(out=br, in_=ps_br)
                xc = X[:, :, cs:cs + wdt]
                nc.vector.tensor_sub(xc, xc, _insdim_ap(bm, 0, 4))
                nc.vector.tensor_mul(xc, xc, _insdim_ap(br, 0, 4))
                ob = None
                if not trivial_gb:
                    if last:
                        ob = sqp.tile([128, 4, 512], BF16, tag="xsq2",
                                      name="ob")
                    for mc in range(4):
                        dst = ob[:, mc, 0:wdt] if last else X[:, mc, cs:cs + wdt]
                        nc.scalar.activation(out=dst, in_=X[:, mc, cs:cs + wdt],
                                             func=AF.Identity,
                                             bias=b[:, mc:mc + 1],
                                             scale=g[:, mc:mc + 1])
                elif last:
                    ob = sqp.tile([128, 4, 512], BF16, tag="xsq2", name="ob")
                    nc.vector.tensor_copy(out=ob[:, :, 0:wdt], in_=xc)
                if last:
                    nc.sync.dma_start(out=d["out"][:, :, cs:cs + wdt],
                                      in_=ob[:, :, 0:wdt])

            def stats_mms(xs_sl, xsq_sl, wdt):
                # mean at partition 0 (f32 MM), mean-square at partition 32
                # (bf16 MM). Lives in the fast-draining "mm" ring.
                ps_st = psmm.tile([33, 512], F32, tag="mm", name="ps_st")
                for kc in range(4):
                    nc.tensor.matmul(ps_st[0:1, :wdt], lhsT=onesf,
                                     rhs=xs_sl[:, kc, :],
                                     start=(kc == 0), stop=(kc == 3))
                for kc in range(4):
                    nc.tensor.matmul(ps_st[32:33, :wdt], lhsT=ones,
                                     rhs=xsq_sl[:, kc, :],
                                     start=(kc == 0), stop=(kc == 3))
                return ps_st

            # ---------------- per-pair attention emitter --------------------
            cast_cache = {}

            def cast_pair(p):
                cs0 = p * 288
                xb = xbp.tile([128, 4, 304], BF16, tag="xb")
                nc.gpsimd.tensor_copy(out=xb, in_=X[:, :, cs0:cs0 + 304])
                # tail tokens of both windows packed at cols {0:16, 32:48}
                xt = xbp.tile([128, 4, 64], BF16, tag="xt")
                nc.gpsimd.tensor_copy(
                    out=_insdim_ap(xt[:, :, 0:16], 32, 2, at=2),
                    in_=_insdim_ap(X[:, :, cs0 + 128:cs0 + 144], 144, 2, at=2))
                cast_cache[p] = (xb, xt)

            def att_pair(p):
                cs0 = p * 288
                xs = X[:, :, cs0:cs0 + 288]
                xb, xt = cast_cache.pop(p)
                qw = winp.tile([128, 4, 288], BF16, tag="qw")
                kw = winp.tile([128, 4, 288], BF16, tag="kw")
                for mc in range(4):
                    pq = psmm.tile([128, 288], F32, tag="mm")
                    for kc in range(4):
                        nc.tensor.matmul(pq, lhsT=wq[:, kc, mc * 128:(mc + 1) * 128],
                                         rhs=xb[:, kc, 0:288], start=(kc == 0), stop=(kc == 3))
                    if trivial_bias:
                        nc.vector.tensor_copy(out=qw[:, mc, :], in_=pq)
                    else:
                        nc.scalar.activation(out=qw[:, mc, :], in_=pq, func=AF.Identity,
                                             bias=bq[:, mc:mc + 1])
                    pk = psmm.tile([128, 288], F32, tag="mm")
                    for kc in range(4):
                        nc.tensor.matmul(pk, lhsT=wk[:, kc, mc * 128:(mc + 1) * 128],
                                         rhs=xb[:, kc, 0:288], start=(kc == 0), stop=(kc == 3))
                    nc.scalar.activation(out=kw[:, mc, :], in_=pk, func=AF.Identity,
                                         bias=bk[:, mc:mc + 1])

                vws = []
                for wi in (0, 1):
                    vw1 = winp.tile([128, NH, 65], BF16, tag=f"vw1_{wi}")
                    off = wi * 144
                    pv1 = psmm.tile([128, 512], F32, tag="mm")
                    for kc in range(4):
                        nc.tensor.matmul(pv1, lhsT=xb[:, kc, off:off + 128],
                                         rhs=wv[:, kc, :], start=(kc == 0), stop=(kc == 3))
                    nc.vector.tensor_add(out=vw1[:, :, 0:64],
                                         in0=pv1.rearrange("p (h e) -> p h e", h=NH),
                                         in1=bv.rearrange("p (h e) -> p h e", h=NH))
                    nc.vector.memset(vw1[:, :, 64:65], 1.0)
                    vws.append(vw1)
                # merged tail-V for both windows: lhsT cols {128:160, 272:304}
                # -> out partitions A-tail 0:16, (garbage 16:32), B-tail 32:48
                vw2p = winp.tile([64, NH, 65], BF16, tag="vw2p")
                pv2 = psmm.tile([64, 512], F32, tag="mm")
                for kc in range(4):
                    nc.tensor.matmul(pv2, lhsT=xt[:, kc, :], rhs=wv[:, kc, :],
                                     start=(kc == 0), stop=(kc == 3))
                nc.vector.tensor_add(out=vw2p[:, :, 0:64],
                                     in0=pv2.rearrange("p (h e) -> p h e", h=NH),
                                     in1=bv[0:64].rearrange("p (h e) -> p h e", h=NH))
                nc.vector.memset(vw2p[:, :, 64:65], 1.0)

                ocm = winp.tile([128, 4, 288], BF16, tag="ocm")
                if skip_heads:
                    nc.vector.tensor_copy(out=ocm, in_=xs)

                # software-pipelined head loop: stage A (S-mm, exp, P-mul) runs
                # `LOOKAHEAD` heads in front of stage B (PV, den) and stage C
                # (per head-pair: e2 bcast-mm, recip, ocm scale), so the PE has
                # independent matmuls queued while act/DVE chew on earlier heads.
                heads = [(wi, h) for wi in (0, 1) for h in range(NH)]
                pts = {}
                psos = {}

                def stage_a(wi, h):
                    off = wi * 144
                    tb = 32 * wi
                    ro, tl = (h % 2) * 64, h // 2
                    ps_s = psmm.tile([128, 288], F32, tag="mm")
                    nc.tensor.matmul(ps_s[:, 0:144],
                                     lhsT=kw[ro:ro + 64, tl, off:off + 128],
                                     rhs=qw[ro:ro + 64, tl, off:off + 144],
                                     start=True, stop=True)
                    nc.tensor.matmul(ps_s[tb:tb + 16, 144:288],
                                     lhsT=kw[ro:ro + 64, tl, off + 128:off + 144],
                                     rhs=qw[ro:ro + 64, tl, off:off + 144],
                                     start=True, stop=True)
                    et = etp.tile([128, 288], BF16, tag="e")
                    nc.scalar.activation(out=et, in_=ps_s, func=AF.Exp)
                    pt = ep.tile([128, 288], BF16, tag="p")
                    nc.vector.tensor_mul(pt, et, eb[:, h, :])
                    pts[(wi, h)] = pt

                def stage_b(wi, h):
                    pt = pts.pop((wi, h))
                    vw1 = vws[wi]
                    hp, hi = h // 2, h % 2
                    if hi == 0:
                        psos[(wi, hp)] = psaux.tile([65, 2, 144], F32,
                                                    tag="aux", name="ps_o2")
                    ps_o = psos[(wi, hp)]
                    nc.tensor.matmul(ps_o[:, hi, :], lhsT=vw1[:, h, :],
                                     rhs=pt[:, 0:144], start=True, stop=False)
                    tb = 32 * wi
                    nc.tensor.matmul(ps_o[:, hi, :],
                                     lhsT=vw2p[tb:tb + 16, h, :],
                                     rhs=pt[tb:tb + 16, 144:288],
                                     start=False, stop=True)
                    if hi == 1:
                        stage_c(wi, hp)

                def stage_c(wi, hpair):
                    off = wi * 144
                    ps_o = psos[(wi, hpair)]
                    # both heads' softmax denominators in one row copy
                    dp = rowp.tile([1, 288], BF16, tag="dpair", name="dp",
                                   bufs=4)
                    nc.scalar.activation(out=dp, in_=ps_o[64:65, 0:2, 0:144],
                                         func=AF.Copy)
                    ps_sc = psmm.tile([128, 144], F32, tag="mm")
                    nc.tensor.matmul(ps_sc[0:64, :], lhsT=ones1[0:1, 0:64],
                                     rhs=dp[0:1, 0:144], start=True, stop=True)
                    nc.tensor.matmul(ps_sc[64:128, :], lhsT=ones1[0:1, 0:64],
                                     rhs=dp[0:1, 144:288], start=True, stop=True)
                    sc = scp.tile([128, 144], F32, tag="scsb")
                    nc.vector.reciprocal_approx_fast(out=sc, in_=ps_sc)
                    p01 = psos.pop((wi, hpair))
                    nc.vector.tensor_mul(ocm[0:64, hpair, off:off + 144],
                                         p01[0:64, 0, :], sc[0:64, :])
                    nc.vector.tensor_mul(ocm[64:128, hpair, off:off + 144],
                                         p01[0:64, 1, :], sc[64:128, :])

                LOOKAHEAD = la
                for i, (wi, h) in enumerate(heads if not skip_heads else []):
                    stage_a(wi, h)
                    if i >= LOOKAHEAD:
                        stage_b(*heads[i - LOOKAHEAD])
                for j in (range(max(0, len(heads) - LOOKAHEAD), len(heads))
                          if not skip_heads else []):
                    stage_b(*heads[j])

                # O projection + residual -> X (pre-LN1), stats, LN1
                for mc in range(4):
                    po = psmm.tile([128, 288], F32, tag="mm")
                    for kc in range(4):
                        nc.tensor.matmul(po, lhsT=wo[:, kc, mc * 128:(mc + 1) * 128],
                                         rhs=ocm[:, kc, :], start=(kc == 0), stop=(kc == 3))
                    nc.vector.tensor_add(out=X[:, mc, cs0:cs0 + 288], in0=po,
                                         in1=X[:, mc, cs0:cs0 + 288])
                    if not trivial_bias:
                        nc.vector.tensor_add(out=X[:, mc, cs0:cs0 + 288],
                                             in0=X[:, mc, cs0:cs0 + 288],
                                             in1=bo[:, mc:mc + 1].broadcast_to([128, 288]))
                xsq = sqp.tile([128, 4, 288], BF16, tag="xsq")
                nc.gpsimd.tensor_mul(xsq, xs, xs)
                ps_st = stats_mms(xs, xsq, 288)
                ln_region(cs0, 288, ps_st, g1, b1)

            # ---------------- FFN chunk emitter (incl. LN2) ----------------
            lastl = (l == NL - 1)

            def ffn_chunk(cc):
                cs = cc * 512
                xc = X[:, :, cs:cs + 512]
                xb2 = xb2p.tile([128, 4, 512], BF16, tag="xb2")
                for kc in range(4):
                    nc.gpsimd.tensor_copy(out=xb2[:, kc, :], in_=xc[:, kc, :])
                hb = hp.tile([128, 16, 512], BF16, tag="hb")
                for fc in range(16):
                    ph = psmm.tile([128, 512], F32, tag="mm")
                    for kc in range(4):
                        nc.tensor.matmul(ph, lhsT=w1[:, kc, fc * 128:(fc + 1) * 128],
                                         rhs=xb2[:, kc, :], start=(kc == 0), stop=(kc == 3))
                    if fc % 2 == 0:
                        nc.scalar.activation(out=hb[:, fc, :], in_=ph, func=AF.Relu,
                                             bias=bf1[:, fc:fc + 1])
                    else:
                        nc.vector.tensor_scalar(
                            out=hb[:, fc, :], in0=ph, scalar1=bf1[:, fc:fc + 1],
                            scalar2=0.0, op0=mybir.AluOpType.add,
                            op1=mybir.AluOpType.max)
                for mc in range(4):
                    pf = psmm.tile([128, 512], F32, tag="mm")
                    for fc in range(16):
                        nc.tensor.matmul(pf, lhsT=w2[:, fc, mc * 128:(mc + 1) * 128],
                                         rhs=hb[:, fc, :], start=(fc == 0), stop=(fc == 15))
                    nc.vector.tensor_add(out=X[:, mc, cs:cs + 512], in0=pf,
                                         in1=X[:, mc, cs:cs + 512])
                    if not trivial_bias:
                        nc.vector.tensor_add(out=X[:, mc, cs:cs + 512],
                                             in0=X[:, mc, cs:cs + 512],
                                             in1=bf2[:, mc:mc + 1].broadcast_to([128, 512]))
                xsq = sqp.tile([128, 4, 512], BF16, tag="xsq2")
                nc.gpsimd.tensor_mul(xsq, xc, xc)
                ps_st = stats_mms(xc, xsq, 512)
                ln_region(cs, 512, ps_st, g2, b2, lastl)

            # ---------------- layer schedule -----------------------------
            if not skip_attn:
                # ffn chunk c runs one pair AFTER its LN1 coverage completes,
                # so the LN1 row/bcast/apply chain and the xb2 casts hide
                # under the next pair's attention burst.
                nxt = 0
                cast_pair(0)
                for p in range(PAIRS):
                    if p + 1 < PAIRS:
                        cast_pair(p + 1)
                    if not skip_ffn:
                        # emit in batches of >=2 so the PE gets long warm
                        # N=512 streams (HAM stays at K=8/8)
                        avail = min(NCH, (p * 288) // 512) - nxt
                        if avail >= 3 or (p == PAIRS - 1 and avail > 0):
                            for _ in range(avail):
                                ffn_chunk(nxt)
                                nxt += 1
                    att_pair(p)
                if not skip_ffn:
                    while nxt < NCH:
                        ffn_chunk(nxt)
                        nxt += 1
            elif not skip_ffn:
                for cc in range(NCH):
                    ffn_chunk(cc)
            if skip_ffn and l == NL - 1:
                for cc in range(NCH):
                    cs = cc * 512
                    nc.sync.dma_start(out=d["out"][:, :, cs:cs + 512],
                                      in_=X[:, :, cs:cs + 512])

    return d


# ---------------------------------------------------------------------------
# Host-side packing + golden model
# ---------------------------------------------------------------------------

def rel_idx():
    coords = np.stack(np.meshgrid(np.arange(WS), np.arange(WS), indexing="ij"))
    flat = coords.reshape(2, -1)
    rel = (flat[:, :, None] - flat[:, None, :]).transpose(1, 2, 0).copy()
    rel[..., 0] += WS - 1
    rel[..., 1] += WS - 1
    rel[..., 0] *= 2 * WS - 1
    return rel.sum(-1)  # [N, N] int


def pack_weights(w, NL):
    """w: dict of reference arrays -> dict of const arrays (np)."""
    bf = ml_dtypes.bfloat16
    scale = HD ** -0.5
    ridx = rel_idx()
    out = {}

    def lhsT_pack(W, kchunks):  # [Cin, Cout] -> [128, kchunks, Cout]
        return np.ascontiguousarray(
            W.reshape(kchunks, 128, W.shape[1]).transpose(1, 0, 2)
        )

    wq = np.stack([lhsT_pack(w["Wq"][l] * scale, 4) for l in range(NL)])
    wk = np.stack([lhsT_pack(w["Wk"][l], 4) for l in range(NL)])
    wv = np.stack([lhsT_pack(w["Wv"][l], 4) for l in range(NL)])
    wo = np.stack([lhsT_pack(w["Wo"][l], 4) for l in range(NL)])
    w1 = np.stack([lhsT_pack(w["W1"][l], 4) for l in range(NL)])
    w2 = np.stack([lhsT_pack(w["W2"][l], 16) for l in range(NL)])
    for nm, arr in (("wq", wq), ("wk", wk), ("wv", wv), ("wo", wo),
                    ("w1", w1), ("w2", w2)):
        out[nm] = arr.astype(bf)

    expb = np.zeros((NL, 128, NH, 288), np.float32)
    for l in range(NL):
        bias = w["rpb"][l][ridx]            # [N(i), N(j), NH]
        ebT = np.exp(bias.transpose(2, 1, 0))  # [NH, j, i]
        expb[l, 0:128, :, 0:144] = ebT[:, 0:128, :].transpose(1, 0, 2)
        expb[l, 0:16, :, 144:288] = ebT[:, 128:144, :].transpose(1, 0, 2)
        expb[l, 32:48, :, 144:288] = ebT[:, 128:144, :].transpose(1, 0, 2)
    out["expb"] = expb.astype(bf)

    def percol(b):  # [NL, C] -> [NL, 128, 4]
        return np.ascontiguousarray(
            b.reshape(NL, 4, 128).transpose(0, 2, 1)).astype(np.float32)

    out["bq"] = percol(w["bq"] * scale)
    out["bk"] = percol(w["bk"])
    out["bo_c"] = percol(w["bo"])
    out["bf2_c"] = percol(w["bf2"])
    out["c_ones1"] = np.ones((1, 128), bf)
    e2 = np.zeros((64, 128), np.float32)
    e2[0, 0:64] = 1.0
    e2[32, 64:128] = 1.0
    out["c_e2"] = e2.astype(bf)
    out["g1"] = percol(w["g1"])
    out["b1"] = percol(w["b1"])
    out["g2"] = percol(w["g2"])
    out["b2"] = percol(w["b2"])
    out["bf1"] = np.ascontiguousarray(
        w["bf1"].reshape(NL, 16, 128).transpose(0, 2, 1)).astype(np.float32)
    out["bvb"] = np.broadcast_to(
        w["bv"].astype(bf)[:, None, :], (NL, 128, 512)).copy()
    out["c_ones"] = np.full((128, 1), 1.0 / 512.0, bf)
    return out


def golden_tm(x_tm, w, NL):
    """fp32 numpy reference on window-major token-major x [T, 512]."""
    T = x_tm.shape[0]
    NW = T // N
    ridx = rel_idx()
    x = x_tm.astype(np.float32)

    def ln(v, g, b):
        m = v.mean(-1, keepdims=True)
        s = v.var(-1, keepdims=True)
        return (v - m) / np.sqrt(s + EPS) * g + b

    for l in range(NL):
        xw = x.reshape(NW, N, C)
        q = (xw @ w["Wq"][l] + w["bq"][l]).reshape(NW, N, NH, HD).transpose(0, 2, 1, 3)
        k = (xw @ w["Wk"][l] + w["bk"][l]).reshape(NW, N, NH, HD).transpose(0, 2, 1, 3)
        v = (xw @ w["Wv"][l] + w["bv"][l]).reshape(NW, N, NH, HD).transpose(0, 2, 1, 3)
        bias = w["rpb"][l][ridx].transpose(2, 0, 1)
        attn = np.einsum("whid,whjd->whij", q, k) * (HD ** -0.5) + bias
        attn = attn - attn.max(-1, keepdims=True)
        p = np.exp(attn)
        p = p / p.sum(-1, keepdims=True)
        o = np.einsum("whij,whjd->whid", p, v).transpose(0, 2, 1, 3).reshape(NW, N, C)
        o = o @ w["Wo"][l] + w["bo"][l]
        x = ln(o.reshape(T, C) + x, w["g1"][l], w["b1"][l])
        h = np.maximum(x @ w["W1"][l] + w["bf1"][l], 0.0) @ w["W2"][l] + w["bf2"][l]
        x = ln(h + x, w["g2"][l], w["b2"][l])
    return x


# ---------------------------------------------------------------------------
# kernel() entry point: full inputs -> full output, 8-way batch data parallel
# ---------------------------------------------------------------------------

NCORES = 8
B_FULL = 64
H_RES = W_RES = 24
L_TOK = H_RES * W_RES
NW_FULL = (B_FULL // NCORES) * (H_RES // WS) * (W_RES // WS)   # 32 windows/core
NL_FULL = 3

_COMPILED = {}


def _pack_x_all(x):
    """[64, 576, 512] f32 -> [8, 128, 4, T] bf16 channel-major window-major."""
    b = x.reshape(NCORES, B_FULL // NCORES, 2, WS, 2, WS, 4, 128)
    v = b.transpose(0, 7, 6, 1, 2, 4, 3, 5)   # [core,128,4, b,hw,ww,hs,ws]
    return np.ascontiguousarray(v.reshape(NCORES, 128, 4, -1)
                                ).astype(ml_dtypes.bfloat16)


def _unpack_out_all(res_list):
    """list of [128, 4, T] -> [64, 576, 512] f32."""
    y = np.stack([r.astype(np.float32) for r in res_list])     # [8,128,4,T]
    bpc = B_FULL // NCORES
    v = y.reshape(NCORES, 128, 4, bpc, 2, 2, WS, WS)
    v = v.transpose(0, 3, 4, 6, 5, 7, 2, 1)   # [core,b,hw,hs,ww,ws,4,128]
    return np.ascontiguousarray(v.reshape(B_FULL, L_TOK, C))


def kernel(x, Wq, bq, Wk, bk, Wv, bv, Wo, bo, rpb,
           g1, b1, W1, bf1, W2, bf2, g2, b2):
    import hashlib
    from concourse.bass_utils import run_bass_kernel_spmd

    w = {"Wq": np.asarray(Wq, np.float32), "bq": np.asarray(bq, np.float32),
         "Wk": np.asarray(Wk, np.float32), "bk": np.asarray(bk, np.float32),
         "Wv": np.asarray(Wv, np.float32), "bv": np.asarray(bv, np.float32),
         "Wo": np.asarray(Wo, np.float32), "bo": np.asarray(bo, np.float32),
         "rpb": np.asarray(rpb, np.float32),
         "g1": np.asarray(g1, np.float32), "b1": np.asarray(b1, np.float32),
         "W1": np.asarray(W1, np.float32), "bf1": np.asarray(bf1, np.float32),
         "W2": np.asarray(W2, np.float32), "bf2": np.asarray(bf2, np.float32),
         "g2": np.asarray(g2, np.float32), "b2": np.asarray(b2, np.float32)}
    hsh = hashlib.blake2b(
        b"".join(np.ascontiguousarray(v).tobytes() for v in w.values()),
        digest_size=16).hexdigest()
    if _COMPILED.get("hash") != hsh:
        packed = pack_weights(w, NL_FULL)
        trivial_gb = bool(np.all(w["g1"] == 1) and np.all(w["b1"] == 0)
                          and np.all(w["g2"] == 1) and np.all(w["b2"] == 0))
        trivial_bias = bool(all(np.all(w[k] == 0)
                                for k in ("bq", "bk", "bv", "bo", "bf1", "bf2")))
        nc = bacc.Bacc("TRN2", target_bir_lowering=False, debug=False)
        build(nc, NW_FULL, NL_FULL, packed,
              trivial_gb=trivial_gb, trivial_bias=trivial_bias)
        nc.compile()
        _COMPILED.update(hash=hsh, nc=nc)

    xp = _pack_x_all(np.asarray(x, np.float32))
    in_maps = [{"x": xp[i]} for i in range(NCORES)]
    res = run_bass_kernel_spmd(_COMPILED["nc"], in_maps, list(range(NCORES)))
    return _unpack_out_all([res.results[i]["out"] for i in range(NCORES)])



# revision 28
# speedup vs baseline: 1.1003x; 1.0164x over previous
"""Swin-style window-attention encoder as a Bass/Tile kernel for TRN2 — v3.

Key design vs v1:
- Residual master X lives in SBUF as FP32 [128, 4, T+16] (channel-major) —
  the residual stream never rounds to bf16 (bf16 master measured 2.2e-2 max
  rel err, over the 2e-2 budget; f32 master 7.8e-3). Matmul inputs are
  bf16 copies cast on the (otherwise idle) GPSIMD engine; per-token mean
  stats contract the f32 master directly (f32 matmul, tiny N).
- Weights are baked into the NEFF as inline consts — per-launch IO is just
  x (bf16 in) and out (bf16).
- Per-layer phase batching: [attention (Exp table)] -> [LN1 rows (Sqrt) +
  apply] -> [FFN (Relu, no table load)] -> [LN2 rows + apply]. 2 activation
  table loads per layer instead of ~64.
- Attention softmax denominators: collected per head into smat rows (act
  Copy), broadcast to 128 partitions via one e2 matmul, ONE fat [128,144]
  DVE reciprocal per head-pair (no 1-lane recips).
- LN row math on [128,36] shuffled layout (SBUF->SBUF strided DMA), not
  1-lane [1,T] ops.
- O-proj / QK-proj / stats at window-pair (288 tokens) granularity; FFN at
  512-token chunks.
"""
from contextlib import ExitStack

import numpy as np
import ml_dtypes

import concourse.bass as bass
import concourse.bacc as bacc
import concourse.tile as tile
import concourse.mybir as mybir

F32 = mybir.dt.float32
F32R = mybir.dt.float32r
BF16 = mybir.dt.bfloat16
AF = mybir.ActivationFunctionType


def _patch_act_tables():
    """Make the act-table-load pass resolve Exp AND Ln to the combined
    `natural_log_exp_and_others` set (it otherwise greedily alternates
    between `exp_and_others` and `natural_log`, reloading tables at every
    attention<->LN boundary, ~150 loads/kernel). We hide exp/ln from every
    other set in the table list the pass consults; set IDs (list order)
    are unchanged, so walrus still emits the right act.json entries."""
    import concourse.hw_specs as hw_specs

    if getattr(bacc, "_ant_act_tables_patched", False):
        return
    orig = hw_specs.get_activation_tables

    def patched(arch):
        tabs = orig(arch)
        exp, ln = AF.Exp, AF.Ln
        if "natural_log_exp_and_others" in tabs:
            for name, fns in tabs.items():
                if name != "natural_log_exp_and_others":
                    fns.discard(exp)
                    fns.discard(ln)
        return tabs

    bacc.get_activation_tables = patched
    bacc._ant_act_tables_patched = True


_patch_act_tables()

WS = 12
N = WS * WS          # 144 tokens per window
C = 512
NH = 8
HD = 64
FF = 2048
EPS = 1e-5


def _insdim_ap(row_ap, stride, num, at=1):
    """Insert a dim of (stride, num) at position `at` of the AP (default:
    right after the partition dim). stride=0 -> broadcast; else gather."""
    dims = [list(d) for d in row_ap.ap]
    return bass.AP(
        tensor=row_ap.tensor,
        offset=row_ap.offset,
        ap=dims[:at] + [[stride, num]] + dims[at:],
    )


def _bcast_ap(row_ap, parts):
    return _insdim_ap(row_ap, 0, parts)


def build(nc: bass.Bass, NW: int, NL: int, w: dict,
          skip_attn=False, skip_ffn=False, skip_heads=False,
          pb=(4, 4), winb=2, epb=7, sqb=1, hbb=1, scb=3, la=6, xbb=2,
          trivial_gb=False, trivial_bias=False):
    """w: packed numpy weight dict (see pack_weights)."""
    T = NW * N
    PAIRS = NW // 2
    NCH = T // 512
    assert T % 512 == 0

    d = {}
    d["x"] = nc.dram_tensor("x", [128, 4, T], BF16, kind="ExternalInput").ap()
    d["out"] = nc.dram_tensor("out", [128, 4, T], BF16, kind="ExternalOutput").ap()
    cst = {nm: nc.inline_tensor(arr, name=nm).ap() for nm, arr in w.items()}

    with tile.TileContext(nc) as tc, ExitStack() as ctx:
        P = lambda name, bufs, **kw: ctx.enter_context(
            tc.tile_pool(name=name, bufs=bufs, **kw)
        )
        xp = P("xmaster", 1)
        cons = P("consts", 1)
        wpA = P("wtsA", 1)     # attention-phase weights
        wpF = P("wtsF", 1)     # ffn-phase weights
        winp = P("win", winb)  # per-pair working tiles
        ep = P("eptiles", epb)  # P tiles
        etp = P("ettiles", 4)   # exp tiles (short-lived)
        sqp = P("sqtiles", sqb)  # squared-x tiles for stats
        scp = P("sctiles", scb)  # recip rows [128,144]
        rowp = P("rows", 2)    # LN stat rows (short-lived, per region)
        hp = P("hbuf", hbb)
        xbp = P("xbcast", 2)
        xb2p = P("xb2cast", 2)
        psmm = P("psmm", pb[0], space="PSUM")
        psaux = P("psaux", 3, space="PSUM")

        # ---- persistent tiles ----
        X = xp.tile([128, 4, T + 16], F32, tag="X")
        for tq in range(NCH):
            xin = sqp.tile([128, 4, 512], BF16, tag="xsq2")
            nc.sync.dma_start(out=xin,
                              in_=d["x"][:, :, tq * 512:(tq + 1) * 512])
            nc.vector.tensor_copy(out=X[:, :, tq * 512:(tq + 1) * 512], in_=xin)
        ones = cons.tile([128, 1], BF16, tag="ones")       # value 1/512
        nc.sync.dma_start(out=ones, in_=cst["c_ones"])
        ones1 = cons.tile([1, 128], BF16, tag="ones1")     # bcast lhsT (1.0)
        nc.sync.dma_start(out=ones1, in_=cst["c_ones1"])
        e2 = cons.tile([64, 128], BF16, tag="e2")
        nc.sync.dma_start(out=e2, in_=cst["c_e2"])
        eps128 = cons.tile([128, 1], F32, tag="eps128")
        nc.vector.memset(eps128, EPS)
        onesf = cons.tile([128, 1], F32, tag="onesf")
        nc.vector.memset(onesf, 1.0 / 512.0)

        for l in range(NL):
            # layer weights (attention set + rows)
            wq = wpA.tile([128, 4, 512], BF16, tag="wq")
            wk = wpA.tile([128, 4, 512], BF16, tag="wk")
            wv = wpA.tile([128, 4, 512], BF16, tag="wv")
            wo = wpA.tile([128, 4, 512], BF16, tag="wo")
            eb = wpA.tile([128, NH, 288], BF16, tag="expb")
            bq = wpA.tile([128, 4], F32, tag="bq")
            bk = wpA.tile([128, 4], F32, tag="bk")
            bo = wpA.tile([128, 4], F32, tag="bo")
            bv = wpA.tile([128, 512], BF16, tag="bvb")
            g1 = wpA.tile([128, 4], F32, tag="g1")
            b1 = wpA.tile([128, 4], F32, tag="b1")
            g2 = wpA.tile([128, 4], F32, tag="g2")
            b2 = wpA.tile([128, 4], F32, tag="b2")
            for nm, t in (("wq", wq), ("wk", wk), ("wv", wv), ("wo", wo),
                          ("expb", eb), ("bq", bq), ("bk", bk), ("bo_c", bo),
                          ("bvb", bv), ("g1", g1), ("b1", b1), ("g2", g2),
                          ("b2", b2)):
                nc.sync.dma_start(out=t, in_=cst[nm][l])
            # ffn weights: issued now, consumed after LN1 (overlaps attention)
            w1 = wpF.tile([128, 4, FF], BF16, tag="w1")
            w2 = wpF.tile([128, 16, 512], BF16, tag="w2")
            bf1 = wpF.tile([128, 16], F32, tag="bf1")
            bf2 = wpF.tile([128, 4], F32, tag="bf2")
            for nm, t in (("w1", w1), ("w2", w2), ("bf1", bf1), ("bf2_c", bf2)):
                nc.sync.dma_start(out=t, in_=cst[nm][l])

            # ---------------- per-region LN (stats already in ps_st) --------
            def ln_region(cs, wdt, ps_st, g, b, last=False):
                # rows: mean (bf16), mean^2, var, ln(var+eps), rstd=exp(-.5ln)
                srow = rowp.tile([1, 512], BF16, tag="srow", name="srow")[:, :wdt]
                nc.scalar.activation(out=srow, in_=ps_st[0:1, :wdt],
                                     func=AF.Copy)
                m2 = rowp.tile([1, 512], F32, tag="m2row", name="m2")[:, :wdt]
                nc.vector.tensor_mul(m2, srow, srow)
                nc.vector.tensor_sub(m2, ps_st[32:33, :wdt], m2)   # var, in place
                nc.scalar.activation(out=m2, in_=m2, func=AF.Ln,
                                     bias=eps128[0:1, :])          # ln(var+eps)
                rrow = rowp.tile([1, 512], BF16, tag="rrow", name="rrow")[:, :wdt]
                nc.scalar.activation(out=rrow, in_=m2, func=AF.Exp, scale=-0.5)
                # broadcast rows to 128 partitions on the PE; stage to SBUF
                # bf16 immediately so the PSUM bank frees fast (tag "bc"
                # bufs=1 -> bm/br serialize through one bank)
                ps_bm = psaux.tile([128, 512], F32, tag="bc", name="ps_bm",
                                   bufs=1)[:, :wdt]
                nc.tensor.matmul(ps_bm, lhsT=ones1, rhs=srow,
                                 start=True, stop=True)
                bm = rowp.tile([128, 512], BF16, tag="bmsb", name="bm")[:, :wdt]
                nc.vector.tensor_copy(out=bm, in_=ps_bm)
                ps_br = psaux.tile([128, 512], F32, tag="bc", name="ps_br",
                                   bufs=1)[:, :wdt]
                nc.tensor.matmul(ps_br, lhsT=ones1, rhs=rrow,
                                 start=True, stop=True)
                br = rowp.tile([128, 512], BF16, tag="brsb", name="br")[:, :wdt]
                nc.vector.tensor_copy(out=br, in_=ps_br)
                xc = X[:, :, cs:cs + wdt]
                nc.vector.tensor_sub(xc, xc, _insdim_ap(bm, 0, 4))
                nc.vector.tensor_mul(xc, xc, _insdim_ap(br, 0, 4))
                ob = None
                if not trivial_gb:
                    if last:
                        ob = sqp.tile([128, 4, 512], BF16, tag="xsq2",
                                      name="ob")
                    for mc in range(4):
                        dst = ob[:, mc, 0:wdt] if last else X[:, mc, cs:cs + wdt]
                        nc.scalar.activation(out=dst, in_=X[:, mc, cs:cs + wdt],
                                             func=AF.Identity,
                                             bias=b[:, mc:mc + 1],
                                             scale=g[:, mc:mc + 1])
                elif last:
                    ob = sqp.tile([128, 4, 512], BF16, tag="xsq2", name="ob")
                    nc.vector.tensor_copy(out=ob[:, :, 0:wdt], in_=xc)
                if last:
                    nc.sync.dma_start(out=d["out"][:, :, cs:cs + wdt],
                                      in_=ob[:, :, 0:wdt])

            def stats_mms(xs_sl, xsq_sl, wdt):
                # mean at partition 0 (f32 MM), mean-square at partition 32
                # (bf16 MM). Lives in the fast-draining "mm" ring.
                ps_st = psmm.tile([33, 512], F32, tag="mm", name="ps_st")
                for kc in range(4):
                    nc.tensor.matmul(ps_st[0:1, :wdt], lhsT=onesf,
                                     rhs=xs_sl[:, kc, :],
                                     start=(kc == 0), stop=(kc == 3))
                for kc in range(4):
                    nc.tensor.matmul(ps_st[32:33, :wdt], lhsT=ones,
                                     rhs=xsq_sl[:, kc, :],
                                     start=(kc == 0), stop=(kc == 3))
                return ps_st

            # ---------------- per-pair attention emitter --------------------
            cast_cache = {}

            def cast_pair(p):
                cs0 = p * 288
                xb = xbp.tile([128, 4, 304], BF16, tag="xb")
                nc.gpsimd.tensor_copy(out=xb, in_=X[:, :, cs0:cs0 + 304])
                # tail tokens of both windows packed at cols {0:16, 32:48}
                xt = xbp.tile([128, 4, 64], BF16, tag="xt")
                nc.gpsimd.tensor_copy(
                    out=_insdim_ap(xt[:, :, 0:16], 32, 2, at=2),
                    in_=_insdim_ap(X[:, :, cs0 + 128:cs0 + 144], 144, 2, at=2))
                cast_cache[p] = (xb, xt)

            def att_pair(p):
                cs0 = p * 288
                xs = X[:, :, cs0:cs0 + 288]
                xb, xt = cast_cache.pop(p)
                qw = winp.tile([128, 4, 288], BF16, tag="qw")
                kw = winp.tile([128, 4, 288], BF16, tag="kw")
                for mc in range(4):
                    pq = psmm.tile([128, 288], F32, tag="mm")
                    for kc in range(4):
                        nc.tensor.matmul(pq, lhsT=wq[:, kc, mc * 128:(mc + 1) * 128],
                                         rhs=xb[:, kc, 0:288], start=(kc == 0), stop=(kc == 3))
                    if trivial_bias:
                        nc.vector.tensor_copy(out=qw[:, mc, :], in_=pq)
                    else:
                        nc.scalar.activation(out=qw[:, mc, :], in_=pq, func=AF.Identity,
                                             bias=bq[:, mc:mc + 1])
                    pk = psmm.tile([128, 288], F32, tag="mm")
                    for kc in range(4):
                        nc.tensor.matmul(pk, lhsT=wk[:, kc, mc * 128:(mc + 1) * 128],
                                         rhs=xb[:, kc, 0:288], start=(kc == 0), stop=(kc == 3))
                    nc.scalar.activation(out=kw[:, mc, :], in_=pk, func=AF.Identity,
                                         bias=bk[:, mc:mc + 1])

                vws = []
                for wi in (0, 1):
                    vw1 = winp.tile([128, NH, 65], BF16, tag=f"vw1_{wi}")
                    off = wi * 144
                    pv1 = psmm.tile([128, 512], F32, tag="mm")
                    for kc in range(4):
                        nc.tensor.matmul(pv1, lhsT=xb[:, kc, off:off + 128],
                                         rhs=wv[:, kc, :], start=(kc == 0), stop=(kc == 3))
                    nc.vector.tensor_add(out=vw1[:, :, 0:64],
                                         in0=pv1.rearrange("p (h e) -> p h e", h=NH),
                                         in1=bv.rearrange("p (h e) -> p h e", h=NH))
                    nc.vector.memset(vw1[:, :, 64:65], 1.0)
                    vws.append(vw1)
                # merged tail-V for both windows: lhsT cols {128:160, 272:304}
                # -> out partitions A-tail 0:16, (garbage 16:32), B-tail 32:48
                vw2p = winp.tile([64, NH, 65], BF16, tag="vw2p")
                pv2 = psmm.tile([64, 512], F32, tag="mm")
                for kc in range(4):
                    nc.tensor.matmul(pv2, lhsT=xt[:, kc, :], rhs=wv[:, kc, :],
                                     start=(kc == 0), stop=(kc == 3))
                nc.vector.tensor_add(out=vw2p[:, :, 0:64],
                                     in0=pv2.rearrange("p (h e) -> p h e", h=NH),
                                     in1=bv[0:64].rearrange("p (h e) -> p h e", h=NH))
                nc.vector.memset(vw2p[:, :, 64:65], 1.0)

                ocm = winp.tile([128, 4, 288], BF16, tag="ocm")
                if skip_heads:
                    nc.vector.tensor_copy(out=ocm, in_=xs)

                # software-pipelined head loop: stage A (S-mm, exp, P-mul) runs
                # `LOOKAHEAD` heads in front of stage B (PV, den) and stage C
                # (per head-pair: e2 bcast-mm, recip, ocm scale), so the PE has
                # independent matmuls queued while act/DVE chew on earlier heads.
                heads = [(wi, h) for wi in (0, 1) for h in range(NH)]
                pts = {}
                psos = {}

                def stage_a(wi, h):
                    off = wi * 144
                    tb = 32 * wi
                    ro, tl = (h % 2) * 64, h // 2
                    ps_s = psmm.tile([128, 288], F32, tag="mm")
                    nc.tensor.matmul(ps_s[:, 0:144],
                                     lhsT=kw[ro:ro + 64, tl, off:off + 128],
                                     rhs=qw[ro:ro + 64, tl, off:off + 144],
                                     start=True, stop=True)
                    nc.tensor.matmul(ps_s[tb:tb + 16, 144:288],
                                     lhsT=kw[ro:ro + 64, tl, off + 128:off + 144],
                                     rhs=qw[ro:ro + 64, tl, off:off + 144],
                                     start=True, stop=True)
                    et = etp.tile([128, 288], BF16, tag="e")
                    nc.scalar.activation(out=et, in_=ps_s, func=AF.Exp)
                    pt = ep.tile([128, 288], BF16, tag="p")
                    nc.vector.tensor_mul(pt, et, eb[:, h, :])
                    pts[(wi, h)] = pt

                def stage_b(wi, h):
                    pt = pts.pop((wi, h))
                    vw1 = vws[wi]
                    hp, hi = h // 2, h % 2
                    if hi == 0:
                        psos[(wi, hp)] = psaux.tile([65, 2, 144], F32,
                                                    tag="aux", name="ps_o2")
                    ps_o = psos[(wi, hp)]
                    nc.tensor.matmul(ps_o[:, hi, :], lhsT=vw1[:, h, :],
                                     rhs=pt[:, 0:144], start=True, stop=False)
                    tb = 32 * wi
                    nc.tensor.matmul(ps_o[:, hi, :],
                                     lhsT=vw2p[tb:tb + 16, h, :],
                                     rhs=pt[tb:tb + 16, 144:288],
                                     start=False, stop=True)
                    if hi == 1:
                        stage_c(wi, hp)

                def stage_c(wi, hpair):
                    off = wi * 144
                    ps_o = psos[(wi, hpair)]
                    # both heads' softmax denominators in one row copy
                    dp = rowp.tile([1, 288], BF16, tag="dpair", name="dp",
                                   bufs=4)
                    nc.scalar.activation(out=dp, in_=ps_o[64:65, 0:2, 0:144],
                                         func=AF.Copy)
                    ps_sc = psmm.tile([128, 144], F32, tag="mm")
                    nc.tensor.matmul(ps_sc[0:64, :], lhsT=ones1[0:1, 0:64],
                                     rhs=dp[0:1, 0:144], start=True, stop=True)
                    nc.tensor.matmul(ps_sc[64:128, :], lhsT=ones1[0:1, 0:64],
                                     rhs=dp[0:1, 144:288], start=True, stop=True)
                    sc = scp.tile([128, 144], F32, tag="scsb")
                    nc.vector.reciprocal_approx_fast(out=sc, in_=ps_sc)
                    p01 = psos.pop((wi, hpair))
                    nc.vector.tensor_mul(ocm[0:64, hpair, off:off + 144],
                                         p01[0:64, 0, :], sc[0:64, :])
                    nc.vector.tensor_mul(ocm[64:128, hpair, off:off + 144],
                                         p01[0:64, 1, :], sc[64:128, :])

                LOOKAHEAD = la
                for i, (wi, h) in enumerate(heads if not skip_heads else []):
                    stage_a(wi, h)
                    if i >= LOOKAHEAD:
                        stage_b(*heads[i - LOOKAHEAD])
                for j in (range(max(0, len(heads) - LOOKAHEAD), len(heads))
                          if not skip_heads else []):
                    stage_b(*heads[j])

                # O projection + residual -> X (pre-LN1), stats, LN1
                for mc in range(4):
                    po = psmm.tile([128, 288], F32, tag="mm")
                    for kc in range(4):
                        nc.tensor.matmul(po, lhsT=wo[:, kc, mc * 128:(mc + 1) * 128],
                                         rhs=ocm[:, kc, :], start=(kc == 0), stop=(kc == 3))
                    nc.vector.tensor_add(out=X[:, mc, cs0:cs0 + 288], in0=po,
                                         in1=X[:, mc, cs0:cs0 + 288])
                    if not trivial_bias:
                        nc.vector.tensor_add(out=X[:, mc, cs0:cs0 + 288],
                                             in0=X[:, mc, cs0:cs0 + 288],
                                             in1=bo[:, mc:mc + 1].broadcast_to([128, 288]))
                xsq = sqp.tile([128, 4, 288], BF16, tag="xsq")
                nc.gpsimd.tensor_mul(xsq, xs, xs)
                ps_st = stats_mms(xs, xsq, 288)
                ln_region(cs0, 288, ps_st, g1, b1)

            # ---------------- FFN chunk emitter (incl. LN2) ----------------
            lastl = (l == NL - 1)

            def ffn_chunk(cc):
                cs = cc * 512
                xc = X[:, :, cs:cs + 512]
                xb2 = xb2p.tile([128, 4, 512], BF16, tag="xb2")
                for kc in range(4):
                    nc.gpsimd.tensor_copy(out=xb2[:, kc, :], in_=xc[:, kc, :])
                hb = hp.tile([128, 16, 512], BF16, tag="hb")
                for fc in range(16):
                    ph = psmm.tile([128, 512], F32, tag="mm")
                    for kc in range(4):
                        nc.tensor.matmul(ph, lhsT=w1[:, kc, fc * 128:(fc + 1) * 128],
                                         rhs=xb2[:, kc, :], start=(kc == 0), stop=(kc == 3))
                    if fc % 2 == 0:
                        nc.scalar.activation(out=hb[:, fc, :], in_=ph, func=AF.Relu,
                                             bias=bf1[:, fc:fc + 1])
                    else:
                        nc.vector.tensor_scalar(
                            out=hb[:, fc, :], in0=ph, scalar1=bf1[:, fc:fc + 1],
                            scalar2=0.0, op0=mybir.AluOpType.add,
                            op1=mybir.AluOpType.max)
                for mc in range(4):
                    pf = psmm.tile([128, 512], F32, tag="mm")
                    for fc in range(16):
                        nc.tensor.matmul(pf, lhsT=w2[:, fc, mc * 128:(mc + 1) * 128],
                                         rhs=hb[:, fc, :], start=(fc == 0), stop=(fc == 15))
                    nc.vector.tensor_add(out=X[:, mc, cs:cs + 512], in0=pf,
                                         in1=X[:, mc, cs:cs + 512])
                    if not trivial_bias:
                        nc.vector.tensor_add(out=X[:, mc, cs:cs + 512],
                                             in0=X[:, mc, cs:cs + 512],
                                             in1=bf2[:, mc:mc + 1].broadcast_to([128, 512]))
                xsq = sqp.tile([128, 4, 512], BF16, tag="xsq2")
                nc.gpsimd.tensor_mul(xsq, xc, xc)
                ps_st = stats_mms(xc, xsq, 512)
                ln_region(cs, 512, ps_st, g2, b2, lastl)

            # ---------------- layer schedule -----------------------------
            if not skip_attn:
                # ffn chunk c runs one pair AFTER its LN1 coverage completes,
                # so the LN1 row/bcast/apply chain and the xb2 casts hide
                # under the next pair's attention burst.
                nxt = 0
                cast_pair(0)
                for p in range(PAIRS):
                    if p + 1 < PAIRS:
                        cast_pair(p + 1)
                    if not skip_ffn:
                        # emit in batches of >=2 so the PE gets long warm
                        # N=512 streams (HAM stays at K=8/8)
                        avail = min(NCH, (p * 288) // 512) - nxt
                        if avail >= 3 or (p == PAIRS - 1 and avail > 0):
                            for _ in range(avail):
                                ffn_chunk(nxt)
                                nxt += 1
                    att_pair(p)
                if not skip_ffn:
                    while nxt < NCH:
                        ffn_chunk(nxt)
                        nxt += 1
            elif not skip_ffn:
                for cc in range(NCH):
                    ffn_chunk(cc)
            if skip_ffn and l == NL - 1:
                for cc in range(NCH):
                    cs = cc * 512
                    nc.sync.dma_start(out=d["out"][:, :, cs:cs + 512],
                                      in_=X[:, :, cs:cs + 512])

    return d


# ---------------------------------------------------------------------------
# Host-side packing + golden model
# ---------------------------------------------------------------------------

def rel_idx():
    coords = np.stack(np.meshgrid(np.arange(WS), np.arange(WS), indexing="ij"))
    flat = coords.reshape(2, -1)
    rel = (flat[:, :, None] - flat[:, None, :]).transpose(1, 2, 0).copy()
    rel[..., 0] += WS - 1
    rel[..., 1] += WS - 1
    rel[..., 0] *= 2 * WS - 1
    return rel.sum(-1)  # [N, N] int


def pack_weights(w, NL):
    """w: dict of reference arrays -> dict of const arrays (np)."""
    bf = ml_dtypes.bfloat16
    scale = HD ** -0.5
    ridx = rel_idx()
    out = {}

    def lhsT_pack(W, kchunks):  # [Cin, Cout] -> [128, kchunks, Cout]
        return np.ascontiguousarray(
            W.reshape(kchunks, 128, W.shape[1]).transpose(1, 0, 2)
        )

    wq = np.stack([lhsT_pack(w["Wq"][l] * scale, 4) for l in range(NL)])
    wk = np.stack([lhsT_pack(w["Wk"][l], 4) for l in range(NL)])
    wv = np.stack([lhsT_pack(w["Wv"][l], 4) for l in range(NL)])
    wo = np.stack([lhsT_pack(w["Wo"][l], 4) for l in range(NL)])
    w1 = np.stack([lhsT_pack(w["W1"][l], 4) for l in range(NL)])
    w2 = np.stack([lhsT_pack(w["W2"][l], 16) for l in range(NL)])
    for nm, arr in (("wq", wq), ("wk", wk), ("wv", wv), ("wo", wo),
                    ("w1", w1), ("w2", w2)):
        out[nm] = arr.astype(bf)

    expb = np.zeros((NL, 128, NH, 288), np.float32)
    for l in range(NL):
        bias = w["rpb"][l][ridx]            # [N(i), N(j), NH]
        ebT = np.exp(bias.transpose(2, 1, 0))  # [NH, j, i]
        expb[l, 0:128, :, 0:144] = ebT[:, 0:128, :].transpose(1, 0, 2)
        expb[l, 0:16, :, 144:288] = ebT[:, 128:144, :].transpose(1, 0, 2)
        expb[l, 32:48, :, 144:288] = ebT[:, 128:144, :].transpose(1, 0, 2)
    out["expb"] = expb.astype(bf)

    def percol(b):  # [NL, C] -> [NL, 128, 4]
        return np.ascontiguousarray(
            b.reshape(NL, 4, 128).transpose(0, 2, 1)).astype(np.float32)

    out["bq"] = percol(w["bq"] * scale)
    out["bk"] = percol(w["bk"])
    out["bo_c"] = percol(w["bo"])
    out["bf2_c"] = percol(w["bf2"])
    out["c_ones1"] = np.ones((1, 128), bf)
    e2 = np.zeros((64, 128), np.float32)
    e2[0, 0:64] = 1.0
    e2[32, 64:128] = 1.0
    out["c_e2"] = e2.astype(bf)
    out["g1"] = percol(w["g1"])
    out["b1"] = percol(w["b1"])
    out["g2"] = percol(w["g2"])
    out["b2"] = percol(w["b2"])
    out["bf1"] = np.ascontiguousarray(
        w["bf1"].reshape(NL, 16, 128).transpose(0, 2, 1)).astype(np.float32)
    out["bvb"] = np.broadcast_to(
        w["bv"].astype(bf)[:, None, :], (NL, 128, 512)).copy()
    out["c_ones"] = np.full((128, 1), 1.0 / 512.0, bf)
    return out


def golden_tm(x_tm, w, NL):
    """fp32 numpy reference on window-major token-major x [T, 512]."""
    T = x_tm.shape[0]
    NW = T // N
    ridx = rel_idx()
    x = x_tm.astype(np.float32)

    def ln(v, g, b):
        m = v.mean(-1, keepdims=True)
        s = v.var(-1, keepdims=True)
        return (v - m) / np.sqrt(s + EPS) * g + b

    for l in range(NL):
        xw = x.reshape(NW, N, C)
        q = (xw @ w["Wq"][l] + w["bq"][l]).reshape(NW, N, NH, HD).transpose(0, 2, 1, 3)
        k = (xw @ w["Wk"][l] + w["bk"][l]).reshape(NW, N, NH, HD).transpose(0, 2, 1, 3)
        v = (xw @ w["Wv"][l] + w["bv"][l]).reshape(NW, N, NH, HD).transpose(0, 2, 1, 3)
        bias = w["rpb"][l][ridx].transpose(2, 0, 1)
        attn = np.einsum("whid,whjd->whij", q, k) * (HD ** -0.5) + bias
        attn = attn - attn.max(-1, keepdims=True)
        p = np.exp(attn)
        p = p / p.sum(-1, keepdims=True)
        o = np.einsum("whij,whjd->whid", p, v).transpose(0, 2, 1, 3).reshape(NW, N, C)
        o = o @ w["Wo"][l] + w["bo"][l]
        x = ln(o.reshape(T, C) + x, w["g1"][l], w["b1"][l])
        h = np.maximum(x @ w["W1"][l] + w["bf1"][l], 0.0) @ w["W2"][l] + w["bf2"][l]
        x = ln(h + x, w["g2"][l], w["b2"][l])
    return x


# ---------------------------------------------------------------------------
# kernel() entry point: full inputs -> full output, 8-way batch data parallel
# ---------------------------------------------------------------------------

NCORES = 8
B_FULL = 64
H_RES = W_RES = 24
L_TOK = H_RES * W_RES
NW_FULL = (B_FULL // NCORES) * (H_RES // WS) * (W_RES // WS)   # 32 windows/core
NL_FULL = 3

_COMPILED = {}


def _pack_x_all(x):
    """[64, 576, 512] f32 -> [8, 128, 4, T] bf16 channel-major window-major."""
    b = x.reshape(NCORES, B_FULL // NCORES, 2, WS, 2, WS, 4, 128)
    v = b.transpose(0, 7, 6, 1, 2, 4, 3, 5)   # [core,128,4, b,hw,ww,hs,ws]
    return np.ascontiguousarray(v.reshape(NCORES, 128, 4, -1)
                                ).astype(ml_dtypes.bfloat16)


def _unpack_out_all(res_list):
    """list of [128, 4, T] -> [64, 576, 512] f32."""
    y = np.stack([r.astype(np.float32) for r in res_list])     # [8,128,4,T]
    bpc = B_FULL // NCORES
    v = y.reshape(NCORES, 128, 4, bpc, 2, 2, WS, WS)
    v = v.transpose(0, 3, 4, 6, 5, 7, 2, 1)   # [core,b,hw,hs,ww,ws,4,128]
    return np.ascontiguousarray(v.reshape(B_FULL, L_TOK, C))


def kernel(x, Wq, bq, Wk, bk, Wv, bv, Wo, bo, rpb,
           g1, b1, W1, bf1, W2, bf2, g2, b2):
    import hashlib
    from concourse.bass_utils import run_bass_kernel_spmd

    w = {"Wq": np.asarray(Wq, np.float32), "bq": np.asarray(bq, np.float32),
         "Wk": np.asarray(Wk, np.float32), "bk": np.asarray(bk, np.float32),
         "Wv": np.asarray(Wv, np.float32), "bv": np.asarray(bv, np.float32),
         "Wo": np.asarray(Wo, np.float32), "bo": np.asarray(bo, np.float32),
         "rpb": np.asarray(rpb, np.float32),
         "g1": np.asarray(g1, np.float32), "b1": np.asarray(b1, np.float32),
         "W1": np.asarray(W1, np.float32), "bf1": np.asarray(bf1, np.float32),
         "W2": np.asarray(W2, np.float32), "bf2": np.asarray(bf2, np.float32),
         "g2": np.asarray(g2, np.float32), "b2": np.asarray(b2, np.float32)}
    hsh = hashlib.blake2b(
        b"".join(np.ascontiguousarray(v).tobytes() for v in w.values()),
        digest_size=16).hexdigest()
    if _COMPILED.get("hash") != hsh:
        packed = pack_weights(w, NL_FULL)
        trivial_gb = bool(np.all(w["g1"] == 1) and np.all(w["b1"] == 0)
                          and np.all(w["g2"] == 1) and np.all(w["b2"] == 0))
        trivial_bias = bool(all(np.all(w[k] == 0)
                                for k in ("bq", "bk", "bv", "bo", "bf1", "bf2")))
        nc = bacc.Bacc("TRN2", target_bir_lowering=False, debug=False)
        build(nc, NW_FULL, NL_FULL, packed,
              trivial_gb=trivial_gb, trivial_bias=trivial_bias)
        nc.compile()
        _COMPILED.update(hash=hsh, nc=nc)

    xp = _pack_x_all(np.asarray(x, np.float32))
    in_maps = [{"x": xp[i]} for i in range(NCORES)]
    res = run_bass_kernel_spmd(_COMPILED["nc"], in_maps, list(range(NCORES)))
    return _unpack_out_all([res.results[i]["out"] for i in range(NCORES)])

